# revision 85
# baseline (speedup 1.0000x reference)
"""Multi-head causal attention (B=2, T=2048, E=1024, H=16, D=64) on 8 trn2 cores.

Sharding: tensor-parallel over heads — core c owns heads {2c, 2c+1} (a 128-wide
slice of the hidden dim). Each core computes q/k/v projections for its heads
over the full sequence, causal attention, and a partial output projection
(contraction over its 128 rows of Wo). The host sums the 8 bf16 partials + bias.

v4 (130.7us, from the 150.5us v2), rebuilt around TimelineSim gap blame.
The engine floor is PE ~102us (proj 41 + scores 29 + PV 14.7 + out-proj
13.7 + transposes) with Act exp ~88us; everything else is scheduling:
 - In-place PSUM reciprocal of the Z column + bf16 o_sb/eye/tp (1 cyc/row
   transpose); PV groups emit mask-dependent-group last so the gpsimd
   affine_select triangle mask only gates the final chain-stop matmul.
 - Tails split into 3 pieces (normalize+transpose / O^T copy / out-proj+
   DMA), pair-interleaved, with the C pieces deferred one tail pair so
   every cross-engine hop has real PE work between emit and consume.
 - Proj units carry (deadline=global slot, pieces) and live in a sorted
   queue; deficit + lookahead pacing spends them where the exp pace
   outruns PE, preserving late-deadline units (q/k/v of t7) as endgame
   filler. The prologue hand-interleaves q/k/v per-kc at xT arrival rate
   with weight DMAs slotted between xts, q/k psum borrowed from the idle
   sc banks, and landing copies on the (idle) Act engine.
 - Chunk order (0,*), (1,1), (1,2), (1,0), (1,3): 8 of (1,3)'s off-diag
   score blocks are pre-scored during (1,2)/(1,0) (pt bufs=3) and its
   own wave is two-phase — off-diag scores first (Act-paced, filler-fed),
   then diag scores + all PV waves + tails with every exp in flight, so
   the kernel does not end on an Act-bound stretch. Endgame tails use
   Act-assisted copies (Act is exp-free by then) + immediate out DMAs.
 - PV emission lags scores by 2 blocks; all xt DMA trains enter the
   in-order SP queue before any out-DMA can park it.

PSUM (8 banks): sc 2x[128,2,512] (4) + O|Z accumulators 2x[128,2,2,65] (2) +
mm [128,512] x2 (2, shared by proj / out-proj / transpose tiles).

Timing signal is concourse TimelineSim (no NTFF under this axon client).
"""

import numpy as np
import ml_dtypes
from collections import deque

import concourse.bass as bass
import concourse.tile as tile
from concourse import bacc, mybir
from concourse.bass_utils import run_bass_kernel_spmd
from concourse.masks import make_identity
from contextlib import ExitStack

B, T, E, H, D = 2, 2048, 1024, 16, 64
BT = B * T            # 4096 tokens total
NCORE = 8
KC = E // 128         # contraction chunks for projections = 8
CQ = 512              # tq chunk width
NQB = T // CQ         # tq chunks per batch = 4
NKB = T // 128        # tk blocks per batch = 16

F32 = mybir.dt.float32
BF16 = mybir.dt.bfloat16
AF = mybir.ActivationFunctionType

_cache = {}


def _build():
    nc = bacc.Bacc("TRN2", target_bir_lowering=False, debug=False,
                   num_devices=NCORE)

    xT = nc.dram_tensor("xT", [E, BT], BF16, kind="ExternalInput").ap()
    wq = nc.dram_tensor("wq", [128, E], BF16, kind="ExternalInput").ap()
    wk = nc.dram_tensor("wk", [128, E], BF16, kind="ExternalInput").ap()
    wv = nc.dram_tensor("wv", [128, E], BF16, kind="ExternalInput").ap()
    wo = nc.dram_tensor("wo", [128, E], BF16, kind="ExternalInput").ap()
    out = nc.dram_tensor("out", [BT, E], BF16, kind="ExternalOutput").ap()

    with tile.TileContext(nc) as tc, ExitStack() as ctx:
        pers = ctx.enter_context(tc.tile_pool(name="pers", bufs=1))

        wq_sb = pers.tile([128, KC, 128], BF16, tag="wq")
        wk_sb = pers.tile([128, KC, 128], BF16, tag="wk")
        wv_sb = pers.tile([128, KC, 128], BF16, tag="wv")
        wo_sb = pers.tile([128, E], BF16, tag="wo")
        eye_sb = pers.tile([128, 128], BF16, tag="eye")
        qt_sb = pers.tile([128, BT], BF16, tag="qt")    # [dims(2 heads), tok]
        kt_sb = pers.tile([128, BT], BF16, tag="kt")
        # V natural + ones col per head: [tok%128, blk, h, d|1]; the ones
        # column makes the flipped P^T-stationary PV matmul emit Z = sum(exp)
        # as output column 64 for free.
        v_sb = pers.tile([128, BT // 128, 2, 65], BF16, tag="v")

        # wq queued first on the sync HWDGE queue so the first projection
        # matmul gates on as little DMA as possible; each extra DMA costs
        # ~625ns of serial HWDGE hold, so weights go as single transfers
        # slotted between the xts that need them.
        nc.sync.dma_start(wq_sb[:],
                          wq.rearrange("p (kc d) -> p kc d", kc=KC))
        nc.vector.memset(v_sb[:, :, :, 64:65], 1.0)
        make_identity(nc, eye_sb[:])

        # SBUF pools
        xts_pool = ctx.enter_context(tc.tile_pool(name="xts", bufs=32))
        pt_pool = ctx.enter_context(tc.tile_pool(name="pt", bufs=2))
        osb_pool = ctx.enter_context(tc.tile_pool(name="osb", bufs=3))
        otsb_pool = ctx.enter_context(tc.tile_pool(name="otsb", bufs=3))
        outsb_pool = ctx.enter_context(tc.tile_pool(name="outsb", bufs=3))

        # PSUM pools: 4 + 2 + 2 = 8 banks
        sc_pool = ctx.enter_context(tc.tile_pool(name="sc", bufs=2,
                                                 space="PSUM"))
        acc_pool = ctx.enter_context(tc.tile_pool(name="acc", bufs=1,
                                                  space="PSUM"))
        mm_pool = ctx.enter_context(tc.tile_pool(name="mm", bufs=2,
                                                 space="PSUM"))


        # ---- projection units -------------------------------------------
        def proj_pair_units(t0, dma_hooks=None):
            """t0: even 512-token chunk index (0..6). Issues the pair's xT
            DMAs now; returns 6 unit callbacks (q,k,v) x (hf 0,1).
            dma_hooks: {kc: callback} run right after that kc's xt DMA is
            queued (prologue interleaves weight DMAs at specific points)."""
            xts = []
            for kc in range(KC):
                xt = xts_pool.tile([128, 2 * CQ], BF16, tag="xt",
                                   name=f"xt_{t0}_{kc}")
                nc.sync.dma_start(
                    xt[:], xT[kc * 128:(kc + 1) * 128,
                              t0 * CQ:(t0 + 2) * CQ])
                if dma_hooks and kc in dma_hooks:
                    dma_hooks[kc]()
                xts.append(xt)

            def qk_unit(w_sb, dst_sb, hf):
                # two ~850ns halves so filler interleaves finely with waves
                t_ = t0 + hf
                state = {}
                def emit_a():
                    ps = mm_pool.tile([128, CQ], F32, tag="mm",
                                      name=f"qkps{t_}_{id(w_sb)}")
                    state["ps"] = ps
                    for kc in range(KC // 2):
                        nc.tensor.matmul(
                            ps[:], w_sb[:, kc],
                            xts[kc][:, hf * CQ:(hf + 1) * CQ],
                            start=(kc == 0), stop=False)
                def emit_b():
                    ps = state["ps"]
                    for kc in range(KC // 2, KC):
                        nc.tensor.matmul(
                            ps[:], w_sb[:, kc],
                            xts[kc][:, hf * CQ:(hf + 1) * CQ],
                            start=False, stop=(kc == KC - 1))
                    nc.vector.tensor_copy(
                        dst_sb[:, t_ * CQ:(t_ + 1) * CQ], ps[:])
                return [emit_a, emit_b]

            def v_unit(hf):
                t_ = t0 + hf
                state = {}
                def emit_a():
                    v_ps = mm_pool.tile([128, CQ], F32, tag="mm",
                                        name=f"vps{t_}")
                    state["ps"] = v_ps
                    for j in (0, 1):
                        jf = hf * CQ + j * 128
                        for kc in range(KC):
                            nc.tensor.matmul(
                                v_ps[:, j * 128:(j + 1) * 128],
                                xts[kc][:, jf:jf + 128],
                                wv_sb[:, kc], start=(kc == 0),
                                stop=(kc == KC - 1))
                def emit_b():
                    v_ps = state["ps"]
                    for j in (2, 3):
                        jf = hf * CQ + j * 128
                        for kc in range(KC):
                            nc.tensor.matmul(
                                v_ps[:, j * 128:(j + 1) * 128],
                                xts[kc][:, jf:jf + 128],
                                wv_sb[:, kc], start=(kc == 0),
                                stop=(kc == KC - 1))
                    b4 = t_ * (CQ // 128)
                    nc.vector.tensor_copy(
                        v_sb[:, b4:b4 + 4, :, 0:64],
                        v_ps[:].rearrange("p (j h v) -> p j h v",
                                          j=4, h=2))
                return [emit_a, emit_b]

            units = (qk_unit(wq_sb, qt_sb, 0) + qk_unit(wk_sb, kt_sb, 0) +
                     v_unit(0) + qk_unit(wq_sb, qt_sb, 1) +
                     qk_unit(wk_sb, kt_sb, 1) + v_unit(1))
            return units, xts

        # ---- filler machinery -------------------------------------------
        # proj_q entries are (deadline_slot, seq, [piece_a, piece_b]): the
        # unit MUST be emitted before the global attention slot that
        # consumes its tokens (a later emission would deadlock the in-order
        # PE queue). Kept sorted by deadline so deficit-paced pops
        # naturally preserve the latest-deadline units as an endgame
        # reserve.
        tails_q = deque()
        pending_cs = []    # tail C (out-proj) pieces deferred one pair
        proj_q = []
        held = []          # pending b-half of a split proj unit (must pop
                           # before any other mm-pool user)
        sched = {"deficit": 0.0, "seq": 0}
        dma_pending = []   # (dram_slice, sbuf_tile): out DMAs deferred one
                           # tail so the SP queue never stalls on copy sems

        PROJ_NS = 853.0    # PE ns per proj half-piece (4 matmuls x 512)

        def flush_out_dma():
            while dma_pending:
                dst, src = dma_pending.pop(0)
                nc.sync.dma_start(dst, src)

        def queue_unit(dead, pieces):
            proj_q.append((dead, sched["seq"], pieces))
            sched["seq"] += 1
            proj_q.sort(key=lambda e: (e[0], e[1]))

        def pop_proj_piece():
            if held:
                held.pop()()
            else:
                _, _, pieces = proj_q.pop(0)
                pieces[0]()
                held.append(pieces[1])
            sched["deficit"] -= PROJ_NS

        def drain_tails():
            # interleave a proj piece between tail pieces so their
            # cross-engine latency chains overlap real PE work
            tails_q.extend(pending_cs)
            pending_cs.clear()
            while tails_q:
                pe_ns, fn = tails_q.popleft()
                fn()
                sched["deficit"] -= pe_ns
                if tails_q and (held or proj_q) and sched["deficit"] > -800:
                    pop_proj_piece()

        def force_proj_upto(slot):
            while held or (proj_q and proj_q[0][0] <= slot):
                pop_proj_piece()

        # ---- prologue ----------------------------------------------------
        # Weight DMAs slot between the pair-0 xT DMAs (wk after xt0, wv
        # after xt2 — each lands just before its first consumer) and
        # q/k/v matmuls interleave per-kc at xT arrival granularity so the
        # PE streams at DMA rate with no burst stalls.
        hooks = {
            0: lambda: nc.sync.dma_start(
                wk_sb[:], wk.rearrange("p (kc d) -> p kc d", kc=KC)),
            2: lambda: nc.sync.dma_start(
                wv_sb[:], wv.rearrange("p (kc d) -> p kc d", kc=KC)),
            7: lambda: nc.sync.dma_start(wo_sb[:], wo[:]),
        }
        units0, xts0 = proj_pair_units(0, dma_hooks=hooks)

        # tokens 0..511: q/k psum tiles borrow the (still idle) sc tag's
        # banks so mm_pool stays free for the interleaved v chains.
        q_ps0 = sc_pool.tile([128, CQ], F32, tag="sc", name="qps_pro")
        k_ps0 = sc_pool.tile([128, CQ], F32, tag="sc", name="kps_pro")
        v_ps0 = mm_pool.tile([128, CQ], F32, tag="mm", name="vps_pro")
        v_started_cell = [False]

        def pro_v(kc):
            # interleaved per-j chains on one bank: only the very first
            # matmul clears the bank's has_written bits (start=True); the
            # other chains' kc==0 matmuls overwrite-where-bit-clear
            for j in range(4):
                nc.tensor.matmul(
                    v_ps0[:, j * 128:(j + 1) * 128],
                    xts0[kc][:, j * 128:(j + 1) * 128],
                    wv_sb[:, kc], start=not v_started_cell[0],
                    stop=(kc == KC - 1), skip_group_check=True)
                v_started_cell[0] = True
        for kc in range(KC):
            nc.tensor.matmul(q_ps0[:], wq_sb[:, kc], xts0[kc][:, 0:CQ],
                             start=(kc == 0), stop=(kc == KC - 1),
                             skip_group_check=True)
            nc.tensor.matmul(k_ps0[:], wk_sb[:, kc], xts0[kc][:, 0:CQ],
                             start=(kc == 0), stop=(kc == KC - 1),
                             skip_group_check=True)
            if kc >= 2:
                pro_v(kc - 2)
        # Act is idle until the first exp (~12us): give it the prologue
        # landing copies so DVE stays clear for the hf=1 unit copies
        nc.scalar.copy(qt_sb[:, 0:CQ], q_ps0[:])
        nc.scalar.copy(kt_sb[:, 0:CQ], k_ps0[:])
        for kc in range(KC - 2, KC):
            pro_v(kc)
        nc.scalar.copy(
            v_sb[:, 0:4, :, 0:64],
            v_ps0[:].rearrange("p (j h v) -> p j h v", j=4, h=2))

        # Chunk order: batch-1 runs [c1, c2, c0, c3] so the kernel ends on
        # the 16-block (1,3) chunk, whose late score blocks (kb>=12) keep
        # q/k/v(t7) units as deadline-reserved PE filler for the Act-paced
        # endgame, instead of draining tails against an empty proj queue.
        chunk_list = [(0, 0), (0, 1), (0, 2), (0, 3),
                      (1, 1), (1, 2), (1, 0), (1, 3)]
        nblks = [4 * (cc + 1) for _, cc in chunk_list]
        base = [0]
        for n in nblks:
            base.append(base[-1] + n)

        # global-slot deadlines: q(t) needed at its chunk's first slot;
        # k/v(t) first consumed when the score wave reaches keys t (slot
        # 4*(t%4) of the earliest chunk with c >= t%4 in list order)
        Q_DEAD = {1: base[1], 2: base[2], 3: base[3],
                  4: base[6], 5: base[4], 6: base[5],
                  7: base[5] + 8}  # q(t7) before the (1,3) pre-scoring
        KV_DEAD = {1: base[1] + 4, 2: base[2] + 8, 3: base[3] + 12,
                   4: base[4], 5: base[4] + 4, 6: base[5] + 8,
                   7: base[7] + 12}

        # v(t7) reserved two slots past k(t7): it pads the endgame's
        # diagonal-score bank rotation and is forced before pv(12) uses it
        V_DEAD = dict(KV_DEAD)
        V_DEAD[7] = base[7] + 14

        def queue_half_units(t, units6):
            queue_unit(Q_DEAD[t], units6[0:2])    # q a/b
            queue_unit(KV_DEAD[t], units6[2:4])   # k a/b
            queue_unit(V_DEAD[t], units6[4:6])    # v a/b

        def queue_pair_units(t0, units):
            queue_half_units(t0, units[:6])
            queue_half_units(t0 + 1, units[6:])

        queue_half_units(1, units0[6:])  # prologue pair: hf=1 only

        # pair creation: issue xT DMAs early — critically, ALL xt trains
        # must enter the in-order SP queue before any mid-kernel out-DMA
        # can park it (an out-DMA whose copy isn't ready blocks the queue
        # for many us, which starved later projections via late xts)
        pair_create = {1: [2], 2: [4, 6]}

        # pt tiles on demand so a later chunk's score wave can start while
        # an earlier chunk is still draining (pre-scoring)
        pts = {}

        def get_pt(b_, c_):
            if (b_, c_) not in pts:
                pts[(b_, c_)] = pt_pool.tile(
                    [128, NKB, 2, CQ], BF16, tag="pt", name=f"pt_{b_}_{c_}")
            return pts[(b_, c_)]

        def emit_scores_g(b_, c_, kb):
            pt = get_pt(b_, c_)
            tb_ = b_ * T
            tq0_ = c_ * CQ
            f0 = max(0, 128 * (kb - 4 * c_))
            sc = sc_pool.tile([128, 2, CQ], F32, tag="sc",
                              name=f"sc_{b_}_{c_}_{kb}")
            tk0 = kb * 128
            for h in range(2):
                hs = slice(h * 64, (h + 1) * 64)
                nc.tensor.matmul(
                    sc[:, h, f0:CQ],
                    kt_sb[hs, tb_ + tk0:tb_ + tk0 + 128],
                    qt_sb[hs, tb_ + tq0_ + f0:tb_ + tq0_ + CQ],
                    start=True, stop=True)
            nc.scalar.activation(
                pt[:, kb, :, f0:CQ], sc[:, :, f0:CQ],
                AF.Exp, scale=float(D) ** -0.5)
            if kb - 4 * c_ >= 0:  # diagonal block: causal triangle mask
                for h in range(2):
                    # keep where tq >= tk (f - p >= 0), else 0
                    nc.gpsimd.affine_select(
                        out=pt[:, kb, h, f0:f0 + 128],
                        in_=pt[:, kb, h, f0:f0 + 128],
                        compare_op=mybir.AluOpType.is_ge,
                        fill=0.0, base=0,
                        pattern=[[1, 128]], channel_multiplier=-1)
            return f0

        NPRE = 8   # (1,3) blocks pre-scored during (1,2)+(1,0)

        for ci, (b, c) in enumerate(chunk_list):
            for t0 in pair_create.get(ci, []):
                units, _ = proj_pair_units(t0)
                queue_pair_units(t0, units)

            tb = b * T
            tq0 = c * CQ
            nblk = 4 * (c + 1)
            pt = get_pt(b, c)
            # per-chunk O|Z accumulators [tq, gsub, h, d|Z]: pool rotation
            # (bufs=1) orders the next chunk's first PV write after this
            # chunk's tail reads
            o_ps = [acc_pool.tile([128, 2, 2, 65], F32, tag=f"o{i}",
                                  name=f"o_ps{i}_{b}_{c}")
                    for i in range(2)]
            zr_tiles = {}

            def make_tail_pieces(g, b=b, c=c, tb=tb, tq0=tq0, o_ps=o_ps):
                """Tail split into 3 pieces so the PE->DVE->PE->DVE chain of
                one tail interleaves with its pair partner + proj filler
                instead of stalling the in-order PE stream. In the final
                chunk the Act engine (done with exps by tail time) takes
                half the copies so DVE isn't the serial drain resource."""
                op = o_ps[g // 2]
                gs = g % 2
                act_assist = (b, c) == chunk_list[-1]
                state = {}

                def piece_a():   # normalize + transpose (PE 53ns)
                    if act_assist:
                        # 1/Z lands in SBUF so the Act engine can use it as
                        # an activation scale (scale APs must be SBUF)
                        if gs == 0:
                            zr = osb_pool.tile([128, 2, 2], F32, tag="zr",
                                               name=f"zr_{b}_{c}_{g}")
                            nc.vector.reciprocal(zr[:], op[:, :, :, 64])
                            zr_tiles[g // 2] = zr
                        zr = zr_tiles[g // 2]
                    elif gs == 0:
                        # 1/Z for the group pair, in place in PSUM col 64
                        # (both chains have stopped by emission time)
                        nc.vector.reciprocal(op[:, :, :, 64],
                                             op[:, :, :, 64])
                    o_sb = osb_pool.tile([128, 128], BF16, tag="osb",
                                         name=f"osb_{b}_{c}_{g}")
                    for h in range(2):
                        if act_assist and h == 1:
                            nc.scalar.activation(
                                o_sb[:, 64:128], op[:, gs, 1, 0:64],
                                AF.Copy, scale=zr[:, gs, 1:2])
                        elif act_assist:
                            nc.vector.tensor_scalar_mul(
                                o_sb[:, h * 64:(h + 1) * 64],
                                op[:, gs, h, 0:64],
                                zr[:, gs, h:h + 1])
                        else:
                            nc.vector.tensor_scalar_mul(
                                o_sb[:, h * 64:(h + 1) * 64],
                                op[:, gs, h, 0:64],
                                op[:, gs, h, 64:65])
                    tp = mm_pool.tile([128, 512], BF16, tag="mm",
                                      name=f"tp_{b}_{c}_{g}")
                    nc.tensor.transpose(tp[:, 0:128], o_sb[:], eye_sb[:])
                    state["tp"] = tp

                def piece_b():   # O^T landing copy (no PE)
                    ot_sb = otsb_pool.tile([128, 128], BF16, tag="otsb",
                                           name=f"otsb_{b}_{c}_{g}")
                    if act_assist:
                        nc.scalar.copy(ot_sb[:], state["tp"][:, 0:128])
                    else:
                        nc.vector.tensor_copy(ot_sb[:], state["tp"][:, 0:128])
                    state["ot"] = ot_sb

                def piece_c():   # output projection + copies + DMA (PE 426)
                    out_sb = outsb_pool.tile([128, E], BF16, tag="outsb",
                                             name=f"outsb_{b}_{c}_{g}")
                    tqg = tb + tq0 + g * 128
                    for eh in range(2):
                        ops = mm_pool.tile([128, 512], F32, tag="mm",
                                           name=f"ops_{b}_{c}_{g}_{eh}")
                        nc.tensor.matmul(
                            ops[:], state["ot"][:],
                            wo_sb[:, eh * 512:(eh + 1) * 512],
                            start=True, stop=True)
                        if (c <= 1 or act_assist) and eh == 1:
                            # Act's light window (short chunks / endgame)
                            nc.scalar.copy(
                                out_sb[:, eh * 512:(eh + 1) * 512], ops[:])
                        else:
                            nc.vector.tensor_copy(
                                out_sb[:, eh * 512:(eh + 1) * 512], ops[:])
                        if act_assist:
                            # endgame: SP is idle — issue half-row DMAs the
                            # moment each copy lands to shorten the drain
                            nc.sync.dma_start(
                                out[tqg:tqg + 128,
                                    eh * 512:(eh + 1) * 512],
                                out_sb[:, eh * 512:(eh + 1) * 512])
                    if not act_assist:
                        flush_out_dma()
                        dma_pending.append(
                            (out[tqg:tqg + 128, :], out_sb[:]))

                return [(53.0, piece_a), (0.0, piece_b), (426.0, piece_c)]

            # PSUM has_written bits: a start=True matmul clears them for the
            # WHOLE bank, so only the first PV matmul per o_ps bank per chunk
            # may use start=True. Later chains' first matmuls (kb==0,
            # start=False) overwrite-where-bit-clear, then accumulate.
            bank_started = [False, False]

            def pv_block(kb, b=b, c=c, pt=pt, o_ps=o_ps,
                         bank_started=bank_started):
                j0 = max(0, kb - 4 * c)
                # diagonal block: group j0's stationary is the masked pt
                # sub-block — emit it LAST so the gpsimd mask only gates the
                # final chain-stop matmul, not the whole block
                gs_order = list(range(j0, NQB))
                if kb - 4 * c >= 0 and len(gs_order) > 1:
                    gs_order = gs_order[1:] + gs_order[:1]
                for g in gs_order:
                    for h in range(2):
                        st = not bank_started[g // 2]
                        bank_started[g // 2] = True
                        nc.tensor.matmul(
                            o_ps[g // 2][:, g % 2, h, :],
                            pt[:, kb, h, g * 128:(g + 1) * 128],
                            v_sb[:, b * NKB + kb, h],
                            start=st, stop=(kb == 4 * c + g),
                            skip_group_check=True)
                j = kb - 4 * c
                if j in (1, 3):  # group pair's chains complete
                    pa = make_tail_pieces(j - 1)
                    pb = make_tail_pieces(j)
                    # interleave A A' B B' now; defer the C (out-proj)
                    # pieces until the NEXT pair so piece_b's DVE copy has
                    # landed long before C's Ldweights needs it
                    inter = [pa[0]]
                    if pending_cs:
                        inter.append(pending_cs.pop(0))
                    inter.append(pb[0])
                    if pending_cs:
                        inter.append(pending_cs.pop(0))
                    inter += [pa[1], pb[1]]
                    tails_q.extend(inter)
                    pending_cs.extend([pa[2], pb[2]])
                return (NQB - j0) * 2 * 65

            def emit_scores(kb):
                return emit_scores_g(b, c, kb)

            def pace(act_ns, pe_ns, slot):
                # deficit-paced filler: keep the PE fed during Act-paced
                # stretches, spend queued proj/tail work exactly where the
                # per-block PE emission falls short of the exp pace.
                sched["deficit"] += act_ns - pe_ns
                sched["deficit"] = max(-2000.0,
                                       min(sched["deficit"], 8000.0))
                # lookahead spread: don't let deadline-bound units burst
                if proj_q and proj_q[0][0] <= slot + 3:
                    pop_proj_piece()
                    if held:
                        pop_proj_piece()
                # alternate tail/proj pops so tail latency chains overlap
                # real PE work instead of stalling the in-order PE stream
                prefer_tail = True
                while sched["deficit"] > 400 and (tails_q or held or proj_q):
                    if prefer_tail and tails_q and not held:
                        pe_ns2, fn = tails_q.popleft()
                        fn()
                        sched["deficit"] -= pe_ns2
                    elif held or proj_q:
                        pop_proj_piece()
                    else:
                        pe_ns2, fn = tails_q.popleft()
                        fn()
                        sched["deficit"] -= pe_ns2
                    prefer_tail = not prefer_tail
                if held:   # never end a slot mid-unit
                    pop_proj_piece()

            def chunk_prefix(kb):
                if kb == 1:
                    # PE meat between sc(0)/exp(0) and the exp-gated
                    # pv(0), then the prev chunk's tail pieces
                    if held or proj_q:
                        pop_proj_piece()
                    drain_tails()

            if ci < len(chunk_list) - 1:
                for kb in range(nblk):
                    # units whose tokens this slot consumes: emit them now
                    force_proj_upto(base[ci] + kb)
                    f0 = emit_scores(kb)
                    pv_cyc = 0
                    if kb >= 1:
                        chunk_prefix(kb)
                        if kb >= 2:
                            pv_cyc = pv_block(kb - 2)
                    pace((2 * (CQ - f0) + 222) / 1.2,
                         (2 * (CQ - f0) + pv_cyc) * 0.4167, base[ci] + kb)
                    if ci == 5 and kb >= nblk - 4:
                        # pre-score an off-diagonal (1,3) block: shifts Act
                        # load out of the Act-saturated endgame
                        emit_scores_g(1, 3, kb - (nblk - 4))
                        pace((2 * CQ + 222) / 1.2, 2 * CQ * 0.4167,
                             base[ci] + kb)
                    if ci == 6 and NPRE == 8:
                        # four more during the Act-light (1,0) chunk
                        emit_scores_g(1, 3, 4 + kb)
                        pace((2 * CQ + 222) / 1.2, 2 * CQ * 0.4167,
                             base[ci] + kb)
                pv_block(nblk - 2)
                pv_block(nblk - 1)
            else:
                # Final chunk, two-phase so the kernel does not end on an
                # Act-bound exp wave:
                # phase A pre-scores the 12 off-diagonal blocks (Act paced,
                # PE kept busy by the deadline-reserved proj filler);
                # phase B runs the diagonal scores + every PV wave + tails
                # with all exps already in flight or done.
                for kb in range(NPRE, 12):
                    force_proj_upto(base[ci] + kb)
                    emit_scores(kb)
                    if kb == NPRE + 1:
                        # prev chunk's tails must fully emit before phase
                        # B's pv(0) rotates into its o_ps banks
                        if held or proj_q:
                            pop_proj_piece()
                        drain_tails()
                    pace((2 * CQ + 222) / 1.2, 2 * CQ * 0.4167,
                         base[ci] + kb)
                force_proj_upto(base[ci] + 12)  # k(t7) ahead of the scores
                emit_scores(12)
                emit_scores(13)
                for kb in range(0, 4):
                    pv_block(kb)
                if held or proj_q:   # v(t7) a: pads exp(12)'s bank WAR
                    pop_proj_piece()
                emit_scores(14)
                for kb in range(4, 8):
                    pv_block(kb)
                if held or proj_q:   # v(t7) b: pads exp(13)'s bank WAR
                    pop_proj_piece()
                emit_scores(15)
                force_proj_upto(base[ci] + 14)  # v(t7) before pv(12)
                for kb in range(8, 16):
                    pv_block(kb)

        drain_tails()
        while held or proj_q:
            pop_proj_piece()
        flush_out_dma()

    nc.compile()
    return nc


def _host_prep(x, Wq, Wk, Wv, Wo):
    bf = ml_dtypes.bfloat16
    xT = np.ascontiguousarray(
        np.asarray(x, dtype=np.float32).reshape(BT, E).T).astype(bf)

    def perm(w):
        # [E, 128] -> [128p, kc, 128d] flattened: w[kc*128+p, d] -> out[p, kc, d]
        return np.ascontiguousarray(
            w.reshape(KC, 128, 128).transpose(1, 0, 2).reshape(128, E)).astype(bf)

    Wq = np.asarray(Wq, dtype=np.float32)
    Wk = np.asarray(Wk, dtype=np.float32)
    Wv = np.asarray(Wv, dtype=np.float32)
    Wo = np.asarray(Wo, dtype=np.float32)

    in_maps = []
    for c in range(NCORE):
        sl = slice(c * 128, (c + 1) * 128)
        in_maps.append({
            "xT": xT,
            "wq": perm(Wq[:, sl]),
            "wk": perm(Wk[:, sl]),
            "wv": perm(Wv[:, sl]),
            "wo": np.ascontiguousarray(Wo[sl, :]).astype(bf),
        })
    return in_maps


def kernel(x, Wq, Wk, Wv, Wo, bo, _trace=False, _trace_kwargs=None):
    if "nc" not in _cache:
        _cache["nc"] = _build()
    nc = _cache["nc"]

    in_maps = _host_prep(x, Wq, Wk, Wv, Wo)
    kw = {}
    if _trace:
        kw = dict(trace=True, trace_cores=[0], **(_trace_kwargs or {}))
    res = run_bass_kernel_spmd(nc, in_maps, core_ids=list(range(NCORE)), **kw)
    _cache["last_result"] = res

    total = np.zeros((BT, E), dtype=np.float32)
    for r in res.results:
        total += np.asarray(r["out"], dtype=np.float32)
    total += np.asarray(bo, dtype=np.float32)[None, :]
    return total.reshape(B, T, E)



# revision 87
# speedup vs baseline: 1.0135x; 1.0135x over previous
"""Multi-head causal attention (B=2, T=2048, E=1024, H=16, D=64) on 8 trn2 cores.

Sharding: tensor-parallel over heads — core c owns heads {2c, 2c+1} (a 128-wide
slice of the hidden dim). Each core computes q/k/v projections for its heads
over the full sequence, causal attention, and a partial output projection
(contraction over its 128 rows of Wo). The host sums the 8 bf16 partials + bias.

v4 (130.7us, from the 150.5us v2), rebuilt around TimelineSim gap blame.
The engine floor is PE ~102us (proj 41 + scores 29 + PV 14.7 + out-proj
13.7 + transposes) with Act exp ~88us; everything else is scheduling:
 - In-place PSUM reciprocal of the Z column + bf16 o_sb/eye/tp (1 cyc/row
   transpose); PV groups emit mask-dependent-group last so the gpsimd
   affine_select triangle mask only gates the final chain-stop matmul.
 - Tails split into 3 pieces (normalize+transpose / O^T copy / out-proj+
   DMA), pair-interleaved, with the C pieces deferred one tail pair so
   every cross-engine hop has real PE work between emit and consume.
 - Proj units carry (deadline=global slot, pieces) and live in a sorted
   queue; deficit + lookahead pacing spends them where the exp pace
   outruns PE, preserving late-deadline units (q/k/v of t7) as endgame
   filler. The prologue hand-interleaves q/k/v per-kc at xT arrival rate
   with weight DMAs slotted between xts, q/k psum borrowed from the idle
   sc banks, and landing copies on the (idle) Act engine.
 - Chunk order (0,*), (1,1), (1,2), (1,0), (1,3): 8 of (1,3)'s off-diag
   score blocks are pre-scored during (1,2)/(1,0) (pt bufs=3) and its
   own wave is two-phase — off-diag scores first (Act-paced, filler-fed),
   then diag scores + all PV waves + tails with every exp in flight, so
   the kernel does not end on an Act-bound stretch. Endgame tails use
   Act-assisted copies (Act is exp-free by then) + immediate out DMAs.
 - PV emission lags scores by 2 blocks; all xt DMA trains enter the
   in-order SP queue before any out-DMA can park it.

PSUM (8 banks): sc 2x[128,2,512] (4) + O|Z accumulators 2x[128,2,2,65] (2) +
mm [128,512] x2 (2, shared by proj / out-proj / transpose tiles).

Timing signal is concourse TimelineSim (no NTFF under this axon client).
"""

import numpy as np
import ml_dtypes
from collections import deque

import concourse.bass as bass
import concourse.tile as tile
from concourse import bacc, mybir
from concourse.bass_utils import run_bass_kernel_spmd
from concourse.masks import make_identity
from contextlib import ExitStack

B, T, E, H, D = 2, 2048, 1024, 16, 64
BT = B * T            # 4096 tokens total
NCORE = 8
KC = E // 128         # contraction chunks for projections = 8
CQ = 512              # tq chunk width
NQB = T // CQ         # tq chunks per batch = 4
NKB = T // 128        # tk blocks per batch = 16

F32 = mybir.dt.float32
BF16 = mybir.dt.bfloat16
AF = mybir.ActivationFunctionType

_cache = {}


def _build():
    nc = bacc.Bacc("TRN2", target_bir_lowering=False, debug=False,
                   num_devices=NCORE)

    xT = nc.dram_tensor("xT", [E, BT], BF16, kind="ExternalInput").ap()
    wq = nc.dram_tensor("wq", [128, E], BF16, kind="ExternalInput").ap()
    wk = nc.dram_tensor("wk", [128, E], BF16, kind="ExternalInput").ap()
    wv = nc.dram_tensor("wv", [128, E], BF16, kind="ExternalInput").ap()
    wo = nc.dram_tensor("wo", [128, E], BF16, kind="ExternalInput").ap()
    out = nc.dram_tensor("out", [BT, E], BF16, kind="ExternalOutput").ap()

    with tile.TileContext(nc) as tc, ExitStack() as ctx:
        pers = ctx.enter_context(tc.tile_pool(name="pers", bufs=1))

        wq_sb = pers.tile([128, KC, 128], BF16, tag="wq")
        wk_sb = pers.tile([128, KC, 128], BF16, tag="wk")
        wv_sb = pers.tile([128, KC, 128], BF16, tag="wv")
        wo_sb = pers.tile([128, E], BF16, tag="wo")
        eye_sb = pers.tile([128, 128], BF16, tag="eye")
        qt_sb = pers.tile([128, BT], BF16, tag="qt")    # [dims(2 heads), tok]
        kt_sb = pers.tile([128, BT], BF16, tag="kt")
        # V natural + ones col per head: [tok%128, blk, h, d|1]; the ones
        # column makes the flipped P^T-stationary PV matmul emit Z = sum(exp)
        # as output column 64 for free.
        v_sb = pers.tile([128, BT // 128, 2, 65], BF16, tag="v")

        # wq queued first on the sync HWDGE queue so the first projection
        # matmul gates on as little DMA as possible; each extra DMA costs
        # ~625ns of serial HWDGE hold, so weights go as single transfers
        # slotted between the xts that need them.
        wq_r = wq.rearrange("p (kc d) -> p kc d", kc=KC)
        nc.sync.dma_start(wq_sb[:, 0:KC // 2], wq_r[:, 0:KC // 2])
        nc.vector.memset(v_sb[:, :, :, 64:65], 1.0)
        make_identity(nc, eye_sb[:])

        # SBUF pools
        xts_pool = ctx.enter_context(tc.tile_pool(name="xts", bufs=32))
        pt_pool = ctx.enter_context(tc.tile_pool(name="pt", bufs=2))
        osb_pool = ctx.enter_context(tc.tile_pool(name="osb", bufs=4))
        otsb_pool = ctx.enter_context(tc.tile_pool(name="otsb", bufs=4))
        outsb_pool = ctx.enter_context(tc.tile_pool(name="outsb", bufs=4))

        # PSUM pools: 4 + 2 + 2 = 8 banks
        sc_pool = ctx.enter_context(tc.tile_pool(name="sc", bufs=2,
                                                 space="PSUM"))
        acc_pool = ctx.enter_context(tc.tile_pool(name="acc", bufs=1,
                                                  space="PSUM"))
        mm_pool = ctx.enter_context(tc.tile_pool(name="mm", bufs=2,
                                                 space="PSUM"))


        # ---- projection units -------------------------------------------
        def proj_pair_units(t0, dma_hooks=None):
            """t0: even 512-token chunk index (0..6). Issues the pair's xT
            DMAs now; returns 6 unit callbacks (q,k,v) x (hf 0,1).
            dma_hooks: {kc: callback} run right after that kc's xt DMA is
            queued (prologue interleaves weight DMAs at specific points)."""
            xts = []
            for kc in range(KC):
                xt = xts_pool.tile([128, 2 * CQ], BF16, tag="xt",
                                   name=f"xt_{t0}_{kc}")
                nc.sync.dma_start(
                    xt[:], xT[kc * 128:(kc + 1) * 128,
                              t0 * CQ:(t0 + 2) * CQ])
                if dma_hooks and kc in dma_hooks:
                    dma_hooks[kc]()
                xts.append(xt)

            def qk_unit(w_sb, dst_sb, hf):
                # two ~850ns halves so filler interleaves finely with waves
                t_ = t0 + hf
                state = {}
                def emit_a():
                    ps = mm_pool.tile([128, CQ], F32, tag="mm",
                                      name=f"qkps{t_}_{id(w_sb)}")
                    state["ps"] = ps
                    for kc in range(KC // 2):
                        nc.tensor.matmul(
                            ps[:], w_sb[:, kc],
                            xts[kc][:, hf * CQ:(hf + 1) * CQ],
                            start=(kc == 0), stop=False)
                def emit_b():
                    ps = state["ps"]
                    for kc in range(KC // 2, KC):
                        nc.tensor.matmul(
                            ps[:], w_sb[:, kc],
                            xts[kc][:, hf * CQ:(hf + 1) * CQ],
                            start=False, stop=(kc == KC - 1))
                    nc.vector.tensor_copy(
                        dst_sb[:, t_ * CQ:(t_ + 1) * CQ], ps[:])
                return [emit_a, emit_b]

            def v_unit(hf):
                t_ = t0 + hf
                state = {}
                def emit_a():
                    v_ps = mm_pool.tile([128, CQ], F32, tag="mm",
                                        name=f"vps{t_}")
                    state["ps"] = v_ps
                    for j in (0, 1):
                        jf = hf * CQ + j * 128
                        for kc in range(KC):
                            nc.tensor.matmul(
                                v_ps[:, j * 128:(j + 1) * 128],
                                xts[kc][:, jf:jf + 128],
                                wv_sb[:, kc], start=(kc == 0),
                                stop=(kc == KC - 1))
                def emit_b():
                    v_ps = state["ps"]
                    for j in (2, 3):
                        jf = hf * CQ + j * 128
                        for kc in range(KC):
                            nc.tensor.matmul(
                                v_ps[:, j * 128:(j + 1) * 128],
                                xts[kc][:, jf:jf + 128],
                                wv_sb[:, kc], start=(kc == 0),
                                stop=(kc == KC - 1))
                    b4 = t_ * (CQ // 128)
                    nc.vector.tensor_copy(
                        v_sb[:, b4:b4 + 4, :, 0:64],
                        v_ps[:].rearrange("p (j h v) -> p j h v",
                                          j=4, h=2))
                return [emit_a, emit_b]

            units = (qk_unit(wq_sb, qt_sb, 0) + qk_unit(wk_sb, kt_sb, 0) +
                     v_unit(0) + qk_unit(wq_sb, qt_sb, 1) +
                     qk_unit(wk_sb, kt_sb, 1) + v_unit(1))
            return units, xts

        # ---- filler machinery -------------------------------------------
        # proj_q entries are (deadline_slot, seq, [piece_a, piece_b]): the
        # unit MUST be emitted before the global attention slot that
        # consumes its tokens (a later emission would deadlock the in-order
        # PE queue). Kept sorted by deadline so deficit-paced pops
        # naturally preserve the latest-deadline units as an endgame
        # reserve.
        tails_q = deque()
        pending_cs = []    # tail C (out-proj) pieces deferred one pair
        proj_q = []
        held = []          # pending b-half of a split proj unit (must pop
                           # before any other mm-pool user)
        sched = {"deficit": 0.0, "seq": 0}
        dma_pending = []   # (dram_slice, sbuf_tile): out DMAs deferred one
                           # tail so the SP queue never stalls on copy sems

        PROJ_NS = 853.0    # PE ns per proj half-piece (4 matmuls x 512)

        def flush_out_dma():
            while dma_pending:
                dst, src = dma_pending.pop(0)
                nc.sync.dma_start(dst, src)

        def queue_unit(dead, pieces):
            proj_q.append((dead, sched["seq"], pieces))
            sched["seq"] += 1
            proj_q.sort(key=lambda e: (e[0], e[1]))

        def pop_proj_piece():
            if held:
                held.pop()()
            else:
                _, _, pieces = proj_q.pop(0)
                pieces[0]()
                held.append(pieces[1])
            sched["deficit"] -= PROJ_NS

        def drain_tails():
            # interleave a proj piece between tail pieces so their
            # cross-engine latency chains overlap real PE work
            tails_q.extend(pending_cs)
            pending_cs.clear()
            while tails_q:
                pe_ns, fn = tails_q.popleft()
                fn()
                sched["deficit"] -= pe_ns
                if tails_q and (held or proj_q) and sched["deficit"] > -800:
                    pop_proj_piece()

        def force_proj_upto(slot):
            while held or (proj_q and proj_q[0][0] <= slot):
                pop_proj_piece()

        # ---- prologue ----------------------------------------------------
        # Weight DMAs slot between the pair-0 xT DMAs (wk after xt0, wv
        # after xt2 — each lands just before its first consumer) and
        # q/k/v matmuls interleave per-kc at xT arrival granularity so the
        # PE streams at DMA rate with no burst stalls.
        hooks = {
            0: lambda: (nc.sync.dma_start(wq_sb[:, KC // 2:KC],
                                          wq_r[:, KC // 2:KC]),
                        nc.sync.dma_start(
                wk_sb[:], wk.rearrange("p (kc d) -> p kc d", kc=KC))),
            2: lambda: nc.sync.dma_start(
                wv_sb[:], wv.rearrange("p (kc d) -> p kc d", kc=KC)),
            7: lambda: nc.sync.dma_start(wo_sb[:], wo[:]),
        }
        units0, xts0 = proj_pair_units(0, dma_hooks=hooks)

        # tokens 0..511: q/k psum tiles borrow the (still idle) sc tag's
        # banks so mm_pool stays free for the interleaved v chains.
        q_ps0 = sc_pool.tile([128, CQ], F32, tag="sc", name="qps_pro")
        k_ps0 = sc_pool.tile([128, CQ], F32, tag="sc", name="kps_pro")
        v_ps0 = mm_pool.tile([128, CQ], F32, tag="mm", name="vps_pro")
        v_started_cell = [False]

        def pro_v(kc):
            # interleaved per-j chains on one bank: only the very first
            # matmul clears the bank's has_written bits (start=True); the
            # other chains' kc==0 matmuls overwrite-where-bit-clear
            for j in range(4):
                nc.tensor.matmul(
                    v_ps0[:, j * 128:(j + 1) * 128],
                    xts0[kc][:, j * 128:(j + 1) * 128],
                    wv_sb[:, kc], start=not v_started_cell[0],
                    stop=(kc == KC - 1), skip_group_check=True)
                v_started_cell[0] = True
        for kc in range(KC):
            nc.tensor.matmul(q_ps0[:], wq_sb[:, kc], xts0[kc][:, 0:CQ],
                             start=(kc == 0), stop=(kc == KC - 1),
                             skip_group_check=True)
            nc.tensor.matmul(k_ps0[:], wk_sb[:, kc], xts0[kc][:, 0:CQ],
                             start=(kc == 0), stop=(kc == KC - 1),
                             skip_group_check=True)
            if kc >= 2:
                pro_v(kc - 2)
        # Act is idle until the first exp (~12us): give it the prologue
        # landing copies so DVE stays clear for the hf=1 unit copies
        nc.scalar.copy(qt_sb[:, 0:CQ], q_ps0[:])
        nc.scalar.copy(kt_sb[:, 0:CQ], k_ps0[:])
        for kc in range(KC - 2, KC):
            pro_v(kc)
        nc.scalar.copy(
            v_sb[:, 0:4, :, 0:64],
            v_ps0[:].rearrange("p (j h v) -> p j h v", j=4, h=2))

        # Chunk order: batch-1 runs [c1, c2, c0, c3] so the kernel ends on
        # the 16-block (1,3) chunk, whose late score blocks (kb>=12) keep
        # q/k/v(t7) units as deadline-reserved PE filler for the Act-paced
        # endgame, instead of draining tails against an empty proj queue.
        chunk_list = [(0, 0), (0, 1), (0, 2), (0, 3),
                      (1, 1), (1, 2), (1, 0), (1, 3)]
        nblks = [4 * (cc + 1) for _, cc in chunk_list]
        base = [0]
        for n in nblks:
            base.append(base[-1] + n)

        # global-slot deadlines: q(t) needed at its chunk's first slot;
        # k/v(t) first consumed when the score wave reaches keys t (slot
        # 4*(t%4) of the earliest chunk with c >= t%4 in list order)
        Q_DEAD = {1: base[1], 2: base[2], 3: base[3],
                  4: base[6], 5: base[4], 6: base[5],
                  7: base[5] + 8}  # q(t7) before the (1,3) pre-scoring
        KV_DEAD = {1: base[1] + 4, 2: base[2] + 8, 3: base[3] + 12,
                   4: base[4], 5: base[4] + 4, 6: base[5] + 8,
                   7: base[7] + 12}

        # v(t7) reserved two slots past k(t7): it pads the endgame's
        # diagonal-score bank rotation and is forced before pv(12) uses it
        V_DEAD = dict(KV_DEAD)
        V_DEAD[7] = base[7] + 14

        def queue_half_units(t, units6):
            queue_unit(Q_DEAD[t], units6[0:2])    # q a/b
            queue_unit(KV_DEAD[t], units6[2:4])   # k a/b
            queue_unit(V_DEAD[t], units6[4:6])    # v a/b

        def queue_pair_units(t0, units):
            queue_half_units(t0, units[:6])
            queue_half_units(t0 + 1, units[6:])

        queue_half_units(1, units0[6:])  # prologue pair: hf=1 only

        # pair creation: issue xT DMAs early — critically, ALL xt trains
        # must enter the in-order SP queue before any mid-kernel out-DMA
        # can park it (an out-DMA whose copy isn't ready blocks the queue
        # for many us, which starved later projections via late xts)
        pair_create = {1: [2], 2: [4, 6]}

        # pt tiles on demand so a later chunk's score wave can start while
        # an earlier chunk is still draining (pre-scoring)
        pts = {}

        def get_pt(b_, c_):
            if (b_, c_) not in pts:
                pts[(b_, c_)] = pt_pool.tile(
                    [128, NKB, 2, CQ], BF16, tag="pt", name=f"pt_{b_}_{c_}")
            return pts[(b_, c_)]

        def emit_scores_g(b_, c_, kb):
            pt = get_pt(b_, c_)
            tb_ = b_ * T
            tq0_ = c_ * CQ
            f0 = max(0, 128 * (kb - 4 * c_))
            sc = sc_pool.tile([128, 2, CQ], F32, tag="sc",
                              name=f"sc_{b_}_{c_}_{kb}")
            tk0 = kb * 128
            for h in range(2):
                hs = slice(h * 64, (h + 1) * 64)
                nc.tensor.matmul(
                    sc[:, h, f0:CQ],
                    kt_sb[hs, tb_ + tk0:tb_ + tk0 + 128],
                    qt_sb[hs, tb_ + tq0_ + f0:tb_ + tq0_ + CQ],
                    start=True, stop=True)
            nc.scalar.activation(
                pt[:, kb, :, f0:CQ], sc[:, :, f0:CQ],
                AF.Exp, scale=float(D) ** -0.5)
            if kb - 4 * c_ >= 0:  # diagonal block: causal triangle mask
                for h in range(2):
                    # keep where tq >= tk (f - p >= 0), else 0
                    nc.gpsimd.affine_select(
                        out=pt[:, kb, h, f0:f0 + 128],
                        in_=pt[:, kb, h, f0:f0 + 128],
                        compare_op=mybir.AluOpType.is_ge,
                        fill=0.0, base=0,
                        pattern=[[1, 128]], channel_multiplier=-1)
            return f0

        NPRE = 8   # (1,3) blocks pre-scored during (1,2)+(1,0)

        for ci, (b, c) in enumerate(chunk_list):
            for t0 in pair_create.get(ci, []):
                units, _ = proj_pair_units(t0)
                queue_pair_units(t0, units)

            tb = b * T
            tq0 = c * CQ
            nblk = 4 * (c + 1)
            pt = get_pt(b, c)
            # per-chunk O|Z accumulators [tq, gsub, h, d|Z]: pool rotation
            # (bufs=1) orders the next chunk's first PV write after this
            # chunk's tail reads
            o_ps = [acc_pool.tile([128, 2, 2, 65], F32, tag=f"o{i}",
                                  name=f"o_ps{i}_{b}_{c}")
                    for i in range(2)]
            zr_tiles = {}

            def make_tail_pieces(g, b=b, c=c, tb=tb, tq0=tq0, o_ps=o_ps):
                """Tail split into 3 pieces so the PE->DVE->PE->DVE chain of
                one tail interleaves with its pair partner + proj filler
                instead of stalling the in-order PE stream. In the final
                chunk the Act engine (done with exps by tail time) takes
                half the copies so DVE isn't the serial drain resource."""
                op = o_ps[g // 2]
                gs = g % 2
                act_assist = (b, c) == chunk_list[-1]
                state = {}

                def piece_a():   # normalize + transpose (PE 53ns)
                    if act_assist:
                        # 1/Z lands in SBUF so the Act engine can use it as
                        # an activation scale (scale APs must be SBUF)
                        if gs == 0:
                            zr = osb_pool.tile([128, 2, 2], F32, tag="zr",
                                               name=f"zr_{b}_{c}_{g}")
                            nc.vector.reciprocal(zr[:], op[:, :, :, 64])
                            zr_tiles[g // 2] = zr
                        zr = zr_tiles[g // 2]
                    elif gs == 0:
                        # 1/Z for the group pair, in place in PSUM col 64
                        # (both chains have stopped by emission time)
                        nc.vector.reciprocal(op[:, :, :, 64],
                                             op[:, :, :, 64])
                    o_sb = osb_pool.tile([128, 128], BF16, tag="osb",
                                         name=f"osb_{b}_{c}_{g}")
                    for h in range(2):
                        if act_assist and h == 1:
                            nc.scalar.activation(
                                o_sb[:, 64:128], op[:, gs, 1, 0:64],
                                AF.Copy, scale=zr[:, gs, 1:2])
                        elif act_assist:
                            nc.vector.tensor_scalar_mul(
                                o_sb[:, h * 64:(h + 1) * 64],
                                op[:, gs, h, 0:64],
                                zr[:, gs, h:h + 1])
                        else:
                            nc.vector.tensor_scalar_mul(
                                o_sb[:, h * 64:(h + 1) * 64],
                                op[:, gs, h, 0:64],
                                op[:, gs, h, 64:65])
                    tp = mm_pool.tile([128, 512], BF16, tag="mm",
                                      name=f"tp_{b}_{c}_{g}")
                    nc.tensor.transpose(tp[:, 0:128], o_sb[:], eye_sb[:])
                    state["tp"] = tp

                def piece_b():   # O^T landing copy (no PE)
                    ot_sb = otsb_pool.tile([128, 128], BF16, tag="otsb",
                                           name=f"otsb_{b}_{c}_{g}")
                    if act_assist:
                        nc.scalar.copy(ot_sb[:], state["tp"][:, 0:128])
                    else:
                        nc.vector.tensor_copy(ot_sb[:], state["tp"][:, 0:128])
                    state["ot"] = ot_sb

                def piece_c():   # output projection + copies + DMA (PE 426)
                    out_sb = outsb_pool.tile([128, E], BF16, tag="outsb",
                                             name=f"outsb_{b}_{c}_{g}")
                    tqg = tb + tq0 + g * 128
                    for eh in range(2):
                        ops = mm_pool.tile([128, 512], F32, tag="mm",
                                           name=f"ops_{b}_{c}_{g}_{eh}")
                        nc.tensor.matmul(
                            ops[:], state["ot"][:],
                            wo_sb[:, eh * 512:(eh + 1) * 512],
                            start=True, stop=True)
                        if ((b == 0 and c == 0) or act_assist) and eh == 1:
                            # Act's light window (short chunks / endgame)
                            nc.scalar.copy(
                                out_sb[:, eh * 512:(eh + 1) * 512], ops[:])
                        else:
                            nc.vector.tensor_copy(
                                out_sb[:, eh * 512:(eh + 1) * 512], ops[:])
                        if act_assist:
                            # endgame: SP is idle — issue half-row DMAs the
                            # moment each copy lands to shorten the drain
                            nc.sync.dma_start(
                                out[tqg:tqg + 128,
                                    eh * 512:(eh + 1) * 512],
                                out_sb[:, eh * 512:(eh + 1) * 512])
                    if not act_assist:
                        flush_out_dma()
                        dma_pending.append(
                            (out[tqg:tqg + 128, :], out_sb[:]))

                return [(53.0, piece_a), (0.0, piece_b), (426.0, piece_c)]

            # PSUM has_written bits: a start=True matmul clears them for the
            # WHOLE bank, so only the first PV matmul per o_ps bank per chunk
            # may use start=True. Later chains' first matmuls (kb==0,
            # start=False) overwrite-where-bit-clear, then accumulate.
            bank_started = [False, False]

            def pv_block(kb, b=b, c=c, pt=pt, o_ps=o_ps,
                         bank_started=bank_started):
                j0 = max(0, kb - 4 * c)
                # diagonal block: group j0's stationary is the masked pt
                # sub-block — emit it LAST so the gpsimd mask only gates the
                # final chain-stop matmul, not the whole block
                gs_order = list(range(j0, NQB))
                if kb - 4 * c >= 0 and len(gs_order) > 1:
                    gs_order = gs_order[1:] + gs_order[:1]
                for g in gs_order:
                    for h in range(2):
                        st = not bank_started[g // 2]
                        bank_started[g // 2] = True
                        nc.tensor.matmul(
                            o_ps[g // 2][:, g % 2, h, :],
                            pt[:, kb, h, g * 128:(g + 1) * 128],
                            v_sb[:, b * NKB + kb, h],
                            start=st, stop=(kb == 4 * c + g),
                            skip_group_check=True)
                j = kb - 4 * c
                if j in (1, 3):  # group pair's chains complete
                    pa = make_tail_pieces(j - 1)
                    pb = make_tail_pieces(j)
                    # interleave A A' B B' now; defer the C (out-proj)
                    # pieces until the NEXT pair so piece_b's DVE copy has
                    # landed long before C's Ldweights needs it
                    inter = [pa[0]]
                    if pending_cs:
                        inter.append(pending_cs.pop(0))
                    inter.append(pb[0])
                    if pending_cs:
                        inter.append(pending_cs.pop(0))
                    inter += [pa[1], pb[1]]
                    tails_q.extend(inter)
                    pending_cs.extend([pa[2], pb[2]])
                return (NQB - j0) * 2 * 65

            def emit_scores(kb):
                return emit_scores_g(b, c, kb)

            def pace(act_ns, pe_ns, slot):
                # deficit-paced filler: keep the PE fed during Act-paced
                # stretches, spend queued proj/tail work exactly where the
                # per-block PE emission falls short of the exp pace.
                sched["deficit"] += act_ns - pe_ns
                sched["deficit"] = max(-2000.0,
                                       min(sched["deficit"], 8000.0))
                # lookahead spread: don't let deadline-bound units burst
                if proj_q and proj_q[0][0] <= slot + 3:
                    pop_proj_piece()
                    if held:
                        pop_proj_piece()
                # alternate tail/proj pops so tail latency chains overlap
                # real PE work instead of stalling the in-order PE stream
                prefer_tail = True
                while sched["deficit"] > 400 and (tails_q or held or proj_q):
                    if prefer_tail and tails_q and not held:
                        pe_ns2, fn = tails_q.popleft()
                        fn()
                        sched["deficit"] -= pe_ns2
                    elif held or proj_q:
                        pop_proj_piece()
                    else:
                        pe_ns2, fn = tails_q.popleft()
                        fn()
                        sched["deficit"] -= pe_ns2
                    prefer_tail = not prefer_tail
                if held:   # never end a slot mid-unit
                    pop_proj_piece()

            def chunk_prefix(kb):
                if kb == 1:
                    # PE meat between sc(0)/exp(0) and the exp-gated
                    # pv(0), then the prev chunk's tail pieces
                    if held or proj_q:
                        pop_proj_piece()
                    drain_tails()

            if ci < len(chunk_list) - 1:
                for kb in range(nblk):
                    # units whose tokens this slot consumes: emit them now
                    force_proj_upto(base[ci] + kb)
                    f0 = emit_scores(kb)
                    pv_cyc = 0
                    if kb >= 1:
                        chunk_prefix(kb)
                        if kb >= 2:
                            pv_cyc = pv_block(kb - 2)
                    pace((2 * (CQ - f0) + 222) / 1.2,
                         (2 * (CQ - f0) + pv_cyc) * 0.4167, base[ci] + kb)
                    if ci == 5 and kb >= nblk - 4:
                        # pre-score an off-diagonal (1,3) block: shifts Act
                        # load out of the Act-saturated endgame
                        emit_scores_g(1, 3, kb - (nblk - 4))
                        pace((2 * CQ + 222) / 1.2, 2 * CQ * 0.4167,
                             base[ci] + kb)
                    if ci == 6 and NPRE == 8:
                        # four more during the Act-light (1,0) chunk
                        emit_scores_g(1, 3, 4 + kb)
                        pace((2 * CQ + 222) / 1.2, 2 * CQ * 0.4167,
                             base[ci] + kb)
                pv_block(nblk - 2)
                pv_block(nblk - 1)
            else:
                # Final chunk, two-phase so the kernel does not end on an
                # Act-bound exp wave:
                # phase A pre-scores the 12 off-diagonal blocks (Act paced,
                # PE kept busy by the deadline-reserved proj filler);
                # phase B runs the diagonal scores + every PV wave + tails
                # with all exps already in flight or done.
                for kb in range(NPRE, 12):
                    force_proj_upto(base[ci] + kb)
                    emit_scores(kb)
                    if kb == NPRE + 1:
                        # prev chunk's tails must fully emit before phase
                        # B's pv(0) rotates into its o_ps banks
                        if held or proj_q:
                            pop_proj_piece()
                        drain_tails()
                    pace((2 * CQ + 222) / 1.2, 2 * CQ * 0.4167,
                         base[ci] + kb)
                force_proj_upto(base[ci] + 12)  # k(t7) ahead of the scores
                emit_scores(12)
                emit_scores(13)
                for kb in range(0, 4):
                    pv_block(kb)
                if held or proj_q:   # v(t7) a: pads exp(12)'s bank WAR
                    pop_proj_piece()
                emit_scores(14)
                for kb in range(4, 8):
                    pv_block(kb)
                if held or proj_q:   # v(t7) b: pads exp(13)'s bank WAR
                    pop_proj_piece()
                emit_scores(15)
                force_proj_upto(base[ci] + 14)  # v(t7) before pv(12)
                for kb in range(8, 16):
                    pv_block(kb)

        drain_tails()
        while held or proj_q:
            pop_proj_piece()
        flush_out_dma()

    nc.compile()
    return nc


def _host_prep(x, Wq, Wk, Wv, Wo):
    bf = ml_dtypes.bfloat16
    xT = np.ascontiguousarray(
        np.asarray(x, dtype=np.float32).reshape(BT, E).T).astype(bf)

    def perm(w):
        # [E, 128] -> [128p, kc, 128d] flattened: w[kc*128+p, d] -> out[p, kc, d]
        return np.ascontiguousarray(
            w.reshape(KC, 128, 128).transpose(1, 0, 2).reshape(128, E)).astype(bf)

    Wq = np.asarray(Wq, dtype=np.float32)
    Wk = np.asarray(Wk, dtype=np.float32)
    Wv = np.asarray(Wv, dtype=np.float32)
    Wo = np.asarray(Wo, dtype=np.float32)

    in_maps = []
    for c in range(NCORE):
        sl = slice(c * 128, (c + 1) * 128)
        in_maps.append({
            "xT": xT,
            "wq": perm(Wq[:, sl]),
            "wk": perm(Wk[:, sl]),
            "wv": perm(Wv[:, sl]),
            "wo": np.ascontiguousarray(Wo[sl, :]).astype(bf),
        })
    return in_maps


def kernel(x, Wq, Wk, Wv, Wo, bo, _trace=False, _trace_kwargs=None):
    if "nc" not in _cache:
        _cache["nc"] = _build()
    nc = _cache["nc"]

    in_maps = _host_prep(x, Wq, Wk, Wv, Wo)
    kw = {}
    if _trace:
        kw = dict(trace=True, trace_cores=[0], **(_trace_kwargs or {}))
    res = run_bass_kernel_spmd(nc, in_maps, core_ids=list(range(NCORE)), **kw)
    _cache["last_result"] = res

    total = np.zeros((BT, E), dtype=np.float32)
    for r in res.results:
        total += np.asarray(r["out"], dtype=np.float32)
    total += np.asarray(bo, dtype=np.float32)[None, :]
    return total.reshape(B, T, E)



# revision 90
# speedup vs baseline: 1.0149x; 1.0014x over previous
"""Multi-head causal attention (B=2, T=2048, E=1024, H=16, D=64) on 8 trn2 cores.

Sharding: tensor-parallel over heads — core c owns heads {2c, 2c+1} (a 128-wide
slice of the hidden dim). Each core computes q/k/v projections for its heads
over the full sequence, causal attention, and a partial output projection
(contraction over its 128 rows of Wo). The host sums the 8 bf16 partials + bias.

v4 (128.8us, from the 150.5us v2), rebuilt around TimelineSim gap blame.
The engine floor is PE ~102us (proj 41 + scores 29 + PV 14.7 + out-proj
13.7 + transposes) with Act exp ~88us; everything else is scheduling:
 - In-place PSUM reciprocal of the Z column + bf16 o_sb/eye/tp (1 cyc/row
   transpose); PV groups emit mask-dependent-group last so the gpsimd
   affine_select triangle mask only gates the final chain-stop matmul.
 - Tails split into 3 pieces (normalize+transpose / O^T copy / out-proj+
   DMA), pair-interleaved, with the C pieces deferred one tail pair so
   every cross-engine hop has real PE work between emit and consume.
 - Proj units carry (deadline=global slot, pieces) and live in a sorted
   queue; deficit + lookahead pacing spends them where the exp pace
   outruns PE, preserving late-deadline units (q/k/v of t7) as endgame
   filler. The prologue hand-interleaves q/k/v per-kc at xT arrival rate
   with weight DMAs slotted between xts, q/k psum borrowed from the idle
   sc banks, and landing copies on the (idle) Act engine.
 - Chunk order (0,*), (1,1), (1,2), (1,0), (1,3): 8 of (1,3)'s off-diag
   score blocks are pre-scored during (1,2)/(1,0) (pt bufs=3) and its
   own wave is two-phase — off-diag scores first (Act-paced, filler-fed),
   then diag scores + all PV waves + tails with every exp in flight, so
   the kernel does not end on an Act-bound stretch. Endgame tails use
   Act-assisted copies (Act is exp-free by then) + immediate out DMAs;
   mid-kernel Act borrows copies only in (0,0)/(0,1)-adjacent windows
   where it is measurably idle. Tail pools run 6 bufs deep.
 - PV emission lags scores by 2 blocks; all xt DMA trains enter the
   in-order SP queue before any out-DMA can park it.

PSUM (8 banks): sc 2x[128,2,512] (4) + O|Z accumulators 2x[128,2,2,65] (2) +
mm [128,512] x2 (2, shared by proj / out-proj / transpose tiles).

Timing signal is concourse TimelineSim (no NTFF under this axon client).
"""

import numpy as np
import ml_dtypes
from collections import deque

import concourse.bass as bass
import concourse.tile as tile
from concourse import bacc, mybir
from concourse.bass_utils import run_bass_kernel_spmd
from concourse.masks import make_identity
from contextlib import ExitStack

B, T, E, H, D = 2, 2048, 1024, 16, 64
BT = B * T            # 4096 tokens total
NCORE = 8
KC = E // 128         # contraction chunks for projections = 8
CQ = 512              # tq chunk width
NQB = T // CQ         # tq chunks per batch = 4
NKB = T // 128        # tk blocks per batch = 16

F32 = mybir.dt.float32
BF16 = mybir.dt.bfloat16
AF = mybir.ActivationFunctionType

_cache = {}


def _build():
    nc = bacc.Bacc("TRN2", target_bir_lowering=False, debug=False,
                   num_devices=NCORE)

    xT = nc.dram_tensor("xT", [E, BT], BF16, kind="ExternalInput").ap()
    wq = nc.dram_tensor("wq", [128, E], BF16, kind="ExternalInput").ap()
    wk = nc.dram_tensor("wk", [128, E], BF16, kind="ExternalInput").ap()
    wv = nc.dram_tensor("wv", [128, E], BF16, kind="ExternalInput").ap()
    wo = nc.dram_tensor("wo", [128, E], BF16, kind="ExternalInput").ap()
    out = nc.dram_tensor("out", [BT, E], BF16, kind="ExternalOutput").ap()

    with tile.TileContext(nc) as tc, ExitStack() as ctx:
        pers = ctx.enter_context(tc.tile_pool(name="pers", bufs=1))

        wq_sb = pers.tile([128, KC, 128], BF16, tag="wq")
        wk_sb = pers.tile([128, KC, 128], BF16, tag="wk")
        wv_sb = pers.tile([128, KC, 128], BF16, tag="wv")
        wo_sb = pers.tile([128, E], BF16, tag="wo")
        eye_sb = pers.tile([128, 128], BF16, tag="eye")
        qt_sb = pers.tile([128, BT], BF16, tag="qt")    # [dims(2 heads), tok]
        kt_sb = pers.tile([128, BT], BF16, tag="kt")
        # V natural + ones col per head: [tok%128, blk, h, d|1]; the ones
        # column makes the flipped P^T-stationary PV matmul emit Z = sum(exp)
        # as output column 64 for free.
        v_sb = pers.tile([128, BT // 128, 2, 65], BF16, tag="v")

        # wq queued first on the sync HWDGE queue so the first projection
        # matmul gates on as little DMA as possible; each extra DMA costs
        # ~625ns of serial HWDGE hold, so weights go as single transfers
        # slotted between the xts that need them.
        wq_r = wq.rearrange("p (kc d) -> p kc d", kc=KC)
        nc.sync.dma_start(wq_sb[:, 0:KC // 2], wq_r[:, 0:KC // 2])
        nc.vector.memset(v_sb[:, :, :, 64:65], 1.0)
        make_identity(nc, eye_sb[:])

        # SBUF pools
        xts_pool = ctx.enter_context(tc.tile_pool(name="xts", bufs=32))
        pt_pool = ctx.enter_context(tc.tile_pool(name="pt", bufs=2))
        osb_pool = ctx.enter_context(tc.tile_pool(name="osb", bufs=6))
        otsb_pool = ctx.enter_context(tc.tile_pool(name="otsb", bufs=6))
        outsb_pool = ctx.enter_context(tc.tile_pool(name="outsb", bufs=6))

        # PSUM pools: 4 + 2 + 2 = 8 banks
        sc_pool = ctx.enter_context(tc.tile_pool(name="sc", bufs=2,
                                                 space="PSUM"))
        acc_pool = ctx.enter_context(tc.tile_pool(name="acc", bufs=1,
                                                  space="PSUM"))
        mm_pool = ctx.enter_context(tc.tile_pool(name="mm", bufs=2,
                                                 space="PSUM"))


        # ---- projection units -------------------------------------------
        def proj_pair_units(t0, dma_hooks=None):
            """t0: even 512-token chunk index (0..6). Issues the pair's xT
            DMAs now; returns 6 unit callbacks (q,k,v) x (hf 0,1).
            dma_hooks: {kc: callback} run right after that kc's xt DMA is
            queued (prologue interleaves weight DMAs at specific points)."""
            xts = []
            for kc in range(KC):
                xt = xts_pool.tile([128, 2 * CQ], BF16, tag="xt",
                                   name=f"xt_{t0}_{kc}")
                nc.sync.dma_start(
                    xt[:], xT[kc * 128:(kc + 1) * 128,
                              t0 * CQ:(t0 + 2) * CQ])
                if dma_hooks and kc in dma_hooks:
                    dma_hooks[kc]()
                xts.append(xt)

            def qk_unit(w_sb, dst_sb, hf):
                # two ~850ns halves so filler interleaves finely with waves
                t_ = t0 + hf
                state = {}
                def emit_a():
                    ps = mm_pool.tile([128, CQ], F32, tag="mm",
                                      name=f"qkps{t_}_{id(w_sb)}")
                    state["ps"] = ps
                    for kc in range(KC // 2):
                        nc.tensor.matmul(
                            ps[:], w_sb[:, kc],
                            xts[kc][:, hf * CQ:(hf + 1) * CQ],
                            start=(kc == 0), stop=False)
                def emit_b():
                    ps = state["ps"]
                    for kc in range(KC // 2, KC):
                        nc.tensor.matmul(
                            ps[:], w_sb[:, kc],
                            xts[kc][:, hf * CQ:(hf + 1) * CQ],
                            start=False, stop=(kc == KC - 1))
                    nc.vector.tensor_copy(
                        dst_sb[:, t_ * CQ:(t_ + 1) * CQ], ps[:])
                return [emit_a, emit_b]

            def v_unit(hf):
                t_ = t0 + hf
                state = {}
                def emit_a():
                    v_ps = mm_pool.tile([128, CQ], F32, tag="mm",
                                        name=f"vps{t_}")
                    state["ps"] = v_ps
                    for j in (0, 1):
                        jf = hf * CQ + j * 128
                        for kc in range(KC):
                            nc.tensor.matmul(
                                v_ps[:, j * 128:(j + 1) * 128],
                                xts[kc][:, jf:jf + 128],
                                wv_sb[:, kc], start=(kc == 0),
                                stop=(kc == KC - 1))
                def emit_b():
                    v_ps = state["ps"]
                    for j in (2, 3):
                        jf = hf * CQ + j * 128
                        for kc in range(KC):
                            nc.tensor.matmul(
                                v_ps[:, j * 128:(j + 1) * 128],
                                xts[kc][:, jf:jf + 128],
                                wv_sb[:, kc], start=(kc == 0),
                                stop=(kc == KC - 1))
                    b4 = t_ * (CQ // 128)
                    nc.vector.tensor_copy(
                        v_sb[:, b4:b4 + 4, :, 0:64],
                        v_ps[:].rearrange("p (j h v) -> p j h v",
                                          j=4, h=2))
                return [emit_a, emit_b]

            units = (qk_unit(wq_sb, qt_sb, 0) + qk_unit(wk_sb, kt_sb, 0) +
                     v_unit(0) + qk_unit(wq_sb, qt_sb, 1) +
                     qk_unit(wk_sb, kt_sb, 1) + v_unit(1))
            return units, xts

        # ---- filler machinery -------------------------------------------
        # proj_q entries are (deadline_slot, seq, [piece_a, piece_b]): the
        # unit MUST be emitted before the global attention slot that
        # consumes its tokens (a later emission would deadlock the in-order
        # PE queue). Kept sorted by deadline so deficit-paced pops
        # naturally preserve the latest-deadline units as an endgame
        # reserve.
        tails_q = deque()
        pending_cs = []    # tail C (out-proj) pieces deferred one pair
        proj_q = []
        held = []          # pending b-half of a split proj unit (must pop
                           # before any other mm-pool user)
        sched = {"deficit": 0.0, "seq": 0}
        dma_pending = []   # (dram_slice, sbuf_tile): out DMAs deferred one
                           # tail so the SP queue never stalls on copy sems

        PROJ_NS = 853.0    # PE ns per proj half-piece (4 matmuls x 512)

        def flush_out_dma():
            while dma_pending:
                dst, src = dma_pending.pop(0)
                nc.sync.dma_start(dst, src)

        def queue_unit(dead, pieces):
            proj_q.append((dead, sched["seq"], pieces))
            sched["seq"] += 1
            proj_q.sort(key=lambda e: (e[0], e[1]))

        def pop_proj_piece():
            if held:
                held.pop()()
            else:
                _, _, pieces = proj_q.pop(0)
                pieces[0]()
                held.append(pieces[1])
            sched["deficit"] -= PROJ_NS

        def drain_tails():
            # interleave a proj piece between tail pieces so their
            # cross-engine latency chains overlap real PE work
            tails_q.extend(pending_cs)
            pending_cs.clear()
            while tails_q:
                pe_ns, fn = tails_q.popleft()
                fn()
                sched["deficit"] -= pe_ns
                if tails_q and (held or proj_q) and sched["deficit"] > -800:
                    pop_proj_piece()

        def force_proj_upto(slot):
            while held or (proj_q and proj_q[0][0] <= slot):
                pop_proj_piece()

        # ---- prologue ----------------------------------------------------
        # Weight DMAs slot between the pair-0 xT DMAs (wk after xt0, wv
        # after xt2 — each lands just before its first consumer) and
        # q/k/v matmuls interleave per-kc at xT arrival granularity so the
        # PE streams at DMA rate with no burst stalls.
        hooks = {
            0: lambda: (nc.sync.dma_start(wq_sb[:, KC // 2:KC],
                                          wq_r[:, KC // 2:KC]),
                        nc.sync.dma_start(
                wk_sb[:], wk.rearrange("p (kc d) -> p kc d", kc=KC))),
            2: lambda: nc.sync.dma_start(
                wv_sb[:], wv.rearrange("p (kc d) -> p kc d", kc=KC)),
            7: lambda: nc.sync.dma_start(wo_sb[:], wo[:]),
        }
        units0, xts0 = proj_pair_units(0, dma_hooks=hooks)

        # tokens 0..511: q/k psum tiles borrow the (still idle) sc tag's
        # banks so mm_pool stays free for the interleaved v chains.
        q_ps0 = sc_pool.tile([128, CQ], F32, tag="sc", name="qps_pro")
        k_ps0 = sc_pool.tile([128, CQ], F32, tag="sc", name="kps_pro")
        v_ps0 = mm_pool.tile([128, CQ], F32, tag="mm", name="vps_pro")
        v_started_cell = [False]

        def pro_v(kc):
            # interleaved per-j chains on one bank: only the very first
            # matmul clears the bank's has_written bits (start=True); the
            # other chains' kc==0 matmuls overwrite-where-bit-clear
            for j in range(4):
                nc.tensor.matmul(
                    v_ps0[:, j * 128:(j + 1) * 128],
                    xts0[kc][:, j * 128:(j + 1) * 128],
                    wv_sb[:, kc], start=not v_started_cell[0],
                    stop=(kc == KC - 1), skip_group_check=True)
                v_started_cell[0] = True
        for kc in range(KC):
            nc.tensor.matmul(q_ps0[:], wq_sb[:, kc], xts0[kc][:, 0:CQ],
                             start=(kc == 0), stop=(kc == KC - 1),
                             skip_group_check=True)
            nc.tensor.matmul(k_ps0[:], wk_sb[:, kc], xts0[kc][:, 0:CQ],
                             start=(kc == 0), stop=(kc == KC - 1),
                             skip_group_check=True)
            if kc >= 2:
                pro_v(kc - 2)
        # Act is idle until the first exp (~12us): give it the prologue
        # landing copies so DVE stays clear for the hf=1 unit copies
        nc.scalar.copy(qt_sb[:, 0:CQ], q_ps0[:])
        nc.scalar.copy(kt_sb[:, 0:CQ], k_ps0[:])
        for kc in range(KC - 2, KC):
            pro_v(kc)
        nc.scalar.copy(
            v_sb[:, 0:4, :, 0:64],
            v_ps0[:].rearrange("p (j h v) -> p j h v", j=4, h=2))

        # Chunk order: batch-1 runs [c1, c2, c0, c3] so the kernel ends on
        # the 16-block (1,3) chunk, whose late score blocks (kb>=12) keep
        # q/k/v(t7) units as deadline-reserved PE filler for the Act-paced
        # endgame, instead of draining tails against an empty proj queue.
        chunk_list = [(0, 0), (0, 1), (0, 2), (0, 3),
                      (1, 1), (1, 2), (1, 0), (1, 3)]
        nblks = [4 * (cc + 1) for _, cc in chunk_list]
        base = [0]
        for n in nblks:
            base.append(base[-1] + n)

        # global-slot deadlines: q(t) needed at its chunk's first slot;
        # k/v(t) first consumed when the score wave reaches keys t (slot
        # 4*(t%4) of the earliest chunk with c >= t%4 in list order)
        Q_DEAD = {1: base[1], 2: base[2], 3: base[3],
                  4: base[6], 5: base[4], 6: base[5],
                  7: base[5] + 4}  # q(t7) before the (1,3) pre-scoring
        KV_DEAD = {1: base[1] + 4, 2: base[2] + 8, 3: base[3] + 12,
                   4: base[4], 5: base[4] + 4, 6: base[5] + 8,
                   7: base[7] + 12}

        # v(t7) reserved two slots past k(t7): it pads the endgame's
        # diagonal-score bank rotation and is forced before pv(12) uses it
        V_DEAD = dict(KV_DEAD)
        V_DEAD[7] = base[7] + 14

        def queue_half_units(t, units6):
            queue_unit(Q_DEAD[t], units6[0:2])    # q a/b
            queue_unit(KV_DEAD[t], units6[2:4])   # k a/b
            queue_unit(V_DEAD[t], units6[4:6])    # v a/b

        def queue_pair_units(t0, units):
            queue_half_units(t0, units[:6])
            queue_half_units(t0 + 1, units[6:])

        queue_half_units(1, units0[6:])  # prologue pair: hf=1 only

        # pair creation: issue xT DMAs early — critically, ALL xt trains
        # must enter the in-order SP queue before any mid-kernel out-DMA
        # can park it (an out-DMA whose copy isn't ready blocks the queue
        # for many us, which starved later projections via late xts)
        pair_create = {1: [2], 2: [4, 6]}

        # pt tiles on demand so a later chunk's score wave can start while
        # an earlier chunk is still draining (pre-scoring)
        pts = {}

        def get_pt(b_, c_):
            if (b_, c_) not in pts:
                pts[(b_, c_)] = pt_pool.tile(
                    [128, NKB, 2, CQ], BF16, tag="pt", name=f"pt_{b_}_{c_}")
            return pts[(b_, c_)]

        def emit_scores_g(b_, c_, kb):
            pt = get_pt(b_, c_)
            tb_ = b_ * T
            tq0_ = c_ * CQ
            f0 = max(0, 128 * (kb - 4 * c_))
            sc = sc_pool.tile([128, 2, CQ], F32, tag="sc",
                              name=f"sc_{b_}_{c_}_{kb}")
            tk0 = kb * 128
            for h in range(2):
                hs = slice(h * 64, (h + 1) * 64)
                nc.tensor.matmul(
                    sc[:, h, f0:CQ],
                    kt_sb[hs, tb_ + tk0:tb_ + tk0 + 128],
                    qt_sb[hs, tb_ + tq0_ + f0:tb_ + tq0_ + CQ],
                    start=True, stop=True)
            nc.scalar.activation(
                pt[:, kb, :, f0:CQ], sc[:, :, f0:CQ],
                AF.Exp, scale=float(D) ** -0.5)
            if kb - 4 * c_ >= 0:  # diagonal block: causal triangle mask
                for h in range(2):
                    # keep where tq >= tk (f - p >= 0), else 0
                    nc.gpsimd.affine_select(
                        out=pt[:, kb, h, f0:f0 + 128],
                        in_=pt[:, kb, h, f0:f0 + 128],
                        compare_op=mybir.AluOpType.is_ge,
                        fill=0.0, base=0,
                        pattern=[[1, 128]], channel_multiplier=-1)
            return f0

        NPRE = 8   # (1,3) blocks pre-scored during (1,2)+(1,0)

        for ci, (b, c) in enumerate(chunk_list):
            for t0 in pair_create.get(ci, []):
                units, _ = proj_pair_units(t0)
                queue_pair_units(t0, units)

            tb = b * T
            tq0 = c * CQ
            nblk = 4 * (c + 1)
            pt = get_pt(b, c)
            # per-chunk O|Z accumulators [tq, gsub, h, d|Z]: pool rotation
            # (bufs=1) orders the next chunk's first PV write after this
            # chunk's tail reads
            o_ps = [acc_pool.tile([128, 2, 2, 65], F32, tag=f"o{i}",
                                  name=f"o_ps{i}_{b}_{c}")
                    for i in range(2)]
            zr_tiles = {}

            def make_tail_pieces(g, b=b, c=c, tb=tb, tq0=tq0, o_ps=o_ps):
                """Tail split into 3 pieces so the PE->DVE->PE->DVE chain of
                one tail interleaves with its pair partner + proj filler
                instead of stalling the in-order PE stream. In the final
                chunk the Act engine (done with exps by tail time) takes
                half the copies so DVE isn't the serial drain resource."""
                op = o_ps[g // 2]
                gs = g % 2
                act_assist = (b, c) == chunk_list[-1]
                state = {}

                def piece_a():   # normalize + transpose (PE 53ns)
                    if act_assist:
                        # 1/Z lands in SBUF so the Act engine can use it as
                        # an activation scale (scale APs must be SBUF)
                        if gs == 0:
                            zr = osb_pool.tile([128, 2, 2], F32, tag="zr",
                                               name=f"zr_{b}_{c}_{g}")
                            nc.vector.reciprocal(zr[:], op[:, :, :, 64])
                            zr_tiles[g // 2] = zr
                        zr = zr_tiles[g // 2]
                    elif gs == 0:
                        # 1/Z for the group pair, in place in PSUM col 64
                        # (both chains have stopped by emission time)
                        nc.vector.reciprocal(op[:, :, :, 64],
                                             op[:, :, :, 64])
                    o_sb = osb_pool.tile([128, 128], BF16, tag="osb",
                                         name=f"osb_{b}_{c}_{g}")
                    for h in range(2):
                        if act_assist and h == 1:
                            nc.scalar.activation(
                                o_sb[:, 64:128], op[:, gs, 1, 0:64],
                                AF.Copy, scale=zr[:, gs, 1:2])
                        elif act_assist:
                            nc.vector.tensor_scalar_mul(
                                o_sb[:, h * 64:(h + 1) * 64],
                                op[:, gs, h, 0:64],
                                zr[:, gs, h:h + 1])
                        else:
                            nc.vector.tensor_scalar_mul(
                                o_sb[:, h * 64:(h + 1) * 64],
                                op[:, gs, h, 0:64],
                                op[:, gs, h, 64:65])
                    tp = mm_pool.tile([128, 512], BF16, tag="mm",
                                      name=f"tp_{b}_{c}_{g}")
                    nc.tensor.transpose(tp[:, 0:128], o_sb[:], eye_sb[:])
                    state["tp"] = tp

                def piece_b():   # O^T landing copy (no PE)
                    ot_sb = otsb_pool.tile([128, 128], BF16, tag="otsb",
                                           name=f"otsb_{b}_{c}_{g}")
                    if act_assist:
                        nc.scalar.copy(ot_sb[:], state["tp"][:, 0:128])
                    else:
                        nc.vector.tensor_copy(ot_sb[:], state["tp"][:, 0:128])
                    state["ot"] = ot_sb

                def piece_c():   # output projection + copies + DMA (PE 426)
                    out_sb = outsb_pool.tile([128, E], BF16, tag="outsb",
                                             name=f"outsb_{b}_{c}_{g}")
                    tqg = tb + tq0 + g * 128
                    for eh in range(2):
                        ops = mm_pool.tile([128, 512], F32, tag="mm",
                                           name=f"ops_{b}_{c}_{g}_{eh}")
                        nc.tensor.matmul(
                            ops[:], state["ot"][:],
                            wo_sb[:, eh * 512:(eh + 1) * 512],
                            start=True, stop=True)
                        if ((b == 0 and c == 0) or act_assist) and eh == 1:
                            # Act's light window (short chunks / endgame)
                            nc.scalar.copy(
                                out_sb[:, eh * 512:(eh + 1) * 512], ops[:])
                        else:
                            nc.vector.tensor_copy(
                                out_sb[:, eh * 512:(eh + 1) * 512], ops[:])
                        if act_assist:
                            # endgame: SP is idle — issue half-row DMAs the
                            # moment each copy lands to shorten the drain
                            nc.sync.dma_start(
                                out[tqg:tqg + 128,
                                    eh * 512:(eh + 1) * 512],
                                out_sb[:, eh * 512:(eh + 1) * 512])
                    if not act_assist:
                        flush_out_dma()
                        dma_pending.append(
                            (out[tqg:tqg + 128, :], out_sb[:]))

                return [(53.0, piece_a), (0.0, piece_b), (426.0, piece_c)]

            # PSUM has_written bits: a start=True matmul clears them for the
            # WHOLE bank, so only the first PV matmul per o_ps bank per chunk
            # may use start=True. Later chains' first matmuls (kb==0,
            # start=False) overwrite-where-bit-clear, then accumulate.
            bank_started = [False, False]

            def pv_block(kb, b=b, c=c, pt=pt, o_ps=o_ps,
                         bank_started=bank_started):
                j0 = max(0, kb - 4 * c)
                # diagonal block: group j0's stationary is the masked pt
                # sub-block — emit it LAST so the gpsimd mask only gates the
                # final chain-stop matmul, not the whole block
                gs_order = list(range(j0, NQB))
                if kb - 4 * c >= 0 and len(gs_order) > 1:
                    gs_order = gs_order[1:] + gs_order[:1]
                for g in gs_order:
                    for h in range(2):
                        st = not bank_started[g // 2]
                        bank_started[g // 2] = True
                        nc.tensor.matmul(
                            o_ps[g // 2][:, g % 2, h, :],
                            pt[:, kb, h, g * 128:(g + 1) * 128],
                            v_sb[:, b * NKB + kb, h],
                            start=st, stop=(kb == 4 * c + g),
                            skip_group_check=True)
                j = kb - 4 * c
                if j in (1, 3):  # group pair's chains complete
                    pa = make_tail_pieces(j - 1)
                    pb = make_tail_pieces(j)
                    # interleave A A' B B' now; defer the C (out-proj)
                    # pieces until the NEXT pair so piece_b's DVE copy has
                    # landed long before C's Ldweights needs it
                    inter = [pa[0]]
                    if pending_cs:
                        inter.append(pending_cs.pop(0))
                    inter.append(pb[0])
                    if pending_cs:
                        inter.append(pending_cs.pop(0))
                    inter += [pa[1], pb[1]]
                    tails_q.extend(inter)
                    pending_cs.extend([pa[2], pb[2]])
                return (NQB - j0) * 2 * 65

            def emit_scores(kb):
                return emit_scores_g(b, c, kb)

            def pace(act_ns, pe_ns, slot):
                # deficit-paced filler: keep the PE fed during Act-paced
                # stretches, spend queued proj/tail work exactly where the
                # per-block PE emission falls short of the exp pace.
                sched["deficit"] += act_ns - pe_ns
                sched["deficit"] = max(-2000.0,
                                       min(sched["deficit"], 8000.0))
                # lookahead spread: don't let deadline-bound units burst
                if proj_q and proj_q[0][0] <= slot + 3:
                    pop_proj_piece()
                    if held:
                        pop_proj_piece()
                # alternate tail/proj pops so tail latency chains overlap
                # real PE work instead of stalling the in-order PE stream
                prefer_tail = True
                while sched["deficit"] > 400 and (tails_q or held or proj_q):
                    if prefer_tail and tails_q and not held:
                        pe_ns2, fn = tails_q.popleft()
                        fn()
                        sched["deficit"] -= pe_ns2
                    elif held or proj_q:
                        pop_proj_piece()
                    else:
                        pe_ns2, fn = tails_q.popleft()
                        fn()
                        sched["deficit"] -= pe_ns2
                    prefer_tail = not prefer_tail
                if held:   # never end a slot mid-unit
                    pop_proj_piece()

            def chunk_prefix(kb):
                if kb == 1:
                    # PE meat between sc(0)/exp(0) and the exp-gated
                    # pv(0), then the prev chunk's tail pieces
                    if held or proj_q:
                        pop_proj_piece()
                    drain_tails()

            if ci < len(chunk_list) - 1:
                for kb in range(nblk):
                    # units whose tokens this slot consumes: emit them now
                    force_proj_upto(base[ci] + kb)
                    f0 = emit_scores(kb)
                    pv_cyc = 0
                    if kb >= 1:
                        chunk_prefix(kb)
                        if kb >= 2:
                            pv_cyc = pv_block(kb - 2)
                    pace((2 * (CQ - f0) + 222) / 1.2,
                         (2 * (CQ - f0) + pv_cyc) * 0.4167, base[ci] + kb)
                    if ci == 5 and kb >= nblk - 4:
                        # pre-score an off-diagonal (1,3) block: shifts Act
                        # load out of the Act-saturated endgame
                        emit_scores_g(1, 3, kb - (nblk - 4))
                        pace((2 * CQ + 222) / 1.2, 2 * CQ * 0.4167,
                             base[ci] + kb)
                    if ci == 6 and NPRE == 8:
                        # four more during the Act-light (1,0) chunk
                        emit_scores_g(1, 3, 4 + kb)
                        pace((2 * CQ + 222) / 1.2, 2 * CQ * 0.4167,
                             base[ci] + kb)
                pv_block(nblk - 2)
                pv_block(nblk - 1)
            else:
                # Final chunk, two-phase so the kernel does not end on an
                # Act-bound exp wave:
                # phase A pre-scores the 12 off-diagonal blocks (Act paced,
                # PE kept busy by the deadline-reserved proj filler);
                # phase B runs the diagonal scores + every PV wave + tails
                # with all exps already in flight or done.
                for kb in range(NPRE, 12):
                    force_proj_upto(base[ci] + kb)
                    emit_scores(kb)
                    if kb == NPRE + 1:
                        # prev chunk's tails must fully emit before phase
                        # B's pv(0) rotates into its o_ps banks
                        if held or proj_q:
                            pop_proj_piece()
                        drain_tails()
                    pace((2 * CQ + 222) / 1.2, 2 * CQ * 0.4167,
                         base[ci] + kb)
                force_proj_upto(base[ci] + 12)  # k(t7) ahead of the scores
                emit_scores(12)
                emit_scores(13)
                for kb in range(0, 4):
                    pv_block(kb)
                if held or proj_q:   # v(t7) a: pads exp(12)'s bank WAR
                    pop_proj_piece()
                emit_scores(14)
                for kb in range(4, 8):
                    pv_block(kb)
                if held or proj_q:   # v(t7) b: pads exp(13)'s bank WAR
                    pop_proj_piece()
                emit_scores(15)
                force_proj_upto(base[ci] + 14)  # v(t7) before pv(12)
                for kb in range(8, 16):
                    pv_block(kb)

        drain_tails()
        while held or proj_q:
            pop_proj_piece()
        flush_out_dma()

    nc.compile()
    return nc


def _host_prep(x, Wq, Wk, Wv, Wo):
    bf = ml_dtypes.bfloat16
    xT = np.ascontiguousarray(
        np.asarray(x, dtype=np.float32).reshape(BT, E).T).astype(bf)

    def perm(w):
        # [E, 128] -> [128p, kc, 128d] flattened: w[kc*128+p, d] -> out[p, kc, d]
        return np.ascontiguousarray(
            w.reshape(KC, 128, 128).transpose(1, 0, 2).reshape(128, E)).astype(bf)

    Wq = np.asarray(Wq, dtype=np.float32)
    Wk = np.asarray(Wk, dtype=np.float32)
    Wv = np.asarray(Wv, dtype=np.float32)
    Wo = np.asarray(Wo, dtype=np.float32)

    in_maps = []
    for c in range(NCORE):
        sl = slice(c * 128, (c + 1) * 128)
        in_maps.append({
            "xT": xT,
            "wq": perm(Wq[:, sl]),
            "wk": perm(Wk[:, sl]),
            "wv": perm(Wv[:, sl]),
            "wo": np.ascontiguousarray(Wo[sl, :]).astype(bf),
        })
    return in_maps


def kernel(x, Wq, Wk, Wv, Wo, bo, _trace=False, _trace_kwargs=None):
    if "nc" not in _cache:
        _cache["nc"] = _build()
    nc = _cache["nc"]

    in_maps = _host_prep(x, Wq, Wk, Wv, Wo)
    kw = {}
    if _trace:
        kw = dict(trace=True, trace_cores=[0], **(_trace_kwargs or {}))
    res = run_bass_kernel_spmd(nc, in_maps, core_ids=list(range(NCORE)), **kw)
    _cache["last_result"] = res

    total = np.zeros((BT, E), dtype=np.float32)
    for r in res.results:
        total += np.asarray(r["out"], dtype=np.float32)
    total += np.asarray(bo, dtype=np.float32)[None, :]
    return total.reshape(B, T, E)



# revision 92
# speedup vs baseline: 1.0158x; 1.0009x over previous
"""Multi-head causal attention (B=2, T=2048, E=1024, H=16, D=64) on 8 trn2 cores.

Sharding: tensor-parallel over heads — core c owns heads {2c, 2c+1} (a 128-wide
slice of the hidden dim). Each core computes q/k/v projections for its heads
over the full sequence, causal attention, and a partial output projection
(contraction over its 128 rows of Wo). The host sums the 8 bf16 partials + bias.

v4 (128.8us, from the 150.5us v2), rebuilt around TimelineSim gap blame.
The engine floor is PE ~102us (proj 41 + scores 29 + PV 14.7 + out-proj
13.7 + transposes) with Act exp ~88us; everything else is scheduling:
 - In-place PSUM reciprocal of the Z column + bf16 o_sb/eye/tp (1 cyc/row
   transpose); PV groups emit mask-dependent-group last so the gpsimd
   affine_select triangle mask only gates the final chain-stop matmul.
 - Tails split into 3 pieces (normalize+transpose / O^T copy / out-proj+
   DMA), pair-interleaved, with the C pieces deferred one tail pair so
   every cross-engine hop has real PE work between emit and consume.
 - Proj units carry (deadline=global slot, pieces) and live in a sorted
   queue; deficit + lookahead pacing spends them where the exp pace
   outruns PE, preserving late-deadline units (q/k/v of t7) as endgame
   filler. The prologue hand-interleaves q/k/v per-kc at xT arrival rate
   with weight DMAs slotted between xts, q/k psum borrowed from the idle
   sc banks, and landing copies on the (idle) Act engine.
 - Chunk order (0,*), (1,1), (1,2), (1,0), (1,3): 8 of (1,3)'s off-diag
   score blocks are pre-scored during (1,2)/(1,0) (pt bufs=3) and its
   own wave is two-phase — off-diag scores first (Act-paced, filler-fed),
   then diag scores + all PV waves + tails with every exp in flight, so
   the kernel does not end on an Act-bound stretch. Endgame tails use
   Act-assisted copies (Act is exp-free by then) + immediate out DMAs;
   mid-kernel Act borrows copies only in (0,0)/(0,1)-adjacent windows
   where it is measurably idle. Tail pools run 6 bufs deep.
 - PV emission lags scores by 2 blocks; all xt DMA trains enter the
   in-order SP queue before any out-DMA can park it.

PSUM (8 banks): sc 2x[128,2,512] (4) + O|Z accumulators 2x[128,2,2,65] (2) +
mm [128,512] x2 (2, shared by proj / out-proj / transpose tiles).

Timing signal is concourse TimelineSim (no NTFF under this axon client).
"""

import numpy as np
import ml_dtypes
from collections import deque

import concourse.bass as bass
import concourse.tile as tile
from concourse import bacc, mybir
from concourse.bass_utils import run_bass_kernel_spmd
from concourse.masks import make_identity
from contextlib import ExitStack

B, T, E, H, D = 2, 2048, 1024, 16, 64
BT = B * T            # 4096 tokens total
NCORE = 8
KC = E // 128         # contraction chunks for projections = 8
CQ = 512              # tq chunk width
NQB = T // CQ         # tq chunks per batch = 4
NKB = T // 128        # tk blocks per batch = 16

F32 = mybir.dt.float32
BF16 = mybir.dt.bfloat16
AF = mybir.ActivationFunctionType

_cache = {}


def _build():
    nc = bacc.Bacc("TRN2", target_bir_lowering=False, debug=False,
                   num_devices=NCORE)

    xT = nc.dram_tensor("xT", [E, BT], BF16, kind="ExternalInput").ap()
    wq = nc.dram_tensor("wq", [128, E], BF16, kind="ExternalInput").ap()
    wk = nc.dram_tensor("wk", [128, E], BF16, kind="ExternalInput").ap()
    wv = nc.dram_tensor("wv", [128, E], BF16, kind="ExternalInput").ap()
    wo = nc.dram_tensor("wo", [128, E], BF16, kind="ExternalInput").ap()
    out = nc.dram_tensor("out", [BT, E], BF16, kind="ExternalOutput").ap()

    with tile.TileContext(nc) as tc, ExitStack() as ctx:
        pers = ctx.enter_context(tc.tile_pool(name="pers", bufs=1))

        wq_sb = pers.tile([128, KC, 128], BF16, tag="wq")
        wk_sb = pers.tile([128, KC, 128], BF16, tag="wk")
        wv_sb = pers.tile([128, KC, 128], BF16, tag="wv")
        wo_sb = pers.tile([128, E], BF16, tag="wo")
        eye_sb = pers.tile([128, 128], BF16, tag="eye")
        qt_sb = pers.tile([128, BT], BF16, tag="qt")    # [dims(2 heads), tok]
        kt_sb = pers.tile([128, BT], BF16, tag="kt")
        # V natural + ones col per head: [tok%128, blk, h, d|1]; the ones
        # column makes the flipped P^T-stationary PV matmul emit Z = sum(exp)
        # as output column 64 for free.
        v_sb = pers.tile([128, BT // 128, 2, 65], BF16, tag="v")

        # wq queued first on the sync HWDGE queue so the first projection
        # matmul gates on as little DMA as possible; each extra DMA costs
        # ~625ns of serial HWDGE hold, so weights go as single transfers
        # slotted between the xts that need them.
        wq_r = wq.rearrange("p (kc d) -> p kc d", kc=KC)
        nc.sync.dma_start(wq_sb[:, 0:KC // 2], wq_r[:, 0:KC // 2])
        nc.vector.memset(v_sb[:, :, :, 64:65], 1.0)
        make_identity(nc, eye_sb[:])

        # SBUF pools
        xts_pool = ctx.enter_context(tc.tile_pool(name="xts", bufs=32))
        pt_pool = ctx.enter_context(tc.tile_pool(name="pt", bufs=2))
        osb_pool = ctx.enter_context(tc.tile_pool(name="osb", bufs=6))
        otsb_pool = ctx.enter_context(tc.tile_pool(name="otsb", bufs=6))
        outsb_pool = ctx.enter_context(tc.tile_pool(name="outsb", bufs=6))

        # PSUM pools: 4 + 2 + 2 = 8 banks
        sc_pool = ctx.enter_context(tc.tile_pool(name="sc", bufs=2,
                                                 space="PSUM"))
        acc_pool = ctx.enter_context(tc.tile_pool(name="acc", bufs=1,
                                                  space="PSUM"))
        mm_pool = ctx.enter_context(tc.tile_pool(name="mm", bufs=2,
                                                 space="PSUM"))


        # ---- projection units -------------------------------------------
        def proj_pair_units(t0, dma_hooks=None):
            """t0: even 512-token chunk index (0..6). Issues the pair's xT
            DMAs now; returns 6 unit callbacks (q,k,v) x (hf 0,1).
            dma_hooks: {kc: callback} run right after that kc's xt DMA is
            queued (prologue interleaves weight DMAs at specific points)."""
            xts = []
            for kc in range(KC):
                xt = xts_pool.tile([128, 2 * CQ], BF16, tag="xt",
                                   name=f"xt_{t0}_{kc}")
                nc.sync.dma_start(
                    xt[:], xT[kc * 128:(kc + 1) * 128,
                              t0 * CQ:(t0 + 2) * CQ])
                if dma_hooks and kc in dma_hooks:
                    dma_hooks[kc]()
                xts.append(xt)

            def qk_unit(w_sb, dst_sb, hf):
                # two ~850ns halves so filler interleaves finely with waves
                t_ = t0 + hf
                state = {}
                def emit_a():
                    ps = mm_pool.tile([128, CQ], F32, tag="mm",
                                      name=f"qkps{t_}_{id(w_sb)}")
                    state["ps"] = ps
                    for kc in range(KC // 2):
                        nc.tensor.matmul(
                            ps[:], w_sb[:, kc],
                            xts[kc][:, hf * CQ:(hf + 1) * CQ],
                            start=(kc == 0), stop=False)
                def emit_b():
                    ps = state["ps"]
                    for kc in range(KC // 2, KC):
                        nc.tensor.matmul(
                            ps[:], w_sb[:, kc],
                            xts[kc][:, hf * CQ:(hf + 1) * CQ],
                            start=False, stop=(kc == KC - 1))
                    nc.vector.tensor_copy(
                        dst_sb[:, t_ * CQ:(t_ + 1) * CQ], ps[:])
                return [emit_a, emit_b]

            def v_unit(hf):
                t_ = t0 + hf
                state = {}
                def emit_a():
                    v_ps = mm_pool.tile([128, CQ], F32, tag="mm",
                                        name=f"vps{t_}")
                    state["ps"] = v_ps
                    for j in (0, 1):
                        jf = hf * CQ + j * 128
                        for kc in range(KC):
                            nc.tensor.matmul(
                                v_ps[:, j * 128:(j + 1) * 128],
                                xts[kc][:, jf:jf + 128],
                                wv_sb[:, kc], start=(kc == 0),
                                stop=(kc == KC - 1))
                def emit_b():
                    v_ps = state["ps"]
                    for j in (2, 3):
                        jf = hf * CQ + j * 128
                        for kc in range(KC):
                            nc.tensor.matmul(
                                v_ps[:, j * 128:(j + 1) * 128],
                                xts[kc][:, jf:jf + 128],
                                wv_sb[:, kc], start=(kc == 0),
                                stop=(kc == KC - 1))
                    b4 = t_ * (CQ // 128)
                    nc.vector.tensor_copy(
                        v_sb[:, b4:b4 + 4, :, 0:64],
                        v_ps[:].rearrange("p (j h v) -> p j h v",
                                          j=4, h=2))
                return [emit_a, emit_b]

            units = (qk_unit(wq_sb, qt_sb, 0) + qk_unit(wk_sb, kt_sb, 0) +
                     v_unit(0) + qk_unit(wq_sb, qt_sb, 1) +
                     qk_unit(wk_sb, kt_sb, 1) + v_unit(1))
            return units, xts

        # ---- filler machinery -------------------------------------------
        # proj_q entries are (deadline_slot, seq, [piece_a, piece_b]): the
        # unit MUST be emitted before the global attention slot that
        # consumes its tokens (a later emission would deadlock the in-order
        # PE queue). Kept sorted by deadline so deficit-paced pops
        # naturally preserve the latest-deadline units as an endgame
        # reserve.
        tails_q = deque()
        pending_cs = []    # tail C (out-proj) pieces deferred one pair
        proj_q = []
        held = []          # pending b-half of a split proj unit (must pop
                           # before any other mm-pool user)
        sched = {"deficit": 0.0, "seq": 0}
        dma_pending = []   # (dram_slice, sbuf_tile): out DMAs deferred one
                           # tail so the SP queue never stalls on copy sems

        PROJ_NS = 853.0    # PE ns per proj half-piece (4 matmuls x 512)

        def flush_out_dma():
            while dma_pending:
                dst, src = dma_pending.pop(0)
                nc.sync.dma_start(dst, src)

        def queue_unit(dead, pieces):
            proj_q.append((dead, sched["seq"], pieces))
            sched["seq"] += 1
            proj_q.sort(key=lambda e: (e[0], e[1]))

        def pop_proj_piece():
            if held:
                held.pop()()
            else:
                _, _, pieces = proj_q.pop(0)
                pieces[0]()
                held.append(pieces[1])
            sched["deficit"] -= PROJ_NS

        def drain_tails():
            # interleave a proj piece between tail pieces so their
            # cross-engine latency chains overlap real PE work
            tails_q.extend(pending_cs)
            pending_cs.clear()
            while tails_q:
                pe_ns, fn = tails_q.popleft()
                fn()
                sched["deficit"] -= pe_ns
                if tails_q and (held or proj_q) and sched["deficit"] > -800:
                    pop_proj_piece()

        def force_proj_upto(slot):
            while held or (proj_q and proj_q[0][0] <= slot):
                pop_proj_piece()

        # ---- prologue ----------------------------------------------------
        # Weight DMAs slot between the pair-0 xT DMAs (wk after xt0, wv
        # after xt2 — each lands just before its first consumer) and
        # q/k/v matmuls interleave per-kc at xT arrival granularity so the
        # PE streams at DMA rate with no burst stalls.
        hooks = {
            0: lambda: (nc.sync.dma_start(wq_sb[:, KC // 2:KC],
                                          wq_r[:, KC // 2:KC]),
                        nc.sync.dma_start(
                wk_sb[:], wk.rearrange("p (kc d) -> p kc d", kc=KC))),
            2: lambda: nc.sync.dma_start(
                wv_sb[:], wv.rearrange("p (kc d) -> p kc d", kc=KC)),
            7: lambda: nc.sync.dma_start(wo_sb[:], wo[:]),
        }
        units0, xts0 = proj_pair_units(0, dma_hooks=hooks)

        # tokens 0..511: q/k psum tiles borrow the (still idle) sc tag's
        # banks so mm_pool stays free for the interleaved v chains.
        q_ps0 = sc_pool.tile([128, CQ], F32, tag="sc", name="qps_pro")
        k_ps0 = sc_pool.tile([128, CQ], F32, tag="sc", name="kps_pro")
        v_ps0 = mm_pool.tile([128, CQ], F32, tag="mm", name="vps_pro")
        v_started_cell = [False]

        def pro_v(kc):
            # interleaved per-j chains on one bank: only the very first
            # matmul clears the bank's has_written bits (start=True); the
            # other chains' kc==0 matmuls overwrite-where-bit-clear
            for j in range(4):
                nc.tensor.matmul(
                    v_ps0[:, j * 128:(j + 1) * 128],
                    xts0[kc][:, j * 128:(j + 1) * 128],
                    wv_sb[:, kc], start=not v_started_cell[0],
                    stop=(kc == KC - 1), skip_group_check=True)
                v_started_cell[0] = True
        for kc in range(KC):
            nc.tensor.matmul(q_ps0[:], wq_sb[:, kc], xts0[kc][:, 0:CQ],
                             start=(kc == 0), stop=(kc == KC - 1),
                             skip_group_check=True)
            nc.tensor.matmul(k_ps0[:], wk_sb[:, kc], xts0[kc][:, 0:CQ],
                             start=(kc == 0), stop=(kc == KC - 1),
                             skip_group_check=True)
            if kc >= 2:
                pro_v(kc - 2)
        # Act is idle until the first exp (~12us): give it the prologue
        # landing copies so DVE stays clear for the hf=1 unit copies
        nc.scalar.copy(qt_sb[:, 0:CQ], q_ps0[:])
        nc.scalar.copy(kt_sb[:, 0:CQ], k_ps0[:])
        for kc in range(KC - 2, KC):
            pro_v(kc)
        nc.scalar.copy(
            v_sb[:, 0:4, :, 0:64],
            v_ps0[:].rearrange("p (j h v) -> p j h v", j=4, h=2))

        # Chunk order: batch-1 runs [c1, c2, c0, c3] so the kernel ends on
        # the 16-block (1,3) chunk, whose late score blocks (kb>=12) keep
        # q/k/v(t7) units as deadline-reserved PE filler for the Act-paced
        # endgame, instead of draining tails against an empty proj queue.
        chunk_list = [(0, 0), (0, 1), (0, 2), (0, 3),
                      (1, 1), (1, 2), (1, 0), (1, 3)]
        nblks = [4 * (cc + 1) for _, cc in chunk_list]
        base = [0]
        for n in nblks:
            base.append(base[-1] + n)

        # global-slot deadlines: q(t) needed at its chunk's first slot;
        # k/v(t) first consumed when the score wave reaches keys t (slot
        # 4*(t%4) of the earliest chunk with c >= t%4 in list order)
        Q_DEAD = {1: base[1], 2: base[2], 3: base[3],
                  4: base[6], 5: base[4], 6: base[5],
                  7: base[5] + 4}  # q(t7) before the (1,3) pre-scoring
        KV_DEAD = {1: base[1] + 4, 2: base[2] + 8, 3: base[3] + 12,
                   4: base[4], 5: base[4] + 4, 6: base[5] + 8,
                   7: base[7] + 12}

        # v(t7) reserved two slots past k(t7): it pads the endgame's
        # diagonal-score bank rotation and is forced before pv(12) uses it
        V_DEAD = dict(KV_DEAD)
        V_DEAD[7] = base[7] + 14

        def queue_half_units(t, units6):
            queue_unit(Q_DEAD[t], units6[0:2])    # q a/b
            queue_unit(KV_DEAD[t], units6[2:4])   # k a/b
            queue_unit(V_DEAD[t], units6[4:6])    # v a/b

        def queue_pair_units(t0, units):
            queue_half_units(t0, units[:6])
            queue_half_units(t0 + 1, units[6:])

        queue_half_units(1, units0[6:])  # prologue pair: hf=1 only

        # pair creation: issue xT DMAs early — critically, ALL xt trains
        # must enter the in-order SP queue before any mid-kernel out-DMA
        # can park it (an out-DMA whose copy isn't ready blocks the queue
        # for many us, which starved later projections via late xts)
        pair_create = {1: [2], 2: [4, 6]}

        # pt tiles on demand so a later chunk's score wave can start while
        # an earlier chunk is still draining (pre-scoring)
        pts = {}

        def get_pt(b_, c_):
            if (b_, c_) not in pts:
                pts[(b_, c_)] = pt_pool.tile(
                    [128, NKB, 2, CQ], BF16, tag="pt", name=f"pt_{b_}_{c_}")
            return pts[(b_, c_)]

        def emit_scores_g(b_, c_, kb):
            pt = get_pt(b_, c_)
            tb_ = b_ * T
            tq0_ = c_ * CQ
            f0 = max(0, 128 * (kb - 4 * c_))
            sc = sc_pool.tile([128, 2, CQ], F32, tag="sc",
                              name=f"sc_{b_}_{c_}_{kb}")
            tk0 = kb * 128
            for h in range(2):
                hs = slice(h * 64, (h + 1) * 64)
                nc.tensor.matmul(
                    sc[:, h, f0:CQ],
                    kt_sb[hs, tb_ + tk0:tb_ + tk0 + 128],
                    qt_sb[hs, tb_ + tq0_ + f0:tb_ + tq0_ + CQ],
                    start=True, stop=True)
            nc.scalar.activation(
                pt[:, kb, :, f0:CQ], sc[:, :, f0:CQ],
                AF.Exp, scale=float(D) ** -0.5)
            if kb - 4 * c_ >= 0:  # diagonal block: causal triangle mask
                for h in range(2):
                    # keep where tq >= tk (f - p >= 0), else 0
                    nc.gpsimd.affine_select(
                        out=pt[:, kb, h, f0:f0 + 128],
                        in_=pt[:, kb, h, f0:f0 + 128],
                        compare_op=mybir.AluOpType.is_ge,
                        fill=0.0, base=0,
                        pattern=[[1, 128]], channel_multiplier=-1)
            return f0

        NPRE = 8   # (1,3) blocks pre-scored during (1,2)+(1,0)

        for ci, (b, c) in enumerate(chunk_list):
            for t0 in pair_create.get(ci, []):
                units, _ = proj_pair_units(t0)
                queue_pair_units(t0, units)

            tb = b * T
            tq0 = c * CQ
            nblk = 4 * (c + 1)
            pt = get_pt(b, c)
            # per-chunk O|Z accumulators [tq, gsub, h, d|Z]: pool rotation
            # (bufs=1) orders the next chunk's first PV write after this
            # chunk's tail reads
            o_ps = [acc_pool.tile([128, 2, 2, 65], F32, tag=f"o{i}",
                                  name=f"o_ps{i}_{b}_{c}")
                    for i in range(2)]
            zr_tiles = {}

            def make_tail_pieces(g, b=b, c=c, tb=tb, tq0=tq0, o_ps=o_ps):
                """Tail split into 3 pieces so the PE->DVE->PE->DVE chain of
                one tail interleaves with its pair partner + proj filler
                instead of stalling the in-order PE stream. In the final
                chunk the Act engine (done with exps by tail time) takes
                half the copies so DVE isn't the serial drain resource."""
                op = o_ps[g // 2]
                gs = g % 2
                act_assist = (b, c) == chunk_list[-1]
                state = {}

                def piece_a():   # normalize + transpose (PE 53ns)
                    if act_assist:
                        # 1/Z lands in SBUF so the Act engine can use it as
                        # an activation scale (scale APs must be SBUF)
                        if gs == 0:
                            zr = osb_pool.tile([128, 2, 2], F32, tag="zr",
                                               name=f"zr_{b}_{c}_{g}")
                            nc.vector.reciprocal(zr[:], op[:, :, :, 64])
                            zr_tiles[g // 2] = zr
                        zr = zr_tiles[g // 2]
                    elif gs == 0:
                        # 1/Z for the group pair, in place in PSUM col 64
                        # (both chains have stopped by emission time)
                        nc.vector.reciprocal(op[:, :, :, 64],
                                             op[:, :, :, 64])
                    o_sb = osb_pool.tile([128, 128], BF16, tag="osb",
                                         name=f"osb_{b}_{c}_{g}")
                    for h in range(2):
                        if act_assist and h == 1:
                            nc.scalar.activation(
                                o_sb[:, 64:128], op[:, gs, 1, 0:64],
                                AF.Copy, scale=zr[:, gs, 1:2])
                        elif act_assist:
                            nc.vector.tensor_scalar_mul(
                                o_sb[:, h * 64:(h + 1) * 64],
                                op[:, gs, h, 0:64],
                                zr[:, gs, h:h + 1])
                        else:
                            nc.vector.tensor_scalar_mul(
                                o_sb[:, h * 64:(h + 1) * 64],
                                op[:, gs, h, 0:64],
                                op[:, gs, h, 64:65])
                    tp = mm_pool.tile([128, 512], BF16, tag="mm",
                                      name=f"tp_{b}_{c}_{g}")
                    nc.tensor.transpose(tp[:, 0:128], o_sb[:], eye_sb[:])
                    state["tp"] = tp

                def piece_b():   # O^T landing copy (no PE)
                    ot_sb = otsb_pool.tile([128, 128], BF16, tag="otsb",
                                           name=f"otsb_{b}_{c}_{g}")
                    if act_assist:
                        nc.scalar.copy(ot_sb[:], state["tp"][:, 0:128])
                    else:
                        nc.vector.tensor_copy(ot_sb[:], state["tp"][:, 0:128])
                    state["ot"] = ot_sb

                def piece_c():   # output projection + copies + DMA (PE 426)
                    out_sb = outsb_pool.tile([128, E], BF16, tag="outsb",
                                             name=f"outsb_{b}_{c}_{g}")
                    tqg = tb + tq0 + g * 128
                    for eh in range(2):
                        ops = mm_pool.tile([128, 512], F32, tag="mm",
                                           name=f"ops_{b}_{c}_{g}_{eh}")
                        nc.tensor.matmul(
                            ops[:], state["ot"][:],
                            wo_sb[:, eh * 512:(eh + 1) * 512],
                            start=True, stop=True)
                        if ((b == 0 and c == 0) or act_assist) and eh == 1:
                            # Act's light window (short chunks / endgame)
                            nc.scalar.copy(
                                out_sb[:, eh * 512:(eh + 1) * 512], ops[:])
                        else:
                            nc.vector.tensor_copy(
                                out_sb[:, eh * 512:(eh + 1) * 512], ops[:])
                        if act_assist:
                            # endgame: SP is idle — issue half-row DMAs the
                            # moment each copy lands to shorten the drain
                            nc.sync.dma_start(
                                out[tqg:tqg + 128,
                                    eh * 512:(eh + 1) * 512],
                                out_sb[:, eh * 512:(eh + 1) * 512])
                    if not act_assist:
                        flush_out_dma()
                        dma_pending.append(
                            (out[tqg:tqg + 128, :], out_sb[:]))

                return [(53.0, piece_a), (0.0, piece_b), (426.0, piece_c)]

            # PSUM has_written bits: a start=True matmul clears them for the
            # WHOLE bank, so only the first PV matmul per o_ps bank per chunk
            # may use start=True. Later chains' first matmuls (kb==0,
            # start=False) overwrite-where-bit-clear, then accumulate.
            bank_started = [False, False]

            def pv_block(kb, b=b, c=c, pt=pt, o_ps=o_ps,
                         bank_started=bank_started):
                j0 = max(0, kb - 4 * c)
                # diagonal block: group j0's stationary is the masked pt
                # sub-block — emit it LAST so the gpsimd mask only gates the
                # final chain-stop matmul, not the whole block
                gs_order = list(range(j0, NQB))
                if kb - 4 * c >= 0 and len(gs_order) > 1:
                    gs_order = gs_order[1:] + gs_order[:1]
                for g in gs_order:
                    for h in range(2):
                        st = not bank_started[g // 2]
                        bank_started[g // 2] = True
                        nc.tensor.matmul(
                            o_ps[g // 2][:, g % 2, h, :],
                            pt[:, kb, h, g * 128:(g + 1) * 128],
                            v_sb[:, b * NKB + kb, h],
                            start=st, stop=(kb == 4 * c + g),
                            skip_group_check=True)
                j = kb - 4 * c
                if j in (1, 3):  # group pair's chains complete
                    pa = make_tail_pieces(j - 1)
                    pb = make_tail_pieces(j)
                    # interleave A A' B B' now; defer the C (out-proj)
                    # pieces until the NEXT pair so piece_b's DVE copy has
                    # landed long before C's Ldweights needs it
                    inter = [pa[0]]
                    if pending_cs:
                        inter.append(pending_cs.pop(0))
                    inter.append(pb[0])
                    if pending_cs:
                        inter.append(pending_cs.pop(0))
                    inter += [pa[1], pb[1]]
                    tails_q.extend(inter)
                    pending_cs.extend([pa[2], pb[2]])
                return (NQB - j0) * 2 * 65

            def emit_scores(kb):
                return emit_scores_g(b, c, kb)

            def pace(act_ns, pe_ns, slot):
                # deficit-paced filler: keep the PE fed during Act-paced
                # stretches, spend queued proj/tail work exactly where the
                # per-block PE emission falls short of the exp pace.
                sched["deficit"] += act_ns - pe_ns
                sched["deficit"] = max(-2000.0,
                                       min(sched["deficit"], 8000.0))
                # lookahead spread: don't let deadline-bound units burst
                if proj_q and proj_q[0][0] <= slot + 3:
                    pop_proj_piece()
                    if held:
                        pop_proj_piece()
                # alternate tail/proj pops so tail latency chains overlap
                # real PE work instead of stalling the in-order PE stream
                prefer_tail = True
                while sched["deficit"] > 400 and (tails_q or held or proj_q):
                    if prefer_tail and tails_q and not held:
                        pe_ns2, fn = tails_q.popleft()
                        fn()
                        sched["deficit"] -= pe_ns2
                    elif held or proj_q:
                        pop_proj_piece()
                    else:
                        pe_ns2, fn = tails_q.popleft()
                        fn()
                        sched["deficit"] -= pe_ns2
                    prefer_tail = not prefer_tail
                if held:   # never end a slot mid-unit
                    pop_proj_piece()

            def chunk_prefix(kb):
                if kb == 1:
                    # PE meat between sc(0)/exp(0) and the exp-gated
                    # pv(0), then the prev chunk's tail pieces
                    if held or proj_q:
                        pop_proj_piece()
                    drain_tails()

            if ci < len(chunk_list) - 1:
                for kb in range(nblk):
                    # units whose tokens this slot consumes: emit them now
                    force_proj_upto(base[ci] + kb)
                    f0 = emit_scores(kb)
                    pv_cyc = 0
                    if kb >= 1:
                        chunk_prefix(kb)
                        if kb >= 2:
                            pv_cyc = pv_block(kb - 2)
                    if ci == 5 and kb >= nblk - 4:
                        # pre-score an off-diagonal (1,3) block right after
                        # the host scores so Act sees it ASAP: shifts Act
                        # load out of the Act-saturated endgame
                        emit_scores_g(1, 3, kb - (nblk - 4))
                    pace((2 * (CQ - f0) + 222) / 1.2,
                         (2 * (CQ - f0) + pv_cyc) * 0.4167, base[ci] + kb)
                    if ci == 5 and kb >= nblk - 4:
                        pace((2 * CQ + 222) / 1.2, 2 * CQ * 0.4167,
                             base[ci] + kb)
                    if ci == 6 and NPRE == 8:
                        # four more during the Act-light (1,0) chunk
                        emit_scores_g(1, 3, 4 + kb)
                        pace((2 * CQ + 222) / 1.2, 2 * CQ * 0.4167,
                             base[ci] + kb)
                pv_block(nblk - 2)
                pv_block(nblk - 1)
            else:
                # Final chunk, two-phase so the kernel does not end on an
                # Act-bound exp wave:
                # phase A pre-scores the 12 off-diagonal blocks (Act paced,
                # PE kept busy by the deadline-reserved proj filler);
                # phase B runs the diagonal scores + every PV wave + tails
                # with all exps already in flight or done.
                for kb in range(NPRE, 12):
                    force_proj_upto(base[ci] + kb)
                    emit_scores(kb)
                    if kb == NPRE + 1:
                        # prev chunk's tails must fully emit before phase
                        # B's pv(0) rotates into its o_ps banks
                        if held or proj_q:
                            pop_proj_piece()
                        drain_tails()
                    pace((2 * CQ + 222) / 1.2, 2 * CQ * 0.4167,
                         base[ci] + kb)
                force_proj_upto(base[ci] + 12)  # k(t7) ahead of the scores
                emit_scores(12)
                emit_scores(13)
                for kb in range(0, 4):
                    pv_block(kb)
                if held or proj_q:   # v(t7) a: pads exp(12)'s bank WAR
                    pop_proj_piece()
                emit_scores(14)
                for kb in range(4, 8):
                    pv_block(kb)
                if held or proj_q:   # v(t7) b: pads exp(13)'s bank WAR
                    pop_proj_piece()
                emit_scores(15)
                force_proj_upto(base[ci] + 14)  # v(t7) before pv(12)
                for kb in range(8, 16):
                    pv_block(kb)

        drain_tails()
        while held or proj_q:
            pop_proj_piece()
        flush_out_dma()

    nc.compile()
    return nc


def _host_prep(x, Wq, Wk, Wv, Wo):
    bf = ml_dtypes.bfloat16
    xT = np.ascontiguousarray(
        np.asarray(x, dtype=np.float32).reshape(BT, E).T).astype(bf)

    def perm(w):
        # [E, 128] -> [128p, kc, 128d] flattened: w[kc*128+p, d] -> out[p, kc, d]
        return np.ascontiguousarray(
            w.reshape(KC, 128, 128).transpose(1, 0, 2).reshape(128, E)).astype(bf)

    Wq = np.asarray(Wq, dtype=np.float32)
    Wk = np.asarray(Wk, dtype=np.float32)
    Wv = np.asarray(Wv, dtype=np.float32)
    Wo = np.asarray(Wo, dtype=np.float32)

    in_maps = []
    for c in range(NCORE):
        sl = slice(c * 128, (c + 1) * 128)
        in_maps.append({
            "xT": xT,
            "wq": perm(Wq[:, sl]),
            "wk": perm(Wk[:, sl]),
            "wv": perm(Wv[:, sl]),
            "wo": np.ascontiguousarray(Wo[sl, :]).astype(bf),
        })
    return in_maps


def kernel(x, Wq, Wk, Wv, Wo, bo, _trace=False, _trace_kwargs=None):
    if "nc" not in _cache:
        _cache["nc"] = _build()
    nc = _cache["nc"]

    in_maps = _host_prep(x, Wq, Wk, Wv, Wo)
    kw = {}
    if _trace:
        kw = dict(trace=True, trace_cores=[0], **(_trace_kwargs or {}))
    res = run_bass_kernel_spmd(nc, in_maps, core_ids=list(range(NCORE)), **kw)
    _cache["last_result"] = res

    total = np.zeros((BT, E), dtype=np.float32)
    for r in res.results:
        total += np.asarray(r["out"], dtype=np.float32)
    total += np.asarray(bo, dtype=np.float32)[None, :]
    return total.reshape(B, T, E)



# revision 93
# speedup vs baseline: 1.0182x; 1.0024x over previous
"""Multi-head causal attention (B=2, T=2048, E=1024, H=16, D=64) on 8 trn2 cores.

Sharding: tensor-parallel over heads — core c owns heads {2c, 2c+1} (a 128-wide
slice of the hidden dim). Each core computes q/k/v projections for its heads
over the full sequence, causal attention, and a partial output projection
(contraction over its 128 rows of Wo). The host sums the 8 bf16 partials + bias.

v4 (128.8us, from the 150.5us v2), rebuilt around TimelineSim gap blame.
The engine floor is PE ~102us (proj 41 + scores 29 + PV 14.7 + out-proj
13.7 + transposes) with Act exp ~88us; everything else is scheduling:
 - In-place PSUM reciprocal of the Z column + bf16 o_sb/eye/tp (1 cyc/row
   transpose); PV groups emit mask-dependent-group last so the gpsimd
   affine_select triangle mask only gates the final chain-stop matmul.
 - Tails split into 3 pieces (normalize+transpose / O^T copy / out-proj+
   DMA), pair-interleaved, with the C pieces deferred one tail pair so
   every cross-engine hop has real PE work between emit and consume.
 - Proj units carry (deadline=global slot, pieces) and live in a sorted
   queue; deficit + lookahead pacing spends them where the exp pace
   outruns PE, preserving late-deadline units (q/k/v of t7) as endgame
   filler. The prologue hand-interleaves q/k/v per-kc at xT arrival rate
   with weight DMAs slotted between xts, q/k psum borrowed from the idle
   sc banks, and landing copies on the (idle) Act engine.
 - Chunk order (0,*), (1,1), (1,2), (1,0), (1,3): 8 of (1,3)'s off-diag
   score blocks are pre-scored during (1,2)/(1,0) (pt bufs=3) and its
   own wave is two-phase — off-diag scores first (Act-paced, filler-fed),
   then diag scores + all PV waves + tails with every exp in flight, so
   the kernel does not end on an Act-bound stretch. Endgame tails use
   Act-assisted copies (Act is exp-free by then) + immediate out DMAs;
   mid-kernel Act borrows copies only in (0,0)/(0,1)-adjacent windows
   where it is measurably idle. Tail pools run 6 bufs deep.
 - PV emission lags scores by 2 blocks; all xt DMA trains enter the
   in-order SP queue before any out-DMA can park it.

PSUM (8 banks): sc 2x[128,2,512] (4) + O|Z accumulators 2x[128,2,2,65] (2) +
mm [128,512] x2 (2, shared by proj / out-proj / transpose tiles).

Timing signal is concourse TimelineSim (no NTFF under this axon client).
"""

import numpy as np
import ml_dtypes
from collections import deque

import concourse.bass as bass
import concourse.tile as tile
from concourse import bacc, mybir
from concourse.bass_utils import run_bass_kernel_spmd
from concourse.masks import make_identity
from contextlib import ExitStack

B, T, E, H, D = 2, 2048, 1024, 16, 64
BT = B * T            # 4096 tokens total
NCORE = 8
KC = E // 128         # contraction chunks for projections = 8
CQ = 512              # tq chunk width
NQB = T // CQ         # tq chunks per batch = 4
NKB = T // 128        # tk blocks per batch = 16

F32 = mybir.dt.float32
BF16 = mybir.dt.bfloat16
AF = mybir.ActivationFunctionType

_cache = {}


def _build():
    nc = bacc.Bacc("TRN2", target_bir_lowering=False, debug=False,
                   num_devices=NCORE)

    xT = nc.dram_tensor("xT", [E, BT], BF16, kind="ExternalInput").ap()
    wq = nc.dram_tensor("wq", [128, E], BF16, kind="ExternalInput").ap()
    wk = nc.dram_tensor("wk", [128, E], BF16, kind="ExternalInput").ap()
    wv = nc.dram_tensor("wv", [128, E], BF16, kind="ExternalInput").ap()
    wo = nc.dram_tensor("wo", [128, E], BF16, kind="ExternalInput").ap()
    out = nc.dram_tensor("out", [BT, E], BF16, kind="ExternalOutput").ap()

    with tile.TileContext(nc) as tc, ExitStack() as ctx:
        pers = ctx.enter_context(tc.tile_pool(name="pers", bufs=1))

        wq_sb = pers.tile([128, KC, 128], BF16, tag="wq")
        wk_sb = pers.tile([128, KC, 128], BF16, tag="wk")
        wv_sb = pers.tile([128, KC, 128], BF16, tag="wv")
        wo_sb = pers.tile([128, E], BF16, tag="wo")
        eye_sb = pers.tile([128, 128], BF16, tag="eye")
        qt_sb = pers.tile([128, BT], BF16, tag="qt")    # [dims(2 heads), tok]
        kt_sb = pers.tile([128, BT], BF16, tag="kt")
        # V natural + ones col per head: [tok%128, blk, h, d|1]; the ones
        # column makes the flipped P^T-stationary PV matmul emit Z = sum(exp)
        # as output column 64 for free.
        v_sb = pers.tile([128, BT // 128, 2, 65], BF16, tag="v")

        # wq queued first on the sync HWDGE queue so the first projection
        # matmul gates on as little DMA as possible; each extra DMA costs
        # ~625ns of serial HWDGE hold, so weights go as single transfers
        # slotted between the xts that need them.
        wq_r = wq.rearrange("p (kc d) -> p kc d", kc=KC)
        nc.sync.dma_start(wq_sb[:, 0:KC // 2], wq_r[:, 0:KC // 2])
        nc.vector.memset(v_sb[:, :, :, 64:65], 1.0)
        make_identity(nc, eye_sb[:])

        # SBUF pools
        xts_pool = ctx.enter_context(tc.tile_pool(name="xts", bufs=32))
        pt_pool = ctx.enter_context(tc.tile_pool(name="pt", bufs=2))
        osb_pool = ctx.enter_context(tc.tile_pool(name="osb", bufs=6))
        otsb_pool = ctx.enter_context(tc.tile_pool(name="otsb", bufs=6))
        outsb_pool = ctx.enter_context(tc.tile_pool(name="outsb", bufs=6))

        # PSUM pools: 4 + 2 + 2 = 8 banks
        sc_pool = ctx.enter_context(tc.tile_pool(name="sc", bufs=2,
                                                 space="PSUM"))
        acc_pool = ctx.enter_context(tc.tile_pool(name="acc", bufs=1,
                                                  space="PSUM"))
        mm_pool = ctx.enter_context(tc.tile_pool(name="mm", bufs=2,
                                                 space="PSUM"))


        # ---- projection units -------------------------------------------
        def proj_pair_units(t0, dma_hooks=None):
            """t0: even 512-token chunk index (0..6). Issues the pair's xT
            DMAs now; returns 6 unit callbacks (q,k,v) x (hf 0,1).
            dma_hooks: {kc: callback} run right after that kc's xt DMA is
            queued (prologue interleaves weight DMAs at specific points)."""
            xts = []
            for kc in range(KC):
                xt = xts_pool.tile([128, 2 * CQ], BF16, tag="xt",
                                   name=f"xt_{t0}_{kc}")
                nc.sync.dma_start(
                    xt[:], xT[kc * 128:(kc + 1) * 128,
                              t0 * CQ:(t0 + 2) * CQ])
                if dma_hooks and kc in dma_hooks:
                    dma_hooks[kc]()
                xts.append(xt)

            def qk_unit(w_sb, dst_sb, hf):
                # two ~850ns halves so filler interleaves finely with waves
                t_ = t0 + hf
                state = {}
                def emit_a():
                    ps = mm_pool.tile([128, CQ], F32, tag="mm",
                                      name=f"qkps{t_}_{id(w_sb)}")
                    state["ps"] = ps
                    for kc in range(KC // 2):
                        nc.tensor.matmul(
                            ps[:], w_sb[:, kc],
                            xts[kc][:, hf * CQ:(hf + 1) * CQ],
                            start=(kc == 0), stop=False)
                def emit_b():
                    ps = state["ps"]
                    for kc in range(KC // 2, KC):
                        nc.tensor.matmul(
                            ps[:], w_sb[:, kc],
                            xts[kc][:, hf * CQ:(hf + 1) * CQ],
                            start=False, stop=(kc == KC - 1))
                    nc.vector.tensor_copy(
                        dst_sb[:, t_ * CQ:(t_ + 1) * CQ], ps[:])
                return [emit_a, emit_b]

            def v_unit(hf):
                t_ = t0 + hf
                state = {}
                def emit_a():
                    v_ps = mm_pool.tile([128, CQ], F32, tag="mm",
                                        name=f"vps{t_}")
                    state["ps"] = v_ps
                    for j in (0, 1):
                        jf = hf * CQ + j * 128
                        for kc in range(KC):
                            nc.tensor.matmul(
                                v_ps[:, j * 128:(j + 1) * 128],
                                xts[kc][:, jf:jf + 128],
                                wv_sb[:, kc], start=(kc == 0),
                                stop=(kc == KC - 1))
                def emit_b():
                    v_ps = state["ps"]
                    for j in (2, 3):
                        jf = hf * CQ + j * 128
                        for kc in range(KC):
                            nc.tensor.matmul(
                                v_ps[:, j * 128:(j + 1) * 128],
                                xts[kc][:, jf:jf + 128],
                                wv_sb[:, kc], start=(kc == 0),
                                stop=(kc == KC - 1))
                    b4 = t_ * (CQ // 128)
                    nc.vector.tensor_copy(
                        v_sb[:, b4:b4 + 4, :, 0:64],
                        v_ps[:].rearrange("p (j h v) -> p j h v",
                                          j=4, h=2))
                return [emit_a, emit_b]

            units = (qk_unit(wq_sb, qt_sb, 0) + qk_unit(wk_sb, kt_sb, 0) +
                     v_unit(0) + qk_unit(wq_sb, qt_sb, 1) +
                     qk_unit(wk_sb, kt_sb, 1) + v_unit(1))
            return units, xts

        # ---- filler machinery -------------------------------------------
        # proj_q entries are (deadline_slot, seq, [piece_a, piece_b]): the
        # unit MUST be emitted before the global attention slot that
        # consumes its tokens (a later emission would deadlock the in-order
        # PE queue). Kept sorted by deadline so deficit-paced pops
        # naturally preserve the latest-deadline units as an endgame
        # reserve.
        tails_q = deque()
        pending_cs = []    # tail C (out-proj) pieces deferred one pair
        proj_q = []
        held = []          # pending b-half of a split proj unit (must pop
                           # before any other mm-pool user)
        sched = {"deficit": 0.0, "seq": 0}
        dma_pending = []   # (dram_slice, sbuf_tile): out DMAs deferred one
                           # tail so the SP queue never stalls on copy sems

        PROJ_NS = 853.0    # PE ns per proj half-piece (4 matmuls x 512)

        def flush_out_dma():
            while dma_pending:
                dst, src = dma_pending.pop(0)
                nc.sync.dma_start(dst, src)

        def queue_unit(dead, pieces):
            proj_q.append((dead, sched["seq"], pieces))
            sched["seq"] += 1
            proj_q.sort(key=lambda e: (e[0], e[1]))

        def pop_proj_piece():
            if held:
                held.pop()()
            else:
                _, _, pieces = proj_q.pop(0)
                pieces[0]()
                held.append(pieces[1])
            sched["deficit"] -= PROJ_NS

        def drain_tails():
            # interleave a proj piece between tail pieces so their
            # cross-engine latency chains overlap real PE work
            tails_q.extend(pending_cs)
            pending_cs.clear()
            while tails_q:
                pe_ns, fn = tails_q.popleft()
                fn()
                sched["deficit"] -= pe_ns
                if tails_q and (held or proj_q) and sched["deficit"] > -800:
                    pop_proj_piece()

        def force_proj_upto(slot):
            while held or (proj_q and proj_q[0][0] <= slot):
                pop_proj_piece()

        # ---- prologue ----------------------------------------------------
        # Weight DMAs slot between the pair-0 xT DMAs (wk after xt0, wv
        # after xt2 — each lands just before its first consumer) and
        # q/k/v matmuls interleave per-kc at xT arrival granularity so the
        # PE streams at DMA rate with no burst stalls.
        hooks = {
            0: lambda: (nc.sync.dma_start(wq_sb[:, KC // 2:KC],
                                          wq_r[:, KC // 2:KC]),
                        nc.sync.dma_start(
                wk_sb[:], wk.rearrange("p (kc d) -> p kc d", kc=KC))),
            2: lambda: nc.sync.dma_start(
                wv_sb[:], wv.rearrange("p (kc d) -> p kc d", kc=KC)),
            7: lambda: nc.sync.dma_start(wo_sb[:], wo[:]),
        }
        units0, xts0 = proj_pair_units(0, dma_hooks=hooks)

        # tokens 0..511: q/k psum tiles borrow the (still idle) sc tag's
        # banks so mm_pool stays free for the interleaved v chains.
        q_ps0 = sc_pool.tile([128, CQ], F32, tag="sc", name="qps_pro")
        k_ps0 = sc_pool.tile([128, CQ], F32, tag="sc", name="kps_pro")
        v_ps0 = mm_pool.tile([128, CQ], F32, tag="mm", name="vps_pro")
        v_started_cell = [False]

        def pro_v(kc):
            # interleaved per-j chains on one bank: only the very first
            # matmul clears the bank's has_written bits (start=True); the
            # other chains' kc==0 matmuls overwrite-where-bit-clear
            for j in range(4):
                nc.tensor.matmul(
                    v_ps0[:, j * 128:(j + 1) * 128],
                    xts0[kc][:, j * 128:(j + 1) * 128],
                    wv_sb[:, kc], start=not v_started_cell[0],
                    stop=(kc == KC - 1), skip_group_check=True)
                v_started_cell[0] = True
        for kc in range(KC):
            nc.tensor.matmul(q_ps0[:], wq_sb[:, kc], xts0[kc][:, 0:CQ],
                             start=(kc == 0), stop=(kc == KC - 1),
                             skip_group_check=True)
            nc.tensor.matmul(k_ps0[:], wk_sb[:, kc], xts0[kc][:, 0:CQ],
                             start=(kc == 0), stop=(kc == KC - 1),
                             skip_group_check=True)
            if kc >= 2:
                pro_v(kc - 2)
        # Act is idle until the first exp (~12us): give it the prologue
        # landing copies so DVE stays clear for the hf=1 unit copies
        nc.scalar.copy(qt_sb[:, 0:CQ], q_ps0[:])
        nc.scalar.copy(kt_sb[:, 0:CQ], k_ps0[:])
        for kc in range(KC - 2, KC):
            pro_v(kc)
        nc.scalar.copy(
            v_sb[:, 0:4, :, 0:64],
            v_ps0[:].rearrange("p (j h v) -> p j h v", j=4, h=2))

        # Chunk order: batch-1 runs [c1, c2, c0, c3] so the kernel ends on
        # the 16-block (1,3) chunk, whose late score blocks (kb>=12) keep
        # q/k/v(t7) units as deadline-reserved PE filler for the Act-paced
        # endgame, instead of draining tails against an empty proj queue.
        chunk_list = [(0, 0), (0, 1), (0, 2), (0, 3),
                      (1, 1), (1, 2), (1, 0), (1, 3)]
        nblks = [4 * (cc + 1) for _, cc in chunk_list]
        base = [0]
        for n in nblks:
            base.append(base[-1] + n)

        # global-slot deadlines: q(t) needed at its chunk's first slot;
        # k/v(t) first consumed when the score wave reaches keys t (slot
        # 4*(t%4) of the earliest chunk with c >= t%4 in list order)
        Q_DEAD = {1: base[1], 2: base[2], 3: base[3],
                  4: base[6], 5: base[4], 6: base[5],
                  7: base[5] + 4}  # q(t7) before the (1,3) pre-scoring
        KV_DEAD = {1: base[1] + 4, 2: base[2] + 8, 3: base[3] + 12,
                   4: base[4], 5: base[4] + 4, 6: base[5] + 8,
                   7: base[7] + 12}

        # v(t7) reserved two slots past k(t7): it pads the endgame's
        # diagonal-score bank rotation and is forced before pv(12) uses it
        V_DEAD = dict(KV_DEAD)
        V_DEAD[7] = base[7] + 14

        def queue_half_units(t, units6):
            queue_unit(Q_DEAD[t], units6[0:2])    # q a/b
            queue_unit(KV_DEAD[t], units6[2:4])   # k a/b
            queue_unit(V_DEAD[t], units6[4:6])    # v a/b

        def queue_pair_units(t0, units):
            queue_half_units(t0, units[:6])
            queue_half_units(t0 + 1, units[6:])

        queue_half_units(1, units0[6:])  # prologue pair: hf=1 only

        # pair creation: issue xT DMAs early — critically, ALL xt trains
        # must enter the in-order SP queue before any mid-kernel out-DMA
        # can park it (an out-DMA whose copy isn't ready blocks the queue
        # for many us, which starved later projections via late xts)
        pair_create = {1: [2], 2: [4, 6]}

        # pt tiles on demand so a later chunk's score wave can start while
        # an earlier chunk is still draining (pre-scoring)
        pts = {}

        def get_pt(b_, c_):
            if (b_, c_) not in pts:
                pts[(b_, c_)] = pt_pool.tile(
                    [128, NKB, 2, CQ], BF16, tag="pt", name=f"pt_{b_}_{c_}")
            return pts[(b_, c_)]

        def emit_scores_g(b_, c_, kb):
            pt = get_pt(b_, c_)
            tb_ = b_ * T
            tq0_ = c_ * CQ
            f0 = max(0, 128 * (kb - 4 * c_))
            sc = sc_pool.tile([128, 2, CQ], F32, tag="sc",
                              name=f"sc_{b_}_{c_}_{kb}")
            tk0 = kb * 128
            for h in range(2):
                hs = slice(h * 64, (h + 1) * 64)
                nc.tensor.matmul(
                    sc[:, h, f0:CQ],
                    kt_sb[hs, tb_ + tk0:tb_ + tk0 + 128],
                    qt_sb[hs, tb_ + tq0_ + f0:tb_ + tq0_ + CQ],
                    start=True, stop=True)
            nc.scalar.activation(
                pt[:, kb, :, f0:CQ], sc[:, :, f0:CQ],
                AF.Exp, scale=float(D) ** -0.5)
            if kb - 4 * c_ >= 0:  # diagonal block: causal triangle mask
                for h in range(2):
                    # keep where tq >= tk (f - p >= 0), else 0
                    nc.gpsimd.affine_select(
                        out=pt[:, kb, h, f0:f0 + 128],
                        in_=pt[:, kb, h, f0:f0 + 128],
                        compare_op=mybir.AluOpType.is_ge,
                        fill=0.0, base=0,
                        pattern=[[1, 128]], channel_multiplier=-1)
            return f0

        NPRE = 8   # (1,3) blocks pre-scored during (1,2)+(1,0)

        for ci, (b, c) in enumerate(chunk_list):
            for t0 in pair_create.get(ci, []):
                units, _ = proj_pair_units(t0)
                queue_pair_units(t0, units)

            tb = b * T
            tq0 = c * CQ
            nblk = 4 * (c + 1)
            pt = get_pt(b, c)
            # per-chunk O|Z accumulators [tq, gsub, h, d|Z]: pool rotation
            # (bufs=1) orders the next chunk's first PV write after this
            # chunk's tail reads
            o_ps = [acc_pool.tile([128, 2, 2, 65], F32, tag=f"o{i}",
                                  name=f"o_ps{i}_{b}_{c}")
                    for i in range(2)]
            zr_tiles = {}

            def make_tail_pieces(g, b=b, c=c, tb=tb, tq0=tq0, o_ps=o_ps):
                """Tail split into 3 pieces so the PE->DVE->PE->DVE chain of
                one tail interleaves with its pair partner + proj filler
                instead of stalling the in-order PE stream. In the final
                chunk the Act engine (done with exps by tail time) takes
                half the copies so DVE isn't the serial drain resource."""
                op = o_ps[g // 2]
                gs = g % 2
                act_assist = (b, c) == chunk_list[-1]
                state = {}

                def piece_a():   # normalize + transpose (PE 53ns)
                    if act_assist:
                        # 1/Z lands in SBUF so the Act engine can use it as
                        # an activation scale (scale APs must be SBUF)
                        if gs == 0:
                            zr = osb_pool.tile([128, 2, 2], F32, tag="zr",
                                               name=f"zr_{b}_{c}_{g}")
                            nc.vector.reciprocal(zr[:], op[:, :, :, 64])
                            zr_tiles[g // 2] = zr
                        zr = zr_tiles[g // 2]
                    elif gs == 0:
                        # 1/Z for the group pair, in place in PSUM col 64
                        # (both chains have stopped by emission time)
                        nc.vector.reciprocal(op[:, :, :, 64],
                                             op[:, :, :, 64])
                    o_sb = osb_pool.tile([128, 128], BF16, tag="osb",
                                         name=f"osb_{b}_{c}_{g}")
                    for h in range(2):
                        if act_assist and h == 1:
                            nc.scalar.activation(
                                o_sb[:, 64:128], op[:, gs, 1, 0:64],
                                AF.Copy, scale=zr[:, gs, 1:2])
                        elif act_assist:
                            nc.vector.tensor_scalar_mul(
                                o_sb[:, h * 64:(h + 1) * 64],
                                op[:, gs, h, 0:64],
                                zr[:, gs, h:h + 1])
                        else:
                            nc.vector.tensor_scalar_mul(
                                o_sb[:, h * 64:(h + 1) * 64],
                                op[:, gs, h, 0:64],
                                op[:, gs, h, 64:65])
                    tp = mm_pool.tile([128, 512], BF16, tag="mm",
                                      name=f"tp_{b}_{c}_{g}")
                    nc.tensor.transpose(tp[:, 0:128], o_sb[:], eye_sb[:])
                    state["tp"] = tp

                def piece_b():   # O^T landing copy (no PE)
                    ot_sb = otsb_pool.tile([128, 128], BF16, tag="otsb",
                                           name=f"otsb_{b}_{c}_{g}")
                    if act_assist:
                        nc.scalar.copy(ot_sb[:], state["tp"][:, 0:128])
                    else:
                        nc.vector.tensor_copy(ot_sb[:], state["tp"][:, 0:128])
                    state["ot"] = ot_sb

                def piece_c():   # output projection + copies + DMA (PE 426)
                    out_sb = outsb_pool.tile([128, E], BF16, tag="outsb",
                                             name=f"outsb_{b}_{c}_{g}")
                    tqg = tb + tq0 + g * 128
                    for eh in range(2):
                        ops = mm_pool.tile([128, 512], F32, tag="mm",
                                           name=f"ops_{b}_{c}_{g}_{eh}")
                        nc.tensor.matmul(
                            ops[:], state["ot"][:],
                            wo_sb[:, eh * 512:(eh + 1) * 512],
                            start=True, stop=True)
                        if ((b == 0 and c == 0) or act_assist) and eh == 1:
                            # Act's light window (short chunks / endgame)
                            nc.scalar.copy(
                                out_sb[:, eh * 512:(eh + 1) * 512], ops[:])
                        else:
                            nc.vector.tensor_copy(
                                out_sb[:, eh * 512:(eh + 1) * 512], ops[:])
                        if act_assist:
                            # endgame: SP is idle — issue half-row DMAs the
                            # moment each copy lands to shorten the drain
                            nc.sync.dma_start(
                                out[tqg:tqg + 128,
                                    eh * 512:(eh + 1) * 512],
                                out_sb[:, eh * 512:(eh + 1) * 512])
                    if not act_assist:
                        flush_out_dma()
                        dma_pending.append(
                            (out[tqg:tqg + 128, :], out_sb[:]))

                return [(53.0, piece_a), (0.0, piece_b), (426.0, piece_c)]

            # PSUM has_written bits: a start=True matmul clears them for the
            # WHOLE bank, so only the first PV matmul per o_ps bank per chunk
            # may use start=True. Later chains' first matmuls (kb==0,
            # start=False) overwrite-where-bit-clear, then accumulate.
            bank_started = [False, False]

            def pv_block(kb, b=b, c=c, pt=pt, o_ps=o_ps,
                         bank_started=bank_started):
                j0 = max(0, kb - 4 * c)
                # diagonal block: group j0's stationary is the masked pt
                # sub-block — emit it LAST so the gpsimd mask only gates the
                # final chain-stop matmul, not the whole block
                gs_order = list(range(j0, NQB))
                if kb - 4 * c >= 0 and len(gs_order) > 1:
                    gs_order = gs_order[1:] + gs_order[:1]
                for g in gs_order:
                    for h in range(2):
                        st = not bank_started[g // 2]
                        bank_started[g // 2] = True
                        nc.tensor.matmul(
                            o_ps[g // 2][:, g % 2, h, :],
                            pt[:, kb, h, g * 128:(g + 1) * 128],
                            v_sb[:, b * NKB + kb, h],
                            start=st, stop=(kb == 4 * c + g),
                            skip_group_check=True)
                j = kb - 4 * c
                if j in (1, 3):  # group pair's chains complete
                    pa = make_tail_pieces(j - 1)
                    pb = make_tail_pieces(j)
                    # interleave A A' B B' now; defer the C (out-proj)
                    # pieces until the NEXT pair so piece_b's DVE copy has
                    # landed long before C's Ldweights needs it
                    inter = [pa[0]]
                    if pending_cs:
                        inter.append(pending_cs.pop(0))
                    inter.append(pb[0])
                    if pending_cs:
                        inter.append(pending_cs.pop(0))
                    inter += [pa[1], pb[1]]
                    tails_q.extend(inter)
                    pending_cs.extend([pa[2], pb[2]])
                return (NQB - j0) * 2 * 65

            def emit_scores(kb):
                return emit_scores_g(b, c, kb)

            def pace(act_ns, pe_ns, slot):
                # deficit-paced filler: keep the PE fed during Act-paced
                # stretches, spend queued proj/tail work exactly where the
                # per-block PE emission falls short of the exp pace.
                sched["deficit"] += act_ns - pe_ns
                sched["deficit"] = max(-2000.0,
                                       min(sched["deficit"], 8000.0))
                # lookahead spread: don't let deadline-bound units burst
                if proj_q and proj_q[0][0] <= slot + 3:
                    pop_proj_piece()
                    if held:
                        pop_proj_piece()
                # alternate tail/proj pops so tail latency chains overlap
                # real PE work instead of stalling the in-order PE stream
                prefer_tail = True
                while sched["deficit"] > 400 and (tails_q or held or proj_q):
                    if prefer_tail and tails_q and not held:
                        pe_ns2, fn = tails_q.popleft()
                        fn()
                        sched["deficit"] -= pe_ns2
                    elif held or proj_q:
                        pop_proj_piece()
                    else:
                        pe_ns2, fn = tails_q.popleft()
                        fn()
                        sched["deficit"] -= pe_ns2
                    prefer_tail = not prefer_tail
                if held:   # never end a slot mid-unit
                    pop_proj_piece()

            def chunk_prefix(kb):
                if kb == 1:
                    # PE meat between sc(0)/exp(0) and the exp-gated
                    # pv(0), then the prev chunk's tail pieces
                    if held or proj_q:
                        pop_proj_piece()
                    drain_tails()

            if ci < len(chunk_list) - 1:
                for kb in range(nblk):
                    # units whose tokens this slot consumes: emit them now
                    force_proj_upto(base[ci] + kb)
                    f0 = emit_scores(kb)
                    pv_cyc = 0
                    if kb >= 1:
                        chunk_prefix(kb)
                        if kb >= 2:
                            pv_cyc = pv_block(kb - 2)
                    if ci == 5 and kb >= nblk - 4:
                        # pre-score an off-diagonal (1,3) block right after
                        # the host scores so Act sees it ASAP: shifts Act
                        # load out of the Act-saturated endgame
                        emit_scores_g(1, 3, kb - (nblk - 4))
                    pace((2 * (CQ - f0) + 222) / 1.2,
                         (2 * (CQ - f0) + pv_cyc) * 0.4167, base[ci] + kb)
                    if ci == 6 and NPRE == 8:
                        # four more during the Act-light (1,0) chunk
                        emit_scores_g(1, 3, 4 + kb)
                        pace((2 * CQ + 222) / 1.2, 2 * CQ * 0.4167,
                             base[ci] + kb)
                pv_block(nblk - 2)
                pv_block(nblk - 1)
            else:
                # Final chunk, two-phase so the kernel does not end on an
                # Act-bound exp wave:
                # phase A pre-scores the 12 off-diagonal blocks (Act paced,
                # PE kept busy by the deadline-reserved proj filler);
                # phase B runs the diagonal scores + every PV wave + tails
                # with all exps already in flight or done.
                for kb in range(NPRE, 12):
                    force_proj_upto(base[ci] + kb)
                    emit_scores(kb)
                    if kb == NPRE + 1:
                        # prev chunk's tails must fully emit before phase
                        # B's pv(0) rotates into its o_ps banks
                        if held or proj_q:
                            pop_proj_piece()
                        drain_tails()
                    pace((2 * CQ + 222) / 1.2, 2 * CQ * 0.4167,
                         base[ci] + kb)
                force_proj_upto(base[ci] + 12)  # k(t7) ahead of the scores
                emit_scores(12)
                emit_scores(13)
                for kb in range(0, 4):
                    pv_block(kb)
                if held or proj_q:   # v(t7) a: pads exp(12)'s bank WAR
                    pop_proj_piece()
                emit_scores(14)
                for kb in range(4, 8):
                    pv_block(kb)
                if held or proj_q:   # v(t7) b: pads exp(13)'s bank WAR
                    pop_proj_piece()
                emit_scores(15)
                force_proj_upto(base[ci] + 14)  # v(t7) before pv(12)
                for kb in range(8, 16):
                    pv_block(kb)

        drain_tails()
        while held or proj_q:
            pop_proj_piece()
        flush_out_dma()

    nc.compile()
    return nc


def _host_prep(x, Wq, Wk, Wv, Wo):
    bf = ml_dtypes.bfloat16
    xT = np.ascontiguousarray(
        np.asarray(x, dtype=np.float32).reshape(BT, E).T).astype(bf)

    def perm(w):
        # [E, 128] -> [128p, kc, 128d] flattened: w[kc*128+p, d] -> out[p, kc, d]
        return np.ascontiguousarray(
            w.reshape(KC, 128, 128).transpose(1, 0, 2).reshape(128, E)).astype(bf)

    Wq = np.asarray(Wq, dtype=np.float32)
    Wk = np.asarray(Wk, dtype=np.float32)
    Wv = np.asarray(Wv, dtype=np.float32)
    Wo = np.asarray(Wo, dtype=np.float32)

    in_maps = []
    for c in range(NCORE):
        sl = slice(c * 128, (c + 1) * 128)
        in_maps.append({
            "xT": xT,
            "wq": perm(Wq[:, sl]),
            "wk": perm(Wk[:, sl]),
            "wv": perm(Wv[:, sl]),
            "wo": np.ascontiguousarray(Wo[sl, :]).astype(bf),
        })
    return in_maps


def kernel(x, Wq, Wk, Wv, Wo, bo, _trace=False, _trace_kwargs=None):
    if "nc" not in _cache:
        _cache["nc"] = _build()
    nc = _cache["nc"]

    in_maps = _host_prep(x, Wq, Wk, Wv, Wo)
    kw = {}
    if _trace:
        kw = dict(trace=True, trace_cores=[0], **(_trace_kwargs or {}))
    res = run_bass_kernel_spmd(nc, in_maps, core_ids=list(range(NCORE)), **kw)
    _cache["last_result"] = res

    total = np.zeros((BT, E), dtype=np.float32)
    for r in res.results:
        total += np.asarray(r["out"], dtype=np.float32)
    total += np.asarray(bo, dtype=np.float32)[None, :]
    return total.reshape(B, T, E)



# revision 95
# speedup vs baseline: 1.0196x; 1.0013x over previous
"""Multi-head causal attention (B=2, T=2048, E=1024, H=16, D=64) on 8 trn2 cores.

Sharding: tensor-parallel over heads — core c owns heads {2c, 2c+1} (a 128-wide
slice of the hidden dim). Each core computes q/k/v projections for its heads
over the full sequence, causal attention, and a partial output projection
(contraction over its 128 rows of Wo). The host sums the 8 bf16 partials + bias.

v4 (128.2us, from the 150.5us v2), rebuilt around TimelineSim gap blame.
The engine floor is PE ~102us (proj 41 + scores 29 + PV 14.7 + out-proj
13.7 + transposes) with Act exp ~88us; everything else is scheduling:
 - In-place PSUM reciprocal of the Z column + bf16 o_sb/eye/tp (1 cyc/row
   transpose); PV groups emit mask-dependent-group last so the gpsimd
   affine_select triangle mask only gates the final chain-stop matmul.
 - Tails split into 3 pieces (normalize+transpose / O^T copy / out-proj+
   DMA), pair-interleaved, with the C pieces deferred one tail pair so
   every cross-engine hop has real PE work between emit and consume.
 - Proj units carry (deadline=global slot, pieces) and live in a sorted
   queue; deficit + lookahead pacing spends them where the exp pace
   outruns PE, preserving late-deadline units (q/k/v of t7) as endgame
   filler. The prologue hand-interleaves q/k/v per-kc at xT arrival rate
   with weight DMAs slotted between xts, q/k psum borrowed from the idle
   sc banks, and landing copies on the (idle) Act engine.
 - Chunk order (0,*), (1,1), (1,2), (1,0), (1,3): 8 of (1,3)'s off-diag
   score blocks are pre-scored during (1,2)/(1,0) (pt bufs=3) and its
   own wave is two-phase — off-diag scores first (Act-paced, filler-fed),
   then diag scores + all PV waves + tails with every exp in flight, so
   the kernel does not end on an Act-bound stretch. Endgame tails use
   Act-assisted copies (Act is exp-free by then) + immediate out DMAs;
   mid-kernel Act borrows copies only in (0,0)/(0,1)-adjacent windows
   where it is measurably idle. Tail pools run 6 bufs deep.
 - PV emission lags scores by 2 blocks; all xt DMA trains enter the
   in-order SP queue before any out-DMA can park it.

PSUM (8 banks): sc 2x[128,2,512] (4) + O|Z accumulators 2x[128,2,2,65] (2) +
mm [128,512] x2 (2, shared by proj / out-proj / transpose tiles).

Timing signal is concourse TimelineSim (no NTFF under this axon client).
"""

import numpy as np
import ml_dtypes
from collections import deque

import concourse.bass as bass
import concourse.tile as tile
from concourse import bacc, mybir
from concourse.bass_utils import run_bass_kernel_spmd
from concourse.masks import make_identity
from contextlib import ExitStack

B, T, E, H, D = 2, 2048, 1024, 16, 64
BT = B * T            # 4096 tokens total
NCORE = 8
KC = E // 128         # contraction chunks for projections = 8
CQ = 512              # tq chunk width
NQB = T // CQ         # tq chunks per batch = 4
NKB = T // 128        # tk blocks per batch = 16

F32 = mybir.dt.float32
BF16 = mybir.dt.bfloat16
AF = mybir.ActivationFunctionType

_cache = {}


def _build():
    nc = bacc.Bacc("TRN2", target_bir_lowering=False, debug=False,
                   num_devices=NCORE)

    xT = nc.dram_tensor("xT", [E, BT], BF16, kind="ExternalInput").ap()
    wq = nc.dram_tensor("wq", [128, E], BF16, kind="ExternalInput").ap()
    wk = nc.dram_tensor("wk", [128, E], BF16, kind="ExternalInput").ap()
    wv = nc.dram_tensor("wv", [128, E], BF16, kind="ExternalInput").ap()
    wo = nc.dram_tensor("wo", [128, E], BF16, kind="ExternalInput").ap()
    out = nc.dram_tensor("out", [BT, E], BF16, kind="ExternalOutput").ap()

    with tile.TileContext(nc) as tc, ExitStack() as ctx:
        pers = ctx.enter_context(tc.tile_pool(name="pers", bufs=1))

        wq_sb = pers.tile([128, KC, 128], BF16, tag="wq")
        wk_sb = pers.tile([128, KC, 128], BF16, tag="wk")
        wv_sb = pers.tile([128, KC, 128], BF16, tag="wv")
        wo_sb = pers.tile([128, E], BF16, tag="wo")
        eye_sb = pers.tile([128, 128], BF16, tag="eye")
        qt_sb = pers.tile([128, BT], BF16, tag="qt")    # [dims(2 heads), tok]
        kt_sb = pers.tile([128, BT], BF16, tag="kt")
        # V natural + ones col per head: [tok%128, blk, h, d|1]; the ones
        # column makes the flipped P^T-stationary PV matmul emit Z = sum(exp)
        # as output column 64 for free.
        v_sb = pers.tile([128, BT // 128, 2, 65], BF16, tag="v")

        # wq queued first on the sync HWDGE queue so the first projection
        # matmul gates on as little DMA as possible; each extra DMA costs
        # ~625ns of serial HWDGE hold, so weights go as single transfers
        # slotted between the xts that need them.
        wq_r = wq.rearrange("p (kc d) -> p kc d", kc=KC)
        nc.sync.dma_start(wq_sb[:, 0:KC // 2], wq_r[:, 0:KC // 2])
        nc.vector.memset(v_sb[:, :, :, 64:65], 1.0)
        make_identity(nc, eye_sb[:])

        # SBUF pools
        xts_pool = ctx.enter_context(tc.tile_pool(name="xts", bufs=32))
        pt_pool = ctx.enter_context(tc.tile_pool(name="pt", bufs=2))
        osb_pool = ctx.enter_context(tc.tile_pool(name="osb", bufs=6))
        otsb_pool = ctx.enter_context(tc.tile_pool(name="otsb", bufs=6))
        outsb_pool = ctx.enter_context(tc.tile_pool(name="outsb", bufs=6))

        # PSUM pools: 4 + 2 + 2 = 8 banks
        sc_pool = ctx.enter_context(tc.tile_pool(name="sc", bufs=2,
                                                 space="PSUM"))
        acc_pool = ctx.enter_context(tc.tile_pool(name="acc", bufs=1,
                                                  space="PSUM"))
        mm_pool = ctx.enter_context(tc.tile_pool(name="mm", bufs=2,
                                                 space="PSUM"))


        # ---- projection units -------------------------------------------
        def proj_pair_units(t0, dma_hooks=None):
            """t0: even 512-token chunk index (0..6). Issues the pair's xT
            DMAs now; returns 6 unit callbacks (q,k,v) x (hf 0,1).
            dma_hooks: {kc: callback} run right after that kc's xt DMA is
            queued (prologue interleaves weight DMAs at specific points)."""
            xts = []
            for kc in range(KC):
                xt = xts_pool.tile([128, 2 * CQ], BF16, tag="xt",
                                   name=f"xt_{t0}_{kc}")
                nc.sync.dma_start(
                    xt[:], xT[kc * 128:(kc + 1) * 128,
                              t0 * CQ:(t0 + 2) * CQ])
                if dma_hooks and kc in dma_hooks:
                    dma_hooks[kc]()
                xts.append(xt)

            def qk_unit(w_sb, dst_sb, hf):
                # two ~850ns halves so filler interleaves finely with waves
                t_ = t0 + hf
                state = {}
                def emit_a():
                    ps = mm_pool.tile([128, CQ], F32, tag="mm",
                                      name=f"qkps{t_}_{id(w_sb)}")
                    state["ps"] = ps
                    for kc in range(KC // 2):
                        nc.tensor.matmul(
                            ps[:], w_sb[:, kc],
                            xts[kc][:, hf * CQ:(hf + 1) * CQ],
                            start=(kc == 0), stop=False)
                def emit_b():
                    ps = state["ps"]
                    for kc in range(KC // 2, KC):
                        nc.tensor.matmul(
                            ps[:], w_sb[:, kc],
                            xts[kc][:, hf * CQ:(hf + 1) * CQ],
                            start=False, stop=(kc == KC - 1))
                    nc.vector.tensor_copy(
                        dst_sb[:, t_ * CQ:(t_ + 1) * CQ], ps[:])
                return [emit_a, emit_b]

            def v_unit(hf):
                t_ = t0 + hf
                state = {}
                def emit_a():
                    v_ps = mm_pool.tile([128, CQ], F32, tag="mm",
                                        name=f"vps{t_}")
                    state["ps"] = v_ps
                    for j in (0, 1):
                        jf = hf * CQ + j * 128
                        for kc in range(KC):
                            nc.tensor.matmul(
                                v_ps[:, j * 128:(j + 1) * 128],
                                xts[kc][:, jf:jf + 128],
                                wv_sb[:, kc], start=(kc == 0),
                                stop=(kc == KC - 1))
                def emit_b():
                    v_ps = state["ps"]
                    for j in (2, 3):
                        jf = hf * CQ + j * 128
                        for kc in range(KC):
                            nc.tensor.matmul(
                                v_ps[:, j * 128:(j + 1) * 128],
                                xts[kc][:, jf:jf + 128],
                                wv_sb[:, kc], start=(kc == 0),
                                stop=(kc == KC - 1))
                    b4 = t_ * (CQ // 128)
                    nc.vector.tensor_copy(
                        v_sb[:, b4:b4 + 4, :, 0:64],
                        v_ps[:].rearrange("p (j h v) -> p j h v",
                                          j=4, h=2))
                return [emit_a, emit_b]

            units = (qk_unit(wq_sb, qt_sb, 0) + qk_unit(wk_sb, kt_sb, 0) +
                     v_unit(0) + qk_unit(wq_sb, qt_sb, 1) +
                     qk_unit(wk_sb, kt_sb, 1) + v_unit(1))
            return units, xts

        # ---- filler machinery -------------------------------------------
        # proj_q entries are (deadline_slot, seq, [piece_a, piece_b]): the
        # unit MUST be emitted before the global attention slot that
        # consumes its tokens (a later emission would deadlock the in-order
        # PE queue). Kept sorted by deadline so deficit-paced pops
        # naturally preserve the latest-deadline units as an endgame
        # reserve.
        tails_q = deque()
        pending_cs = []    # tail C (out-proj) pieces deferred one pair
        proj_q = []
        held = []          # pending b-half of a split proj unit (must pop
                           # before any other mm-pool user)
        sched = {"deficit": 0.0, "seq": 0}
        dma_pending = []   # (dram_slice, sbuf_tile): out DMAs deferred one
                           # tail so the SP queue never stalls on copy sems

        PROJ_NS = 853.0    # PE ns per proj half-piece (4 matmuls x 512)

        def flush_out_dma():
            while dma_pending:
                dst, src = dma_pending.pop(0)
                nc.sync.dma_start(dst, src)

        def queue_unit(dead, pieces):
            proj_q.append((dead, sched["seq"], pieces))
            sched["seq"] += 1
            proj_q.sort(key=lambda e: (e[0], e[1]))

        def pop_proj_piece():
            if held:
                held.pop()()
            else:
                _, _, pieces = proj_q.pop(0)
                pieces[0]()
                held.append(pieces[1])
            sched["deficit"] -= PROJ_NS

        def drain_tails():
            # interleave a proj piece between tail pieces so their
            # cross-engine latency chains overlap real PE work
            tails_q.extend(pending_cs)
            pending_cs.clear()
            while tails_q:
                pe_ns, fn = tails_q.popleft()
                fn()
                sched["deficit"] -= pe_ns
                if tails_q and (held or proj_q) and sched["deficit"] > -800:
                    pop_proj_piece()

        def force_proj_upto(slot):
            while held or (proj_q and proj_q[0][0] <= slot):
                pop_proj_piece()

        # ---- prologue ----------------------------------------------------
        # Weight DMAs slot between the pair-0 xT DMAs (wk after xt0, wv
        # after xt2 — each lands just before its first consumer) and
        # q/k/v matmuls interleave per-kc at xT arrival granularity so the
        # PE streams at DMA rate with no burst stalls.
        hooks = {
            0: lambda: (nc.sync.dma_start(wq_sb[:, KC // 2:KC],
                                          wq_r[:, KC // 2:KC]),
                        nc.sync.dma_start(
                wk_sb[:], wk.rearrange("p (kc d) -> p kc d", kc=KC))),
            2: lambda: nc.sync.dma_start(
                wv_sb[:], wv.rearrange("p (kc d) -> p kc d", kc=KC)),
            7: lambda: nc.sync.dma_start(wo_sb[:], wo[:]),
        }
        units0, xts0 = proj_pair_units(0, dma_hooks=hooks)

        # tokens 0..511: q/k psum tiles borrow the (still idle) sc tag's
        # banks so mm_pool stays free for the interleaved v chains.
        q_ps0 = sc_pool.tile([128, CQ], F32, tag="sc", name="qps_pro")
        k_ps0 = sc_pool.tile([128, CQ], F32, tag="sc", name="kps_pro")
        v_ps0 = mm_pool.tile([128, CQ], F32, tag="mm", name="vps_pro")
        v_started_cell = [False]

        def pro_v(kc):
            # interleaved per-j chains on one bank: only the very first
            # matmul clears the bank's has_written bits (start=True); the
            # other chains' kc==0 matmuls overwrite-where-bit-clear
            for j in range(4):
                nc.tensor.matmul(
                    v_ps0[:, j * 128:(j + 1) * 128],
                    xts0[kc][:, j * 128:(j + 1) * 128],
                    wv_sb[:, kc], start=not v_started_cell[0],
                    stop=(kc == KC - 1), skip_group_check=True)
                v_started_cell[0] = True
        for kc in range(KC):
            nc.tensor.matmul(q_ps0[:], wq_sb[:, kc], xts0[kc][:, 0:CQ],
                             start=(kc == 0), stop=(kc == KC - 1),
                             skip_group_check=True)
            nc.tensor.matmul(k_ps0[:], wk_sb[:, kc], xts0[kc][:, 0:CQ],
                             start=(kc == 0), stop=(kc == KC - 1),
                             skip_group_check=True)
            if kc >= 2:
                pro_v(kc - 2)
        # Act is idle until the first exp (~12us): give it the prologue
        # landing copies so DVE stays clear for the hf=1 unit copies
        nc.scalar.copy(qt_sb[:, 0:CQ], q_ps0[:])
        nc.scalar.copy(kt_sb[:, 0:CQ], k_ps0[:])
        for kc in range(KC - 2, KC):
            pro_v(kc)
        nc.scalar.copy(
            v_sb[:, 0:4, :, 0:64],
            v_ps0[:].rearrange("p (j h v) -> p j h v", j=4, h=2))

        # Chunk order: batch-1 runs [c1, c2, c0, c3] so the kernel ends on
        # the 16-block (1,3) chunk, whose late score blocks (kb>=12) keep
        # q/k/v(t7) units as deadline-reserved PE filler for the Act-paced
        # endgame, instead of draining tails against an empty proj queue.
        chunk_list = [(0, 0), (0, 1), (0, 2), (0, 3),
                      (1, 1), (1, 2), (1, 0), (1, 3)]
        nblks = [4 * (cc + 1) for _, cc in chunk_list]
        base = [0]
        for n in nblks:
            base.append(base[-1] + n)

        # global-slot deadlines: q(t) needed at its chunk's first slot;
        # k/v(t) first consumed when the score wave reaches keys t (slot
        # 4*(t%4) of the earliest chunk with c >= t%4 in list order)
        Q_DEAD = {1: base[1], 2: base[2], 3: base[3],
                  4: base[6], 5: base[4], 6: base[5],
                  7: base[5] + 4}  # q(t7) before the (1,3) pre-scoring
        KV_DEAD = {1: base[1] + 4, 2: base[2] + 8, 3: base[3] + 12,
                   4: base[4], 5: base[4] + 4, 6: base[5] + 8,
                   7: base[7] + 12}

        # v(t7) reserved two slots past k(t7): it pads the endgame's
        # diagonal-score bank rotation and is forced before pv(12) uses it
        V_DEAD = dict(KV_DEAD)
        V_DEAD[7] = base[7] + 14

        def queue_half_units(t, units6):
            queue_unit(Q_DEAD[t], units6[0:2])    # q a/b
            queue_unit(KV_DEAD[t], units6[2:4])   # k a/b
            queue_unit(V_DEAD[t], units6[4:6])    # v a/b

        def queue_pair_units(t0, units):
            queue_half_units(t0, units[:6])
            queue_half_units(t0 + 1, units[6:])

        queue_half_units(1, units0[6:])  # prologue pair: hf=1 only

        # pair creation: issue xT DMAs early — critically, ALL xt trains
        # must enter the in-order SP queue before any mid-kernel out-DMA
        # can park it (an out-DMA whose copy isn't ready blocks the queue
        # for many us, which starved later projections via late xts)
        pair_create = {1: [2], 2: [4, 6]}

        # pt tiles on demand so a later chunk's score wave can start while
        # an earlier chunk is still draining (pre-scoring)
        pts = {}

        def get_pt(b_, c_):
            if (b_, c_) not in pts:
                pts[(b_, c_)] = pt_pool.tile(
                    [128, NKB, 2, CQ], BF16, tag="pt", name=f"pt_{b_}_{c_}")
            return pts[(b_, c_)]

        def emit_scores_g(b_, c_, kb):
            pt = get_pt(b_, c_)
            tb_ = b_ * T
            tq0_ = c_ * CQ
            f0 = max(0, 128 * (kb - 4 * c_))
            sc = sc_pool.tile([128, 2, CQ], F32, tag="sc",
                              name=f"sc_{b_}_{c_}_{kb}")
            tk0 = kb * 128
            for h in range(2):
                hs = slice(h * 64, (h + 1) * 64)
                nc.tensor.matmul(
                    sc[:, h, f0:CQ],
                    kt_sb[hs, tb_ + tk0:tb_ + tk0 + 128],
                    qt_sb[hs, tb_ + tq0_ + f0:tb_ + tq0_ + CQ],
                    start=True, stop=True)
            nc.scalar.activation(
                pt[:, kb, :, f0:CQ], sc[:, :, f0:CQ],
                AF.Exp, scale=float(D) ** -0.5)
            if kb - 4 * c_ >= 0:  # diagonal block: causal triangle mask
                for h in range(2):
                    # keep where tq >= tk (f - p >= 0), else 0
                    nc.gpsimd.affine_select(
                        out=pt[:, kb, h, f0:f0 + 128],
                        in_=pt[:, kb, h, f0:f0 + 128],
                        compare_op=mybir.AluOpType.is_ge,
                        fill=0.0, base=0,
                        pattern=[[1, 128]], channel_multiplier=-1)
            return f0

        NPRE = 8   # (1,3) blocks pre-scored during (1,2)+(1,0)

        for ci, (b, c) in enumerate(chunk_list):
            for t0 in pair_create.get(ci, []):
                units, _ = proj_pair_units(t0)
                queue_pair_units(t0, units)

            tb = b * T
            tq0 = c * CQ
            nblk = 4 * (c + 1)
            pt = get_pt(b, c)
            # per-chunk O|Z accumulators [tq, gsub, h, d|Z]: pool rotation
            # (bufs=1) orders the next chunk's first PV write after this
            # chunk's tail reads
            o_ps = [acc_pool.tile([128, 2, 2, 65], F32, tag=f"o{i}",
                                  name=f"o_ps{i}_{b}_{c}")
                    for i in range(2)]
            zr_tiles = {}

            def make_tail_pieces(g, b=b, c=c, tb=tb, tq0=tq0, o_ps=o_ps):
                """Tail split into 3 pieces so the PE->DVE->PE->DVE chain of
                one tail interleaves with its pair partner + proj filler
                instead of stalling the in-order PE stream. In the final
                chunk the Act engine (done with exps by tail time) takes
                half the copies so DVE isn't the serial drain resource."""
                op = o_ps[g // 2]
                gs = g % 2
                act_assist = (b, c) == chunk_list[-1] and g >= 2
                state = {}

                def piece_a():   # normalize + transpose (PE 53ns)
                    if act_assist:
                        # 1/Z lands in SBUF so the Act engine can use it as
                        # an activation scale (scale APs must be SBUF)
                        if gs == 0:
                            zr = osb_pool.tile([128, 2, 2], F32, tag="zr",
                                               name=f"zr_{b}_{c}_{g}")
                            nc.vector.reciprocal(zr[:], op[:, :, :, 64])
                            zr_tiles[g // 2] = zr
                        zr = zr_tiles[g // 2]
                    elif gs == 0:
                        # 1/Z for the group pair, in place in PSUM col 64
                        # (both chains have stopped by emission time)
                        nc.vector.reciprocal(op[:, :, :, 64],
                                             op[:, :, :, 64])
                    o_sb = osb_pool.tile([128, 128], BF16, tag="osb",
                                         name=f"osb_{b}_{c}_{g}")
                    for h in range(2):
                        if act_assist and h == 1:
                            nc.scalar.activation(
                                o_sb[:, 64:128], op[:, gs, 1, 0:64],
                                AF.Copy, scale=zr[:, gs, 1:2])
                        elif act_assist:
                            nc.vector.tensor_scalar_mul(
                                o_sb[:, h * 64:(h + 1) * 64],
                                op[:, gs, h, 0:64],
                                zr[:, gs, h:h + 1])
                        else:
                            nc.vector.tensor_scalar_mul(
                                o_sb[:, h * 64:(h + 1) * 64],
                                op[:, gs, h, 0:64],
                                op[:, gs, h, 64:65])
                    tp = mm_pool.tile([128, 512], BF16, tag="mm",
                                      name=f"tp_{b}_{c}_{g}")
                    nc.tensor.transpose(tp[:, 0:128], o_sb[:], eye_sb[:])
                    state["tp"] = tp

                def piece_b():   # O^T landing copy (no PE)
                    ot_sb = otsb_pool.tile([128, 128], BF16, tag="otsb",
                                           name=f"otsb_{b}_{c}_{g}")
                    if act_assist:
                        nc.scalar.copy(ot_sb[:], state["tp"][:, 0:128])
                    else:
                        nc.vector.tensor_copy(ot_sb[:], state["tp"][:, 0:128])
                    state["ot"] = ot_sb

                def piece_c():   # output projection + copies + DMA (PE 426)
                    out_sb = outsb_pool.tile([128, E], BF16, tag="outsb",
                                             name=f"outsb_{b}_{c}_{g}")
                    tqg = tb + tq0 + g * 128
                    for eh in range(2):
                        ops = mm_pool.tile([128, 512], F32, tag="mm",
                                           name=f"ops_{b}_{c}_{g}_{eh}")
                        nc.tensor.matmul(
                            ops[:], state["ot"][:],
                            wo_sb[:, eh * 512:(eh + 1) * 512],
                            start=True, stop=True)
                        if ((b == 0 and c == 0) or act_assist) and eh == 1:
                            # Act's light window (short chunks / endgame)
                            nc.scalar.copy(
                                out_sb[:, eh * 512:(eh + 1) * 512], ops[:])
                        else:
                            nc.vector.tensor_copy(
                                out_sb[:, eh * 512:(eh + 1) * 512], ops[:])
                        if act_assist:
                            # endgame: SP is idle — issue half-row DMAs the
                            # moment each copy lands to shorten the drain
                            nc.sync.dma_start(
                                out[tqg:tqg + 128,
                                    eh * 512:(eh + 1) * 512],
                                out_sb[:, eh * 512:(eh + 1) * 512])
                    if not act_assist:
                        flush_out_dma()
                        dma_pending.append(
                            (out[tqg:tqg + 128, :], out_sb[:]))

                return [(53.0, piece_a), (0.0, piece_b), (426.0, piece_c)]

            # PSUM has_written bits: a start=True matmul clears them for the
            # WHOLE bank, so only the first PV matmul per o_ps bank per chunk
            # may use start=True. Later chains' first matmuls (kb==0,
            # start=False) overwrite-where-bit-clear, then accumulate.
            bank_started = [False, False]

            def pv_block(kb, b=b, c=c, pt=pt, o_ps=o_ps,
                         bank_started=bank_started):
                j0 = max(0, kb - 4 * c)
                # diagonal block: group j0's stationary is the masked pt
                # sub-block — emit it LAST so the gpsimd mask only gates the
                # final chain-stop matmul, not the whole block
                gs_order = list(range(j0, NQB))
                if kb - 4 * c >= 0 and len(gs_order) > 1:
                    gs_order = gs_order[1:] + gs_order[:1]
                for g in gs_order:
                    for h in range(2):
                        st = not bank_started[g // 2]
                        bank_started[g // 2] = True
                        nc.tensor.matmul(
                            o_ps[g // 2][:, g % 2, h, :],
                            pt[:, kb, h, g * 128:(g + 1) * 128],
                            v_sb[:, b * NKB + kb, h],
                            start=st, stop=(kb == 4 * c + g),
                            skip_group_check=True)
                j = kb - 4 * c
                if j in (1, 3):  # group pair's chains complete
                    pa = make_tail_pieces(j - 1)
                    pb = make_tail_pieces(j)
                    # interleave A A' B B' now; defer the C (out-proj)
                    # pieces until the NEXT pair so piece_b's DVE copy has
                    # landed long before C's Ldweights needs it
                    inter = [pa[0]]
                    if pending_cs:
                        inter.append(pending_cs.pop(0))
                    inter.append(pb[0])
                    if pending_cs:
                        inter.append(pending_cs.pop(0))
                    inter += [pa[1], pb[1]]
                    tails_q.extend(inter)
                    pending_cs.extend([pa[2], pb[2]])
                return (NQB - j0) * 2 * 65

            def emit_scores(kb):
                return emit_scores_g(b, c, kb)

            def pace(act_ns, pe_ns, slot):
                # deficit-paced filler: keep the PE fed during Act-paced
                # stretches, spend queued proj/tail work exactly where the
                # per-block PE emission falls short of the exp pace.
                sched["deficit"] += act_ns - pe_ns
                sched["deficit"] = max(-2000.0,
                                       min(sched["deficit"], 8000.0))
                # lookahead spread: don't let deadline-bound units burst
                if proj_q and proj_q[0][0] <= slot + 3:
                    pop_proj_piece()
                    if held:
                        pop_proj_piece()
                # alternate tail/proj pops so tail latency chains overlap
                # real PE work instead of stalling the in-order PE stream
                prefer_tail = True
                while sched["deficit"] > 400 and (tails_q or held or proj_q):
                    if prefer_tail and tails_q and not held:
                        pe_ns2, fn = tails_q.popleft()
                        fn()
                        sched["deficit"] -= pe_ns2
                    elif held or proj_q:
                        pop_proj_piece()
                    else:
                        pe_ns2, fn = tails_q.popleft()
                        fn()
                        sched["deficit"] -= pe_ns2
                    prefer_tail = not prefer_tail
                if held:   # never end a slot mid-unit
                    pop_proj_piece()

            def chunk_prefix(kb):
                if kb == 1:
                    # PE meat between sc(0)/exp(0) and the exp-gated
                    # pv(0), then the prev chunk's tail pieces
                    if held or proj_q:
                        pop_proj_piece()
                    drain_tails()

            if ci < len(chunk_list) - 1:
                for kb in range(nblk):
                    # units whose tokens this slot consumes: emit them now
                    force_proj_upto(base[ci] + kb)
                    f0 = emit_scores(kb)
                    pv_cyc = 0
                    if kb >= 1:
                        chunk_prefix(kb)
                        if kb >= 2:
                            pv_cyc = pv_block(kb - 2)
                    if ci == 5 and kb >= nblk - 4:
                        # pre-score an off-diagonal (1,3) block right after
                        # the host scores so Act sees it ASAP: shifts Act
                        # load out of the Act-saturated endgame
                        emit_scores_g(1, 3, kb - (nblk - 4))
                    if ci == 6 and NPRE == 8:
                        # four more during the Act-light (1,0) chunk,
                        # emitted before the pace pops so Act sees them ASAP
                        emit_scores_g(1, 3, 4 + kb)
                    pace((2 * (CQ - f0) + 222) / 1.2,
                         (2 * (CQ - f0) + pv_cyc) * 0.4167, base[ci] + kb)
                pv_block(nblk - 2)
                pv_block(nblk - 1)
            else:
                # Final chunk, two-phase so the kernel does not end on an
                # Act-bound exp wave:
                # phase A pre-scores the 12 off-diagonal blocks (Act paced,
                # PE kept busy by the deadline-reserved proj filler);
                # phase B runs the diagonal scores + every PV wave + tails
                # with all exps already in flight or done.
                for kb in range(NPRE, 12):
                    force_proj_upto(base[ci] + kb)
                    emit_scores(kb)
                    if kb == NPRE + 1:
                        # prev chunk's tails must fully emit before phase
                        # B's pv(0) rotates into its o_ps banks
                        if held or proj_q:
                            pop_proj_piece()
                        drain_tails()
                    pace((2 * CQ + 222) / 1.2, 2 * CQ * 0.4167,
                         base[ci] + kb)
                force_proj_upto(base[ci] + 12)  # k(t7) ahead of the scores
                emit_scores(12)
                emit_scores(13)
                for kb in range(0, 4):
                    pv_block(kb)
                if held or proj_q:   # v(t7) a: pads exp(12)'s bank WAR
                    pop_proj_piece()
                emit_scores(14)
                for kb in range(4, 8):
                    pv_block(kb)
                if held or proj_q:   # v(t7) b: pads exp(13)'s bank WAR
                    pop_proj_piece()
                emit_scores(15)
                force_proj_upto(base[ci] + 14)  # v(t7) before pv(12)
                for kb in range(8, 14):
                    pv_block(kb)
                for _ in range(2):   # first tail pair (DVE-only) flows here
                    if tails_q:
                        pe_ns2, fn = tails_q.popleft()
                        fn()
                pv_block(14)
                for _ in range(2):
                    if tails_q:
                        pe_ns2, fn = tails_q.popleft()
                        fn()
                pv_block(15)

        drain_tails()
        while held or proj_q:
            pop_proj_piece()
        flush_out_dma()

    nc.compile()
    return nc


def _host_prep(x, Wq, Wk, Wv, Wo):
    bf = ml_dtypes.bfloat16
    xT = np.ascontiguousarray(
        np.asarray(x, dtype=np.float32).reshape(BT, E).T).astype(bf)

    def perm(w):
        # [E, 128] -> [128p, kc, 128d] flattened: w[kc*128+p, d] -> out[p, kc, d]
        return np.ascontiguousarray(
            w.reshape(KC, 128, 128).transpose(1, 0, 2).reshape(128, E)).astype(bf)

    Wq = np.asarray(Wq, dtype=np.float32)
    Wk = np.asarray(Wk, dtype=np.float32)
    Wv = np.asarray(Wv, dtype=np.float32)
    Wo = np.asarray(Wo, dtype=np.float32)

    in_maps = []
    for c in range(NCORE):
        sl = slice(c * 128, (c + 1) * 128)
        in_maps.append({
            "xT": xT,
            "wq": perm(Wq[:, sl]),
            "wk": perm(Wk[:, sl]),
            "wv": perm(Wv[:, sl]),
            "wo": np.ascontiguousarray(Wo[sl, :]).astype(bf),
        })
    return in_maps


def kernel(x, Wq, Wk, Wv, Wo, bo, _trace=False, _trace_kwargs=None):
    if "nc" not in _cache:
        _cache["nc"] = _build()
    nc = _cache["nc"]

    in_maps = _host_prep(x, Wq, Wk, Wv, Wo)
    kw = {}
    if _trace:
        kw = dict(trace=True, trace_cores=[0], **(_trace_kwargs or {}))
    res = run_bass_kernel_spmd(nc, in_maps, core_ids=list(range(NCORE)), **kw)
    _cache["last_result"] = res

    total = np.zeros((BT, E), dtype=np.float32)
    for r in res.results:
        total += np.asarray(r["out"], dtype=np.float32)
    total += np.asarray(bo, dtype=np.float32)[None, :]
    return total.reshape(B, T, E)



# revision 96
# speedup vs baseline: 1.0210x; 1.0014x over previous
"""Multi-head causal attention (B=2, T=2048, E=1024, H=16, D=64) on 8 trn2 cores.

Sharding: tensor-parallel over heads — core c owns heads {2c, 2c+1} (a 128-wide
slice of the hidden dim). Each core computes q/k/v projections for its heads
over the full sequence, causal attention, and a partial output projection
(contraction over its 128 rows of Wo). The host sums the 8 bf16 partials + bias.

v4 (128.0us, from the 150.5us v2), rebuilt around TimelineSim gap blame.
The engine floor is PE ~102us (proj 41 + scores 29 + PV 14.7 + out-proj
13.7 + transposes) with Act exp ~88us; everything else is scheduling:
 - In-place PSUM reciprocal of the Z column + bf16 o_sb/eye/tp (1 cyc/row
   transpose); PV groups emit mask-dependent-group last so the gpsimd
   affine_select triangle mask only gates the final chain-stop matmul.
 - Tails split into 3 pieces (normalize+transpose / O^T copy / out-proj+
   DMA), pair-interleaved, with the C pieces deferred one tail pair so
   every cross-engine hop has real PE work between emit and consume.
 - Proj units carry (deadline=global slot, pieces) and live in a sorted
   queue; deficit + lookahead pacing spends them where the exp pace
   outruns PE, preserving late-deadline units (q/k/v of t7) as endgame
   filler. The prologue hand-interleaves q/k/v per-kc at xT arrival rate
   with weight DMAs slotted between xts, q/k psum borrowed from the idle
   sc banks, and landing copies on the (idle) Act engine.
 - Chunk order (0,*), (1,1), (1,2), (1,0), (1,3): 8 of (1,3)'s off-diag
   score blocks are pre-scored during (1,2)/(1,0) (pt bufs=3) and its
   own wave is two-phase — off-diag scores first (Act-paced, filler-fed),
   then diag scores + all PV waves + tails with every exp in flight, so
   the kernel does not end on an Act-bound stretch. Endgame tails use
   Act-assisted copies (Act is exp-free by then) + immediate out DMAs;
   mid-kernel Act borrows copies only in (0,0)/(0,1)-adjacent windows
   where it is measurably idle. Tail pools run 6 bufs deep.
 - PV emission lags scores by 2 blocks; all xt DMA trains enter the
   in-order SP queue before any out-DMA can park it.

PSUM (8 banks): sc 2x[128,2,512] (4) + O|Z accumulators 2x[128,2,2,65] (2) +
mm [128,512] x2 (2, shared by proj / out-proj / transpose tiles).

Timing signal is concourse TimelineSim (no NTFF under this axon client).
"""

import numpy as np
import ml_dtypes
from collections import deque

import concourse.bass as bass
import concourse.tile as tile
from concourse import bacc, mybir
from concourse.bass_utils import run_bass_kernel_spmd
from concourse.masks import make_identity
from contextlib import ExitStack

B, T, E, H, D = 2, 2048, 1024, 16, 64
BT = B * T            # 4096 tokens total
NCORE = 8
KC = E // 128         # contraction chunks for projections = 8
CQ = 512              # tq chunk width
NQB = T // CQ         # tq chunks per batch = 4
NKB = T // 128        # tk blocks per batch = 16

F32 = mybir.dt.float32
BF16 = mybir.dt.bfloat16
AF = mybir.ActivationFunctionType

_cache = {}


def _build():
    nc = bacc.Bacc("TRN2", target_bir_lowering=False, debug=False,
                   num_devices=NCORE)

    xT = nc.dram_tensor("xT", [E, BT], BF16, kind="ExternalInput").ap()
    wq = nc.dram_tensor("wq", [128, E], BF16, kind="ExternalInput").ap()
    wk = nc.dram_tensor("wk", [128, E], BF16, kind="ExternalInput").ap()
    wv = nc.dram_tensor("wv", [128, E], BF16, kind="ExternalInput").ap()
    wo = nc.dram_tensor("wo", [128, E], BF16, kind="ExternalInput").ap()
    out = nc.dram_tensor("out", [BT, E], BF16, kind="ExternalOutput").ap()

    with tile.TileContext(nc) as tc, ExitStack() as ctx:
        pers = ctx.enter_context(tc.tile_pool(name="pers", bufs=1))

        wq_sb = pers.tile([128, KC, 128], BF16, tag="wq")
        wk_sb = pers.tile([128, KC, 128], BF16, tag="wk")
        wv_sb = pers.tile([128, KC, 128], BF16, tag="wv")
        wo_sb = pers.tile([128, E], BF16, tag="wo")
        eye_sb = pers.tile([128, 128], BF16, tag="eye")
        qt_sb = pers.tile([128, BT], BF16, tag="qt")    # [dims(2 heads), tok]
        kt_sb = pers.tile([128, BT], BF16, tag="kt")
        # V natural + ones col per head: [tok%128, blk, h, d|1]; the ones
        # column makes the flipped P^T-stationary PV matmul emit Z = sum(exp)
        # as output column 64 for free.
        v_sb = pers.tile([128, BT // 128, 2, 65], BF16, tag="v")

        # wq queued first on the sync HWDGE queue so the first projection
        # matmul gates on as little DMA as possible; each extra DMA costs
        # ~625ns of serial HWDGE hold, so weights go as single transfers
        # slotted between the xts that need them.
        wq_r = wq.rearrange("p (kc d) -> p kc d", kc=KC)
        nc.sync.dma_start(wq_sb[:, 0:KC // 2], wq_r[:, 0:KC // 2])
        nc.vector.memset(v_sb[:, :, :, 64:65], 1.0)
        make_identity(nc, eye_sb[:])

        # SBUF pools
        xts_pool = ctx.enter_context(tc.tile_pool(name="xts", bufs=32))
        pt_pool = ctx.enter_context(tc.tile_pool(name="pt", bufs=2))
        osb_pool = ctx.enter_context(tc.tile_pool(name="osb", bufs=6))
        otsb_pool = ctx.enter_context(tc.tile_pool(name="otsb", bufs=6))
        outsb_pool = ctx.enter_context(tc.tile_pool(name="outsb", bufs=6))

        # PSUM pools: 4 + 2 + 2 = 8 banks
        sc_pool = ctx.enter_context(tc.tile_pool(name="sc", bufs=2,
                                                 space="PSUM"))
        acc_pool = ctx.enter_context(tc.tile_pool(name="acc", bufs=1,
                                                  space="PSUM"))
        mm_pool = ctx.enter_context(tc.tile_pool(name="mm", bufs=2,
                                                 space="PSUM"))


        # ---- projection units -------------------------------------------
        def proj_pair_units(t0, dma_hooks=None):
            """t0: even 512-token chunk index (0..6). Issues the pair's xT
            DMAs now; returns 6 unit callbacks (q,k,v) x (hf 0,1).
            dma_hooks: {kc: callback} run right after that kc's xt DMA is
            queued (prologue interleaves weight DMAs at specific points)."""
            xts = []
            for kc in range(KC):
                xt = xts_pool.tile([128, 2 * CQ], BF16, tag="xt",
                                   name=f"xt_{t0}_{kc}")
                nc.sync.dma_start(
                    xt[:], xT[kc * 128:(kc + 1) * 128,
                              t0 * CQ:(t0 + 2) * CQ])
                if dma_hooks and kc in dma_hooks:
                    dma_hooks[kc]()
                xts.append(xt)

            def qk_unit(w_sb, dst_sb, hf):
                # two ~850ns halves so filler interleaves finely with waves
                t_ = t0 + hf
                state = {}
                def emit_a():
                    ps = mm_pool.tile([128, CQ], F32, tag="mm",
                                      name=f"qkps{t_}_{id(w_sb)}")
                    state["ps"] = ps
                    for kc in range(KC // 2):
                        nc.tensor.matmul(
                            ps[:], w_sb[:, kc],
                            xts[kc][:, hf * CQ:(hf + 1) * CQ],
                            start=(kc == 0), stop=False)
                def emit_b():
                    ps = state["ps"]
                    for kc in range(KC // 2, KC):
                        nc.tensor.matmul(
                            ps[:], w_sb[:, kc],
                            xts[kc][:, hf * CQ:(hf + 1) * CQ],
                            start=False, stop=(kc == KC - 1))
                    nc.vector.tensor_copy(
                        dst_sb[:, t_ * CQ:(t_ + 1) * CQ], ps[:])
                return [emit_a, emit_b]

            def v_unit(hf):
                t_ = t0 + hf
                state = {}
                def emit_a():
                    v_ps = mm_pool.tile([128, CQ], F32, tag="mm",
                                        name=f"vps{t_}")
                    state["ps"] = v_ps
                    for j in (0, 1):
                        jf = hf * CQ + j * 128
                        for kc in range(KC):
                            nc.tensor.matmul(
                                v_ps[:, j * 128:(j + 1) * 128],
                                xts[kc][:, jf:jf + 128],
                                wv_sb[:, kc], start=(kc == 0),
                                stop=(kc == KC - 1))
                def emit_b():
                    v_ps = state["ps"]
                    for j in (2, 3):
                        jf = hf * CQ + j * 128
                        for kc in range(KC):
                            nc.tensor.matmul(
                                v_ps[:, j * 128:(j + 1) * 128],
                                xts[kc][:, jf:jf + 128],
                                wv_sb[:, kc], start=(kc == 0),
                                stop=(kc == KC - 1))
                    b4 = t_ * (CQ // 128)
                    nc.vector.tensor_copy(
                        v_sb[:, b4:b4 + 4, :, 0:64],
                        v_ps[:].rearrange("p (j h v) -> p j h v",
                                          j=4, h=2))
                return [emit_a, emit_b]

            units = (qk_unit(wq_sb, qt_sb, 0) + qk_unit(wk_sb, kt_sb, 0) +
                     v_unit(0) + qk_unit(wq_sb, qt_sb, 1) +
                     qk_unit(wk_sb, kt_sb, 1) + v_unit(1))
            return units, xts

        # ---- filler machinery -------------------------------------------
        # proj_q entries are (deadline_slot, seq, [piece_a, piece_b]): the
        # unit MUST be emitted before the global attention slot that
        # consumes its tokens (a later emission would deadlock the in-order
        # PE queue). Kept sorted by deadline so deficit-paced pops
        # naturally preserve the latest-deadline units as an endgame
        # reserve.
        tails_q = deque()
        pending_cs = []    # tail C (out-proj) pieces deferred one pair
        proj_q = []
        held = []          # pending b-half of a split proj unit (must pop
                           # before any other mm-pool user)
        sched = {"deficit": 0.0, "seq": 0}
        dma_pending = []   # (dram_slice, sbuf_tile): out DMAs deferred one
                           # tail so the SP queue never stalls on copy sems

        PROJ_NS = 853.0    # PE ns per proj half-piece (4 matmuls x 512)

        def flush_out_dma():
            while dma_pending:
                dst, src = dma_pending.pop(0)
                nc.sync.dma_start(dst, src)

        def queue_unit(dead, pieces):
            proj_q.append((dead, sched["seq"], pieces))
            sched["seq"] += 1
            proj_q.sort(key=lambda e: (e[0], e[1]))

        def pop_proj_piece():
            if held:
                held.pop()()
            else:
                _, _, pieces = proj_q.pop(0)
                pieces[0]()
                held.append(pieces[1])
            sched["deficit"] -= PROJ_NS

        def drain_tails():
            # interleave a proj piece between tail pieces so their
            # cross-engine latency chains overlap real PE work
            tails_q.extend(pending_cs)
            pending_cs.clear()
            while tails_q:
                pe_ns, fn = tails_q.popleft()
                fn()
                sched["deficit"] -= pe_ns
                if tails_q and (held or proj_q) and sched["deficit"] > -800:
                    pop_proj_piece()

        def force_proj_upto(slot):
            while held or (proj_q and proj_q[0][0] <= slot):
                pop_proj_piece()

        # ---- prologue ----------------------------------------------------
        # Weight DMAs slot between the pair-0 xT DMAs (wk after xt0, wv
        # after xt2 — each lands just before its first consumer) and
        # q/k/v matmuls interleave per-kc at xT arrival granularity so the
        # PE streams at DMA rate with no burst stalls.
        hooks = {
            0: lambda: (nc.sync.dma_start(wq_sb[:, KC // 2:KC],
                                          wq_r[:, KC // 2:KC]),
                        nc.sync.dma_start(
                wk_sb[:], wk.rearrange("p (kc d) -> p kc d", kc=KC))),
            2: lambda: nc.sync.dma_start(
                wv_sb[:], wv.rearrange("p (kc d) -> p kc d", kc=KC)),
            7: lambda: nc.sync.dma_start(wo_sb[:], wo[:]),
        }
        units0, xts0 = proj_pair_units(0, dma_hooks=hooks)

        # tokens 0..511: q/k psum tiles borrow the (still idle) sc tag's
        # banks so mm_pool stays free for the interleaved v chains.
        q_ps0 = sc_pool.tile([128, CQ], F32, tag="sc", name="qps_pro")
        k_ps0 = sc_pool.tile([128, CQ], F32, tag="sc", name="kps_pro")
        v_ps0 = mm_pool.tile([128, CQ], F32, tag="mm", name="vps_pro")
        v_started_cell = [False]

        def pro_v(kc):
            # interleaved per-j chains on one bank: only the very first
            # matmul clears the bank's has_written bits (start=True); the
            # other chains' kc==0 matmuls overwrite-where-bit-clear
            for j in range(4):
                nc.tensor.matmul(
                    v_ps0[:, j * 128:(j + 1) * 128],
                    xts0[kc][:, j * 128:(j + 1) * 128],
                    wv_sb[:, kc], start=not v_started_cell[0],
                    stop=(kc == KC - 1), skip_group_check=True)
                v_started_cell[0] = True
        for kc in range(KC):
            nc.tensor.matmul(q_ps0[:], wq_sb[:, kc], xts0[kc][:, 0:CQ],
                             start=(kc == 0), stop=(kc == KC - 1),
                             skip_group_check=True)
            nc.tensor.matmul(k_ps0[:], wk_sb[:, kc], xts0[kc][:, 0:CQ],
                             start=(kc == 0), stop=(kc == KC - 1),
                             skip_group_check=True)
            if kc >= 2:
                pro_v(kc - 2)
        # Act is idle until the first exp (~12us): give it the prologue
        # landing copies so DVE stays clear for the hf=1 unit copies
        nc.scalar.copy(qt_sb[:, 0:CQ], q_ps0[:])
        nc.scalar.copy(kt_sb[:, 0:CQ], k_ps0[:])
        for kc in range(KC - 2, KC):
            pro_v(kc)
        nc.scalar.copy(
            v_sb[:, 0:4, :, 0:64],
            v_ps0[:].rearrange("p (j h v) -> p j h v", j=4, h=2))

        # Chunk order: batch-1 runs [c1, c2, c0, c3] so the kernel ends on
        # the 16-block (1,3) chunk, whose late score blocks (kb>=12) keep
        # q/k/v(t7) units as deadline-reserved PE filler for the Act-paced
        # endgame, instead of draining tails against an empty proj queue.
        chunk_list = [(0, 0), (0, 1), (0, 2), (0, 3),
                      (1, 1), (1, 2), (1, 0), (1, 3)]
        nblks = [4 * (cc + 1) for _, cc in chunk_list]
        base = [0]
        for n in nblks:
            base.append(base[-1] + n)

        # global-slot deadlines: q(t) needed at its chunk's first slot;
        # k/v(t) first consumed when the score wave reaches keys t (slot
        # 4*(t%4) of the earliest chunk with c >= t%4 in list order)
        Q_DEAD = {1: base[1], 2: base[2], 3: base[3],
                  4: base[6], 5: base[4], 6: base[5],
                  7: base[5] + 4}  # q(t7) before the (1,3) pre-scoring
        KV_DEAD = {1: base[1] + 4, 2: base[2] + 8, 3: base[3] + 12,
                   4: base[4], 5: base[4] + 4, 6: base[5] + 8,
                   7: base[7] + 12}

        # v(t7) reserved two slots past k(t7): it pads the endgame's
        # diagonal-score bank rotation and is forced before pv(12) uses it
        V_DEAD = dict(KV_DEAD)
        V_DEAD[7] = base[7] + 14

        def queue_half_units(t, units6):
            queue_unit(Q_DEAD[t], units6[0:2])    # q a/b
            queue_unit(KV_DEAD[t], units6[2:4])   # k a/b
            queue_unit(V_DEAD[t], units6[4:6])    # v a/b

        def queue_pair_units(t0, units):
            queue_half_units(t0, units[:6])
            queue_half_units(t0 + 1, units[6:])

        queue_half_units(1, units0[6:])  # prologue pair: hf=1 only

        # pair creation: issue xT DMAs early — critically, ALL xt trains
        # must enter the in-order SP queue before any mid-kernel out-DMA
        # can park it (an out-DMA whose copy isn't ready blocks the queue
        # for many us, which starved later projections via late xts)
        pair_create = {1: [2], 2: [4, 6]}

        # pt tiles on demand so a later chunk's score wave can start while
        # an earlier chunk is still draining (pre-scoring)
        pts = {}

        def get_pt(b_, c_):
            if (b_, c_) not in pts:
                pts[(b_, c_)] = pt_pool.tile(
                    [128, NKB, 2, CQ], BF16, tag="pt", name=f"pt_{b_}_{c_}")
            return pts[(b_, c_)]

        def emit_scores_g(b_, c_, kb):
            pt = get_pt(b_, c_)
            tb_ = b_ * T
            tq0_ = c_ * CQ
            f0 = max(0, 128 * (kb - 4 * c_))
            sc = sc_pool.tile([128, 2, CQ], F32, tag="sc",
                              name=f"sc_{b_}_{c_}_{kb}")
            tk0 = kb * 128
            for h in range(2):
                hs = slice(h * 64, (h + 1) * 64)
                nc.tensor.matmul(
                    sc[:, h, f0:CQ],
                    kt_sb[hs, tb_ + tk0:tb_ + tk0 + 128],
                    qt_sb[hs, tb_ + tq0_ + f0:tb_ + tq0_ + CQ],
                    start=True, stop=True)
            nc.scalar.activation(
                pt[:, kb, :, f0:CQ], sc[:, :, f0:CQ],
                AF.Exp, scale=float(D) ** -0.5)
            if kb - 4 * c_ >= 0:  # diagonal block: causal triangle mask
                for h in range(2):
                    # keep where tq >= tk (f - p >= 0), else 0
                    nc.gpsimd.affine_select(
                        out=pt[:, kb, h, f0:f0 + 128],
                        in_=pt[:, kb, h, f0:f0 + 128],
                        compare_op=mybir.AluOpType.is_ge,
                        fill=0.0, base=0,
                        pattern=[[1, 128]], channel_multiplier=-1)
            return f0

        NPRE = 8   # (1,3) blocks pre-scored during (1,2)+(1,0)

        for ci, (b, c) in enumerate(chunk_list):
            for t0 in pair_create.get(ci, []):
                units, _ = proj_pair_units(t0)
                queue_pair_units(t0, units)

            tb = b * T
            tq0 = c * CQ
            nblk = 4 * (c + 1)
            pt = get_pt(b, c)
            # per-chunk O|Z accumulators [tq, gsub, h, d|Z]: pool rotation
            # (bufs=1) orders the next chunk's first PV write after this
            # chunk's tail reads
            o_ps = [acc_pool.tile([128, 2, 2, 65], F32, tag=f"o{i}",
                                  name=f"o_ps{i}_{b}_{c}")
                    for i in range(2)]
            zr_tiles = {}

            def make_tail_pieces(g, b=b, c=c, tb=tb, tq0=tq0, o_ps=o_ps):
                """Tail split into 3 pieces so the PE->DVE->PE->DVE chain of
                one tail interleaves with its pair partner + proj filler
                instead of stalling the in-order PE stream. In the final
                chunk the Act engine (done with exps by tail time) takes
                half the copies so DVE isn't the serial drain resource."""
                op = o_ps[g // 2]
                gs = g % 2
                act_assist = (b, c) == chunk_list[-1] and g >= 2
                state = {}

                def piece_a():   # normalize + transpose (PE 53ns)
                    if act_assist:
                        # 1/Z lands in SBUF so the Act engine can use it as
                        # an activation scale (scale APs must be SBUF)
                        if gs == 0:
                            zr = osb_pool.tile([128, 2, 2], F32, tag="zr",
                                               name=f"zr_{b}_{c}_{g}")
                            nc.vector.reciprocal(zr[:], op[:, :, :, 64])
                            zr_tiles[g // 2] = zr
                        zr = zr_tiles[g // 2]
                    elif gs == 0:
                        # 1/Z for the group pair, in place in PSUM col 64
                        # (both chains have stopped by emission time)
                        nc.vector.reciprocal(op[:, :, :, 64],
                                             op[:, :, :, 64])
                    o_sb = osb_pool.tile([128, 128], BF16, tag="osb",
                                         name=f"osb_{b}_{c}_{g}")
                    for h in range(2):
                        if act_assist and h == 1:
                            nc.scalar.activation(
                                o_sb[:, 64:128], op[:, gs, 1, 0:64],
                                AF.Copy, scale=zr[:, gs, 1:2])
                        elif act_assist:
                            nc.vector.tensor_scalar_mul(
                                o_sb[:, h * 64:(h + 1) * 64],
                                op[:, gs, h, 0:64],
                                zr[:, gs, h:h + 1])
                        else:
                            nc.vector.tensor_scalar_mul(
                                o_sb[:, h * 64:(h + 1) * 64],
                                op[:, gs, h, 0:64],
                                op[:, gs, h, 64:65])
                    tp = mm_pool.tile([128, 512], BF16, tag="mm",
                                      name=f"tp_{b}_{c}_{g}")
                    nc.tensor.transpose(tp[:, 0:128], o_sb[:], eye_sb[:])
                    state["tp"] = tp

                def piece_b():   # O^T landing copy (no PE)
                    ot_sb = otsb_pool.tile([128, 128], BF16, tag="otsb",
                                           name=f"otsb_{b}_{c}_{g}")
                    if act_assist:
                        nc.scalar.copy(ot_sb[:], state["tp"][:, 0:128])
                    else:
                        nc.vector.tensor_copy(ot_sb[:], state["tp"][:, 0:128])
                    state["ot"] = ot_sb

                def piece_c():   # output projection + copies + DMA (PE 426)
                    out_sb = outsb_pool.tile([128, E], BF16, tag="outsb",
                                             name=f"outsb_{b}_{c}_{g}")
                    tqg = tb + tq0 + g * 128
                    for eh in range(2):
                        ops = mm_pool.tile([128, 512], F32, tag="mm",
                                           name=f"ops_{b}_{c}_{g}_{eh}")
                        nc.tensor.matmul(
                            ops[:], state["ot"][:],
                            wo_sb[:, eh * 512:(eh + 1) * 512],
                            start=True, stop=True)
                        if ((b == 0 and c == 0) or act_assist) and eh == 1:
                            # Act's light window (short chunks / endgame)
                            nc.scalar.copy(
                                out_sb[:, eh * 512:(eh + 1) * 512], ops[:])
                        else:
                            nc.vector.tensor_copy(
                                out_sb[:, eh * 512:(eh + 1) * 512], ops[:])
                        if act_assist:
                            # endgame: SP is idle — issue half-row DMAs the
                            # moment each copy lands to shorten the drain
                            nc.sync.dma_start(
                                out[tqg:tqg + 128,
                                    eh * 512:(eh + 1) * 512],
                                out_sb[:, eh * 512:(eh + 1) * 512])
                    if not act_assist:
                        flush_out_dma()
                        dma_pending.append(
                            (out[tqg:tqg + 128, :], out_sb[:]))

                return [(53.0, piece_a), (0.0, piece_b), (426.0, piece_c)]

            # PSUM has_written bits: a start=True matmul clears them for the
            # WHOLE bank, so only the first PV matmul per o_ps bank per chunk
            # may use start=True. Later chains' first matmuls (kb==0,
            # start=False) overwrite-where-bit-clear, then accumulate.
            bank_started = [False, False]

            def pv_block(kb, b=b, c=c, pt=pt, o_ps=o_ps,
                         bank_started=bank_started):
                j0 = max(0, kb - 4 * c)
                # diagonal block: group j0's stationary is the masked pt
                # sub-block — emit it LAST so the gpsimd mask only gates the
                # final chain-stop matmul, not the whole block
                gs_order = list(range(j0, NQB))
                if kb - 4 * c >= 0 and len(gs_order) > 1:
                    gs_order = gs_order[1:] + gs_order[:1]
                for g in gs_order:
                    for h in range(2):
                        st = not bank_started[g // 2]
                        bank_started[g // 2] = True
                        nc.tensor.matmul(
                            o_ps[g // 2][:, g % 2, h, :],
                            pt[:, kb, h, g * 128:(g + 1) * 128],
                            v_sb[:, b * NKB + kb, h],
                            start=st, stop=(kb == 4 * c + g),
                            skip_group_check=True)
                j = kb - 4 * c
                if j in (1, 3):  # group pair's chains complete
                    pa = make_tail_pieces(j - 1)
                    pb = make_tail_pieces(j)
                    # interleave A A' B B' now; defer the C (out-proj)
                    # pieces until the NEXT pair so piece_b's DVE copy has
                    # landed long before C's Ldweights needs it
                    inter = [pa[0]]
                    if pending_cs:
                        inter.append(pending_cs.pop(0))
                    inter.append(pb[0])
                    if pending_cs:
                        inter.append(pending_cs.pop(0))
                    inter += [pa[1], pb[1]]
                    tails_q.extend(inter)
                    pending_cs.extend([pa[2], pb[2]])
                return (NQB - j0) * 2 * 65

            def emit_scores(kb):
                return emit_scores_g(b, c, kb)

            def pace(act_ns, pe_ns, slot):
                # deficit-paced filler: keep the PE fed during Act-paced
                # stretches, spend queued proj/tail work exactly where the
                # per-block PE emission falls short of the exp pace.
                sched["deficit"] += act_ns - pe_ns
                sched["deficit"] = max(-2000.0,
                                       min(sched["deficit"], 8000.0))
                # lookahead spread: don't let deadline-bound units burst
                if proj_q and proj_q[0][0] <= slot + 3:
                    pop_proj_piece()
                    if held:
                        pop_proj_piece()
                # alternate tail/proj pops so tail latency chains overlap
                # real PE work instead of stalling the in-order PE stream
                prefer_tail = True
                while sched["deficit"] > 400 and (tails_q or held or proj_q):
                    if prefer_tail and tails_q and not held:
                        pe_ns2, fn = tails_q.popleft()
                        fn()
                        sched["deficit"] -= pe_ns2
                    elif held or proj_q:
                        pop_proj_piece()
                    else:
                        pe_ns2, fn = tails_q.popleft()
                        fn()
                        sched["deficit"] -= pe_ns2
                    prefer_tail = not prefer_tail
                if held:   # never end a slot mid-unit
                    pop_proj_piece()

            def chunk_prefix(kb):
                if kb == 1:
                    # PE meat between sc(0)/exp(0) and the exp-gated
                    # pv(0), then the prev chunk's tail pieces
                    if held or proj_q:
                        pop_proj_piece()
                    drain_tails()

            if ci < len(chunk_list) - 1:
                for kb in range(nblk):
                    # units whose tokens this slot consumes: emit them now
                    force_proj_upto(base[ci] + kb)
                    f0 = emit_scores(kb)
                    pv_cyc = 0
                    if kb >= 2:
                        # drain the prev chunk's tails one slot later (just
                        # before pv(0)) so their deps have resolved
                        chunk_prefix(kb - 1)
                        pv_cyc = pv_block(kb - 2)
                    if ci == 5 and kb >= nblk - 4:
                        # pre-score an off-diagonal (1,3) block right after
                        # the host scores so Act sees it ASAP: shifts Act
                        # load out of the Act-saturated endgame
                        emit_scores_g(1, 3, kb - (nblk - 4))
                    if ci == 6 and NPRE == 8:
                        # four more during the Act-light (1,0) chunk,
                        # emitted before the pace pops so Act sees them ASAP
                        emit_scores_g(1, 3, 4 + kb)
                    pace((2 * (CQ - f0) + 222) / 1.2,
                         (2 * (CQ - f0) + pv_cyc) * 0.4167, base[ci] + kb)
                pv_block(nblk - 2)
                pv_block(nblk - 1)
            else:
                # Final chunk, two-phase so the kernel does not end on an
                # Act-bound exp wave:
                # phase A pre-scores the 12 off-diagonal blocks (Act paced,
                # PE kept busy by the deadline-reserved proj filler);
                # phase B runs the diagonal scores + every PV wave + tails
                # with all exps already in flight or done.
                for kb in range(NPRE, 12):
                    force_proj_upto(base[ci] + kb)
                    emit_scores(kb)
                    if kb == NPRE + 1:
                        # prev chunk's tails must fully emit before phase
                        # B's pv(0) rotates into its o_ps banks
                        if held or proj_q:
                            pop_proj_piece()
                        drain_tails()
                    pace((2 * CQ + 222) / 1.2, 2 * CQ * 0.4167,
                         base[ci] + kb)
                force_proj_upto(base[ci] + 12)  # k(t7) ahead of the scores
                emit_scores(12)
                emit_scores(13)
                for kb in range(0, 4):
                    pv_block(kb)
                if held or proj_q:   # v(t7) a: pads exp(12)'s bank WAR
                    pop_proj_piece()
                emit_scores(14)
                for kb in range(4, 8):
                    pv_block(kb)
                if held or proj_q:   # v(t7) b: pads exp(13)'s bank WAR
                    pop_proj_piece()
                emit_scores(15)
                force_proj_upto(base[ci] + 14)  # v(t7) before pv(12)
                for kb in range(8, 14):
                    pv_block(kb)
                for _ in range(2):   # first tail pair (DVE-only) flows here
                    if tails_q:
                        pe_ns2, fn = tails_q.popleft()
                        fn()
                pv_block(14)
                for _ in range(2):
                    if tails_q:
                        pe_ns2, fn = tails_q.popleft()
                        fn()
                pv_block(15)

        drain_tails()
        while held or proj_q:
            pop_proj_piece()
        flush_out_dma()

    nc.compile()
    return nc


def _host_prep(x, Wq, Wk, Wv, Wo):
    bf = ml_dtypes.bfloat16
    xT = np.ascontiguousarray(
        np.asarray(x, dtype=np.float32).reshape(BT, E).T).astype(bf)

    def perm(w):
        # [E, 128] -> [128p, kc, 128d] flattened: w[kc*128+p, d] -> out[p, kc, d]
        return np.ascontiguousarray(
            w.reshape(KC, 128, 128).transpose(1, 0, 2).reshape(128, E)).astype(bf)

    Wq = np.asarray(Wq, dtype=np.float32)
    Wk = np.asarray(Wk, dtype=np.float32)
    Wv = np.asarray(Wv, dtype=np.float32)
    Wo = np.asarray(Wo, dtype=np.float32)

    in_maps = []
    for c in range(NCORE):
        sl = slice(c * 128, (c + 1) * 128)
        in_maps.append({
            "xT": xT,
            "wq": perm(Wq[:, sl]),
            "wk": perm(Wk[:, sl]),
            "wv": perm(Wv[:, sl]),
            "wo": np.ascontiguousarray(Wo[sl, :]).astype(bf),
        })
    return in_maps


def kernel(x, Wq, Wk, Wv, Wo, bo, _trace=False, _trace_kwargs=None):
    if "nc" not in _cache:
        _cache["nc"] = _build()
    nc = _cache["nc"]

    in_maps = _host_prep(x, Wq, Wk, Wv, Wo)
    kw = {}
    if _trace:
        kw = dict(trace=True, trace_cores=[0], **(_trace_kwargs or {}))
    res = run_bass_kernel_spmd(nc, in_maps, core_ids=list(range(NCORE)), **kw)
    _cache["last_result"] = res

    total = np.zeros((BT, E), dtype=np.float32)
    for r in res.results:
        total += np.asarray(r["out"], dtype=np.float32)
    total += np.asarray(bo, dtype=np.float32)[None, :]
    return total.reshape(B, T, E)

